# revision 1
# baseline (speedup 1.0000x reference)
"""Trainium2 Bass kernel for nn_Beta_cVAE (cVAE encoder/decoder + 2-layer QP
trajectory projection, 20 ADMM iterations).

Strategy: pure data-parallel over 8 NeuronCores (128 batch rows each).
On-device layout is fully transposed: batch on the free dim, time/NVAR on
partitions, so every matmul uses weights/basis as the stationary operand and
no transposes appear anywhere.  All arctan2/cos/sin of the reference are
eliminated algebraically (cos(atan2(a,b)) = b/hypot(a,b)): the obstacle
ellipse projection becomes a residual scaling  resid = w*min(r-AB,0)/r  that
is *exactly zero* for inactive obstacles (no large-term cancellation, which
makes the bf16 obstacle pipeline safe), velocity/accel projections become
clip-scalings, and the lane term reduces to y - clip(y, lb, ub).  Obstacle
sums over the 10 obstacles fold into PE matmul accumulations with a shared
stationary operand.  MLP runs in bf16 (fp32 PSUM accumulation); forward basis
matmuls run as float32r (1 cycle/row at N=256 vs 4 for plain fp32).
"""
import sys
import os

sys.path.insert(0, "/opt/trn_rl_repo")

import numpy as np
import ml_dtypes

import concourse.bass as bass
import concourse.mybir as mybir
from concourse.tile import TileContext, ScopedClock
from concourse.bass_utils import run_bass_kernel_spmd

f32 = np.float32
bf16 = ml_dtypes.bfloat16
FP = mybir.dt.float32
FR = mybir.dt.float32r
BF = mybir.dt.bfloat16
AF = mybir.ActivationFunctionType
OP = mybir.AluOpType

NCORES = 8
BPC = 128          # batch rows per core
NUM, NVAR, NOBS = 100, 11, 10
NB = NOBS * BPC    # 1280
A_OBS, B_OBS = 8.0, 4.2
AB = A_OBS * B_OBS
RO, RI, RL, RP = 100.0, 100.0, 100.0, 1.0
VMIN, VMAX, AMAX = 0.1, 30.0, 8.0
MAXITER = 20
HID = 1024


class _TC(TileContext):
    """TileContext whose tail drain splits sem waits: this walrus build's
    TPB_CTRL codegen accepts only one sync wait per instruction."""

    def _drain_and_barrier(self, tick_clock, wait_clock):
        drain_inst = self.nc.sync.drain()
        wait_clock.add_sem_waits(
            drain_inst.ins, ScopedClock({None: tick_clock.global_clock})
        )
        si = drain_inst.ins.sync_info
        waits = list(si.on_wait) if si is not None else []
        if len(waits) > 1:
            drain_inst.ins.sync_info = mybir.SyncInfo(
                on_wait=waits[:1], on_update=list(si.on_update)
            )
            for w in waits[1:]:
                nop = self.nc.sync.nop(nofuse=True, hint="split_drain_wait")
                nop.ins.sync_info = mybir.SyncInfo(on_wait=[w], on_update=[])

        self.nc.all_engine_barrier()
        assert self.sems is not None
        popped = self.nc._tile_sem_poison_stack.pop()
        assert popped is self._sem_poison
        self.nc.clear_and_free_semaphores(list(self.sems.allocated().values()))
        self.nc.all_engine_barrier()


def _split_sync_waits(nc, max_waits=1):
    """Walrus's CoreV3 codegen accepts few sync waits per instruction; hoist
    surplus waits onto same-engine nops inserted just before the offender."""
    ctr = [0]
    for f in nc.m.functions:
        for bb in f.blocks:
            insts = list(bb.instructions)
            out = []
            for inst in insts:
                si = getattr(inst, "sync_info", None)
                if si is not None and len(si.on_wait) > max_waits:
                    waits = list(si.on_wait)
                    keep = waits[-max_waits:]
                    rest = waits[: len(waits) - max_waits]
                    for i in range(0, len(rest), max_waits):
                        ctr[0] += 1
                        nop = mybir.InstNoOp(
                            name=f"swsplit_{ctr[0]}",
                            engine=inst.engine,
                            bass_nofuse=True,
                            sync_info=mybir.SyncInfo(
                                on_wait=rest[i : i + max_waits], on_update=[]
                            ),
                        )
                        out.append(nop)
                    inst.sync_info = mybir.SyncInfo(
                        on_wait=keep, on_update=list(si.on_update)
                    )
                out.append(inst)
            bb.instructions = out


def _r(ap):
    """fp32 AP reinterpreted as float32r for 1-cycle/row matmuls."""
    return ap.bitcast(FR)


# ---------------------------------------------------------------------------
# module construction
# ---------------------------------------------------------------------------

def build_module(maxiter=MAXITER, split_waits=True):
    nc = bass.Bass()
    dp = nc.declare_dram_parameter

    # ---- per-core inputs ----
    enc_in = dp("enc_in", [255, BPC], BF, isOutput=False)
    inp_n_t = dp("inp_n_t", [55, BPC], BF, isOutput=False)
    eps_t = dp("eps_t", [2, BPC], FP, isOutput=False)
    beq1xy = dp("beq1xy", [NVAR, 2 * BPC], FP, isOutput=False)
    beqCxy = dp("beqCxy", [NVAR, 2 * BPC], FP, isOutput=False)
    xyobs = dp("xyobs", [NUM, 2 * NB], BF, isOutput=False)     # {xo | yo}, (a,o,b)
    ylbc = dp("ylbc", [NUM, BPC], BF, isOutput=False)
    yubc = dp("yubc", [NUM, BPC], BF, isOutput=False)

    # ---- replicated weights (bf16) ----
    w1 = dp("w1", [255, HID], BF, isOutput=False)
    b1 = dp("b1", [HID], FP, isOutput=False)
    w2 = dp("w2", [HID, HID], BF, isOutput=False)
    b2 = dp("b2", [HID], FP, isOutput=False)
    wml = dp("wml", [HID, 4], BF, isOutput=False)
    bmu = dp("bmu", [2], FP, isOutput=False)
    blvh = dp("blvh", [2], FP, isOutput=False)
    dw1 = dp("dw1", [57, HID], BF, isOutput=False)
    db1 = dp("db1", [HID], FP, isOutput=False)
    dw2 = dp("dw2", [HID, HID], BF, isOutput=False)
    db2 = dp("db2", [HID], FP, isOutput=False)
    dw3 = dp("dw3", [HID, 8], BF, isOutput=False)
    db3 = dp("db3", [8], FP, isOutput=False)

    # ---- replicated QP constants ----
    PTp = dp("PTp", [NVAR, NUM], FP, isOutput=False)          # P.T
    PdT = dp("PdT", [NVAR, NUM], FP, isOutput=False)
    PddT = dp("PddT", [NVAR, NUM], FP, isOutput=False)
    Pdk = dp("Pdk", [NUM, NVAR], BF, isOutput=False)
    Pddk = dp("Pddk", [NUM, NVAR], BF, isOutput=False)
    Pres = dp("Pres", [NUM, NVAR], BF, isOutput=False)        # RO*P
    Plane = dp("Plane", [NUM, NVAR], BF, isOutput=False)      # RL*P
    J2x2 = dp("J2x2", [NVAR, NVAR], FP, isOutput=False)
    J2xn = dp("J2xn", [NVAR, NVAR], FP, isOutput=False)
    WJx = dp("WJx", [NVAR, NVAR], FP, isOutput=False)
    J2y2 = dp("J2y2", [NVAR, NVAR], FP, isOutput=False)
    J2yn = dp("J2yn", [NVAR, NVAR], FP, isOutput=False)
    WJy = dp("WJy", [NVAR, NVAR], FP, isOutput=False)
    Cx4 = dp("Cx4", [4, NVAR], FP, isOutput=False)
    Cy4 = dp("Cy4", [4, NVAR], FP, isOutput=False)
    Cx4J = dp("Cx4J", [4, NVAR], FP, isOutput=False)
    Cy4J = dp("Cy4J", [4, NVAR], FP, isOutput=False)
    I11 = dp("I11", [NVAR, NVAR], FP, isOutput=False)

    out_t = dp("out_t", [2 * NVAR, BPC], FP, isOutput=True)

    mm = nc.tensor.matmul

    lp = nc.allow_low_precision(reason="bf16 obstacle chain is within tolerance")
    lp.__enter__()
    with _TC(nc) as tc:
        cst = tc.alloc_tile_pool(name="cst", bufs=1)
        act = tc.alloc_tile_pool(name="act", bufs=2)
        lam = tc.alloc_tile_pool(name="lam", bufs=3)
        psp = tc.alloc_tile_pool(name="ps", bufs=3, space="PSUM")

        dmae = [nc.sync, nc.gpsimd, nc.scalar]

        def cload(ap, shape, dtype=FP, tag=None, q=0):
            t = cst.tile(shape, dtype, tag=tag or ap.tensor.name)
            dmae[q % len(dmae)].dma_start(out=t[:], in_=ap)
            return t

        # ---------- constants into SBUF ----------
        sb_PTp = cload(PTp[:], [NVAR, NUM])
        sb_PdT = cload(PdT[:], [NVAR, NUM])
        sb_PddT = cload(PddT[:], [NVAR, NUM])
        sb_Pdk = cload(Pdk[:], [NUM, NVAR], BF)
        sb_Pddk = cload(Pddk[:], [NUM, NVAR], BF)
        sb_Pres = cload(Pres[:], [NUM, NVAR], BF)
        sb_Plane = cload(Plane[:], [NUM, NVAR], BF)
        sb_J2x2 = cload(J2x2[:], [NVAR, NVAR])
        sb_J2xn = cload(J2xn[:], [NVAR, NVAR])
        sb_WJx = cload(WJx[:], [NVAR, NVAR])
        sb_J2y2 = cload(J2y2[:], [NVAR, NVAR])
        sb_J2yn = cload(J2yn[:], [NVAR, NVAR])
        sb_WJy = cload(WJy[:], [NVAR, NVAR])
        sb_Cx4 = cload(Cx4[:], [4, NVAR])
        sb_Cy4 = cload(Cy4[:], [4, NVAR])
        sb_Cx4J = cload(Cx4J[:], [4, NVAR])
        sb_Cy4J = cload(Cy4J[:], [4, NVAR])
        sb_I11 = cload(I11[:], [NVAR, NVAR])
        sb_beq1 = cload(beq1xy[:], [NVAR, 2 * BPC], q=1)
        sb_beqC = cload(beqCxy[:], [NVAR, 2 * BPC], q=1)
        sb_xyobs = cload(xyobs[:], [NUM, 2 * NB], BF, q=2)
        sb_ylbc = cload(ylbc[:], [NUM, BPC], BF, q=1)
        sb_yubc = cload(yubc[:], [NUM, BPC], BF, q=1)

        # ---------- MLP weights into SBUF (spread across DMA queues) ----------
        sb_w1a = cload(w1[0:128, :], [128, HID], BF, tag="w1a", q=1)
        sb_w1b = cload(w1[128:255, :], [127, HID], BF, tag="w1b", q=2)
        sb_w2 = cst.tile([128, 8 * HID], BF, tag="w2")
        for kc in range(8):
            dmae[kc % 3].dma_start(
                out=sb_w2[:, kc * HID : (kc + 1) * HID],
                in_=w2[kc * 128 : (kc + 1) * 128, :],
            )
        sb_wml = cst.tile([128, 8 * 4], BF, tag="wml")
        for kc in range(8):
            nc.sync.dma_start(
                out=sb_wml[:, kc * 4 : (kc + 1) * 4],
                in_=wml[kc * 128 : (kc + 1) * 128, :],
            )
        sb_dw1 = cload(dw1[:], [57, HID], BF, q=3)
        sb_dw2 = cst.tile([128, 8 * HID], BF, tag="dw2")
        for kc in range(8):
            dmae[kc % 3].dma_start(
                out=sb_dw2[:, kc * HID : (kc + 1) * HID],
                in_=dw2[kc * 128 : (kc + 1) * 128, :],
            )
        sb_dw3 = cst.tile([128, 8 * 8], BF, tag="dw3")
        for kc in range(8):
            nc.gpsimd.dma_start(
                out=sb_dw3[:, kc * 8 : (kc + 1) * 8],
                in_=dw3[kc * 128 : (kc + 1) * 128, :],
            )
        sb_b1 = cload(b1[:].rearrange("(c p) -> c p", p=128).rearrange("c p -> p c"), [128, 8], q=3)
        sb_b2 = cload(b2[:].rearrange("(c p) -> c p", p=128).rearrange("c p -> p c"), [128, 8], q=3)
        sb_db1 = cload(db1[:].rearrange("(c p) -> c p", p=128).rearrange("c p -> p c"), [128, 8], q=3)
        sb_db2 = cload(db2[:].rearrange("(c p) -> c p", p=128).rearrange("c p -> p c"), [128, 8], q=3)
        sb_bmu = cload(bmu[:, None], [2, 1], q=3)
        sb_blvh = cload(blvh[:, None], [2, 1], q=3)
        sb_db3v = cload(db3[0:4][:, None], [4, 1], tag="db3v", q=3)
        sb_db3y = cload(db3[4:8][:, None], [4, 1], tag="db3y", q=3)

        sb_x1a = cload(enc_in[0:128, :], [128, BPC], BF, tag="x1a", q=1)
        sb_x1b = cload(enc_in[128:255, :], [127, BPC], BF, tag="x1b", q=1)
        sb_eps = cload(eps_t[:], [2, BPC], q=1)

        # ---------- MLP (bf16 weights/acts, fp32 PSUM) ----------
        h1 = act.tile([128, HID], BF, tag="h1", bufs=1)
        for jc in range(8):
            ps = psp.tile([128, BPC], FP, tag="psum_a", bufs=4)
            js = slice(jc * 128, (jc + 1) * 128)
            mm(ps[:], sb_w1a[:, js], sb_x1a[:], start=True, stop=False)
            mm(ps[:], sb_w1b[:, js], sb_x1b[:], start=False, stop=True)
            nc.scalar.activation(h1[:, js], ps[:], AF.Relu, bias=sb_b1[:, jc : jc + 1])
        h2 = act.tile([128, HID], BF, tag="h2", bufs=1)
        for jc in range(8):
            ps = psp.tile([128, BPC], FP, tag="psum_a", bufs=4)
            js = slice(jc * 128, (jc + 1) * 128)
            for kc in range(8):
                mm(
                    ps[:],
                    sb_w2[:, kc * HID + jc * 128 : kc * HID + (jc + 1) * 128],
                    h1[:, kc * 128 : (kc + 1) * 128],
                    start=(kc == 0),
                    stop=(kc == 7),
                )
            nc.scalar.activation(h2[:, js], ps[:], AF.Relu, bias=sb_b2[:, jc : jc + 1])
        mu_ps = psp.tile([2, BPC], FP, tag="psum_b", bufs=4)
        lv_ps = psp.tile([2, BPC], FP, tag="psum_b", bufs=4)
        for kc in range(8):
            mm(
                mu_ps[:],
                sb_wml[:, kc * 4 : kc * 4 + 2],
                h2[:, kc * 128 : (kc + 1) * 128],
                start=(kc == 0),
                stop=(kc == 7),
            )
            mm(
                lv_ps[:],
                sb_wml[:, kc * 4 + 2 : kc * 4 + 4],
                h2[:, kc * 128 : (kc + 1) * 128],
                start=(kc == 0),
                stop=(kc == 7),
            )
        # z = mu + bmu + exp(0.5*lv + 0.5*blv)*eps  -> rows 0:2 of dec_in (bf16)
        dec_in = act.tile([57, BPC], BF, tag="dec_in", bufs=1)
        nc.sync.dma_start(out=dec_in[2:57, :], in_=inp_n_t[:])
        std = act.tile([2, BPC], FP, tag="std", bufs=1)
        nc.scalar.activation(std[:], lv_ps[:], AF.Exp, bias=sb_blvh[:], scale=0.5)
        zz = act.tile([2, BPC], FP, tag="zz", bufs=1)
        nc.vector.tensor_mul(out=zz[:], in0=std[:], in1=sb_eps[:])
        zmu = act.tile([2, BPC], FP, tag="zmu", bufs=1)
        nc.vector.tensor_add(out=zmu[:], in0=zz[:], in1=mu_ps[:])
        nc.vector.tensor_scalar(
            out=dec_in[0:2, :], in0=zmu[:], scalar1=sb_bmu[:], scalar2=None,
            op0=OP.add,
        )
        g1 = act.tile([128, HID], BF, tag="g1", bufs=1)
        for jc in range(8):
            ps = psp.tile([128, BPC], FP, tag="psum_a", bufs=4)
            js = slice(jc * 128, (jc + 1) * 128)
            mm(ps[:], sb_dw1[:, js], dec_in[:], start=True, stop=True)
            nc.scalar.activation(g1[:, js], ps[:], AF.Relu, bias=sb_db1[:, jc : jc + 1])
        g2 = act.tile([128, HID], BF, tag="g2", bufs=1)
        for jc in range(8):
            ps = psp.tile([128, BPC], FP, tag="psum_a", bufs=4)
            js = slice(jc * 128, (jc + 1) * 128)
            for kc in range(8):
                mm(
                    ps[:],
                    sb_dw2[:, kc * HID + jc * 128 : kc * HID + (jc + 1) * 128],
                    g1[:, kc * 128 : (kc + 1) * 128],
                    start=(kc == 0),
                    stop=(kc == 7),
                )
            nc.scalar.activation(g2[:, js], ps[:], AF.Relu, bias=sb_db2[:, jc : jc + 1])
        nnv_ps = psp.tile([4, BPC], FP, tag="psum_b", bufs=4)
        nny_ps = psp.tile([4, BPC], FP, tag="psum_b", bufs=4)
        for kc in range(8):
            mm(
                nnv_ps[:],
                sb_dw3[:, kc * 8 : kc * 8 + 4],
                g2[:, kc * 128 : (kc + 1) * 128],
                start=(kc == 0),
                stop=(kc == 7),
            )
            mm(
                nny_ps[:],
                sb_dw3[:, kc * 8 + 4 : kc * 8 + 8],
                g2[:, kc * 128 : (kc + 1) * 128],
                start=(kc == 0),
                stop=(kc == 7),
            )
        nn_v = act.tile([4, BPC], FP, tag="nn_v", bufs=1)
        nc.scalar.activation(nn_v[:], nnv_ps[:], AF.Identity, bias=sb_db3v[:])
        nn_y = act.tile([4, BPC], FP, tag="nn_y", bufs=1)
        nc.scalar.activation(nn_y[:], nny_ps[:], AF.Identity, bias=sb_db3y[:])

        # ---------- QP setup ----------
        st_ps = psp.tile([NVAR, 2 * BPC], FP, tag="psum_b", bufs=4)
        mm(st_ps[:, :BPC], sb_Cx4[:], nn_v[:], start=True, stop=False)
        mm(st_ps[:, :BPC], sb_I11[:], sb_beq1[:, :BPC], start=False, stop=True)
        mm(st_ps[:, BPC:], sb_Cy4[:], nn_y[:], start=True, stop=False)
        mm(st_ps[:, BPC:], sb_I11[:], sb_beq1[:, BPC:], start=False, stop=True)
        cxy = lam.tile([NVAR, 2 * BPC], FP, tag="cxy")
        nc.scalar.activation(cxy[:], st_ps[:], AF.Copy)

        sC_ps = psp.tile([NVAR, 2 * BPC], FP, tag="psum_b", bufs=4)
        mm(sC_ps[:, :BPC], sb_Cx4J[:], nn_v[:], start=True, stop=False)
        mm(sC_ps[:, :BPC], sb_I11[:], sb_beqC[:, :BPC], start=False, stop=True)
        mm(sC_ps[:, BPC:], sb_Cy4J[:], nn_y[:], start=True, stop=False)
        mm(sC_ps[:, BPC:], sb_I11[:], sb_beqC[:, BPC:], start=False, stop=True)
        CXC = cst.tile([NVAR, 2 * BPC], FP, tag="CXC")
        nc.scalar.activation(CXC[:], sC_ps[:], AF.Copy)

        sb_tiny = cst.tile([128, 1], FP, tag="tiny")
        nc.vector.memset(sb_tiny[:], 1e-30)
        sb_negone = cst.tile([128, 1], FP, tag="negone")
        nc.vector.memset(sb_negone[:], -1.0)

        # ---------- split into two independent 64-row streams ----------
        B2 = BPC // 2          # 64
        NB2 = NOBS * B2        # 640
        cxyS = []
        CXCS = []
        lxyS = []
        for s in (0, 1):
            cs = lam.tile([NVAR, 2 * B2], FP, tag=f"cxy{s}", name=f"cxyS{s}")
            nc.scalar.activation(
                cs[:].rearrange("p (a b) -> p a b", a=2),
                cxy[:].rearrange("p (a b) -> p a b", a=2)[:, :, s * B2 : (s + 1) * B2],
                AF.Copy,
            )
            cxyS.append(cs)
            xc = cst.tile([NVAR, 2 * B2], FP, tag=f"CXC{s}", name=f"CXCS{s}")
            nc.scalar.activation(
                xc[:].rearrange("p (a b) -> p a b", a=2),
                CXC[:].rearrange("p (a b) -> p a b", a=2)[:, :, s * B2 : (s + 1) * B2],
                AF.Copy,
            )
            CXCS.append(xc)
            lx = lam.tile([NVAR, 2 * B2], FP, tag=f"lxy{s}", name=f"lxyS{s}")
            nc.vector.memset(lx[:], 0.0)
            lxyS.append(lx)

        # ---------- QP iterations (two interleaved streams) ----------
        def qp_iter(s):
            cxy_s = cxyS[s]
            lxy_s = lxyS[s]
            obs = sb_xyobs[:, s * 2 * NB2 : (s + 1) * 2 * NB2]
            fwd = psp.tile([NUM, 6 * B2], FP, tag="psum_a", bufs=4, name=f"fwd{s}")
            xy = fwd[:, 0 : 2 * B2]
            xyd = fwd[:, 2 * B2 : 4 * B2]
            xydd = fwd[:, 4 * B2 : 6 * B2]
            mm(xy, sb_PTp[:], cxy_s[:], start=True, stop=True, skip_group_check=True)
            mm(xyd, sb_PdT[:], cxy_s[:], start=True, stop=True, skip_group_check=True)
            mm(xydd, sb_PddT[:], cxy_s[:], start=True, stop=True, skip_group_check=True)

            xys = act.tile([NUM, 2 * B2], BF, tag=f"xys{s}", name=f"xys_{s}")
            nc.scalar.activation(xys[:], xy, AF.Copy)
            w_s = act.tile([NUM, 2 * NB2], BF, tag=f"w_s{s}", name=f"w_s_{s}")
            nc.vector.tensor_tensor(
                out=w_s[:].rearrange("p (a o b) -> p a o b", a=2, o=NOBS),
                in0=xys[:]
                .rearrange("p (a b) -> p a b", a=2)[:, :, None, :]
                .to_broadcast((NUM, 2, NOBS, B2)),
                in1=obs.rearrange("p (a o b) -> p a o b", a=2, o=NOBS),
                op=OP.subtract,
            )

            sqw = act.tile([NUM, 2 * NB2], BF, tag=f"sqw{s}", name=f"sqw_{s}")
            nc.scalar.activation(sqw[:, :NB2], w_s[:, :NB2], AF.Square, scale=B_OBS)
            nc.scalar.activation(sqw[:, NB2:], w_s[:, NB2:], AF.Square, scale=A_OBS)
            sqd = act.tile([NUM, 2 * B2], BF, tag=f"sqd{s}", name=f"sqd_{s}")
            nc.scalar.activation(sqd[:], xyd, AF.Square)
            sqdd = act.tile([NUM, 2 * B2], BF, tag=f"sqdd{s}", name=f"sqdd_{s}")
            nc.scalar.activation(sqdd[:], xydd, AF.Square)

            r2all = act.tile([NUM, NB2 + 2 * B2], BF, tag=f"r2all{s}", name=f"r2all_{s}")
            nc.vector.tensor_add(out=r2all[:, :NB2], in0=sqw[:, :NB2], in1=sqw[:, NB2:])
            nc.vector.tensor_add(
                out=r2all[:, NB2 : NB2 + B2], in0=sqd[:, :B2], in1=sqd[:, B2:]
            )
            nc.vector.tensor_add(
                out=r2all[:, NB2 + B2 :], in0=sqdd[:, :B2], in1=sqdd[:, B2:]
            )
            r_all = act.tile([NUM, NB2 + 2 * B2], BF, tag=f"r_all{s}", name=f"r_all_{s}")
            nc.scalar.activation(r_all[:], r2all[:], AF.Sqrt, bias=sb_tiny[0:NUM, :])
            rinv = act.tile([NUM, NB2 + 2 * B2], BF, tag=f"rinv{s}", name=f"rinv_{s}")
            nc.vector.reciprocal(out=rinv[:], in_=r_all[:])

            qf = act.tile([NUM, NB2], BF, tag=f"qf{s}", name=f"qf_{s}")
            nc.scalar.activation(
                qf[:], rinv[:, :NB2], AF.Relu, bias=sb_negone[0:NUM, :], scale=AB
            )
            e_s = act.tile([NUM, 2 * NB2], BF, tag=f"e_s{s}", name=f"e_s_{s}")
            nc.vector.tensor_mul(out=e_s[:, :NB2], in0=qf[:], in1=w_s[:, :NB2])
            nc.vector.tensor_mul(out=e_s[:, NB2:], in0=qf[:], in1=w_s[:, NB2:])

            hva = act.tile([NUM, 2 * B2], FP, tag=f"hva{s}", name=f"hva_{s}")
            nc.vector.tensor_scalar(
                out=hva[:, :B2], in0=r_all[:, NB2 : NB2 + B2],
                scalar1=VMIN, scalar2=VMAX, op0=OP.max, op1=OP.min,
            )
            nc.vector.tensor_scalar(
                out=hva[:, B2:], in0=r_all[:, NB2 + B2 :],
                scalar1=AMAX, scalar2=None, op0=OP.min,
            )
            gva = act.tile([NUM, 2 * B2], FP, tag=f"gva{s}", name=f"gva_{s}")
            nc.vector.tensor_mul(out=gva[:], in0=hva[:], in1=rinv[:, NB2:])
            gvam = act.tile([NUM, 2 * B2], FP, tag=f"gvam{s}", name=f"gvam_{s}")
            nc.vector.tensor_scalar(
                out=gvam[:], in0=gva[:], scalar1=-RI, scalar2=RI,
                op0=OP.mult, op1=OP.add,
            )

            ys = xys[:, B2:]
            lt1 = act.tile([NUM, B2], BF, tag=f"lt1{s}", name=f"lt1_{s}")
            nc.vector.tensor_max(
                out=lt1[:], in0=ys, in1=sb_ylbc[:, s * B2 : (s + 1) * B2]
            )
            lt2 = act.tile([NUM, B2], BF, tag=f"lt2{s}", name=f"lt2_{s}")
            nc.vector.tensor_tensor(
                out=lt2[:], in0=lt1[:], in1=sb_yubc[:, s * B2 : (s + 1) * B2], op=OP.min
            )
            d12 = act.tile([NUM, B2], BF, tag=f"d12{s}", name=f"d12_{s}")
            nc.vector.tensor_sub(out=d12[:], in0=ys, in1=lt2[:])

            rvxy = act.tile([NUM, 2 * B2], BF, tag=f"rvxy{s}", name=f"rvxy_{s}")
            nc.vector.tensor_tensor(
                out=rvxy[:].rearrange("p (a b) -> p a b", a=2),
                in0=gvam[:, 0:B2][:, None, :].to_broadcast((NUM, 2, B2)),
                in1=xyd.rearrange("p (a b) -> p a b", a=2),
                op=OP.mult,
            )
            raxy = act.tile([NUM, 2 * B2], BF, tag=f"raxy{s}", name=f"raxy_{s}")
            nc.vector.tensor_tensor(
                out=raxy[:].rearrange("p (a b) -> p a b", a=2),
                in0=gvam[:, B2:][:, None, :].to_broadcast((NUM, 2, B2)),
                in1=xydd.rearrange("p (a b) -> p a b", a=2),
                op=OP.mult,
            )

            Dx = psp.tile([NVAR, B2], FP, tag="psum_b", bufs=4, name=f"Dx{s}")
            for o in range(NOBS):
                mm(
                    Dx[:],
                    sb_Pres[:],
                    e_s[:, o * B2 : (o + 1) * B2],
                    start=(o == 0),
                    stop=False,
                )
            mm(Dx[:], sb_Pddk[:], raxy[:, :B2], start=False, stop=False)
            mm(Dx[:], sb_Pdk[:], rvxy[:, :B2], start=False, stop=True)

            Dy = psp.tile([NVAR, B2], FP, tag="psum_b", bufs=4, name=f"Dy{s}")
            for o in range(NOBS):
                mm(
                    Dy[:],
                    sb_Pres[:],
                    e_s[:, NB2 + o * B2 : NB2 + (o + 1) * B2],
                    start=(o == 0),
                    stop=False,
                )
            mm(Dy[:], sb_Pddk[:], raxy[:, B2:], start=False, stop=False)
            mm(Dy[:], sb_Pdk[:], rvxy[:, B2:], start=False, stop=False)
            mm(Dy[:], sb_Plane[:], d12[:], start=False, stop=True)

            lxy_new = lam.tile([NVAR, 2 * B2], FP, tag=f"lxy{s}", name=f"lxyn_{s}")
            nc.vector.tensor_sub(out=lxy_new[:, :B2], in0=lxy_s[:, :B2], in1=Dx[:])
            nc.vector.tensor_sub(out=lxy_new[:, B2:], in0=lxy_s[:, B2:], in1=Dy[:])

            cn = psp.tile([NVAR, 2 * B2], FP, tag="psum_b", bufs=4, name=f"cn{s}")
            mm(cn[:, :B2], sb_J2x2[:], lxy_new[:, :B2], start=True, stop=False)
            mm(cn[:, :B2], sb_J2xn[:], lxy_s[:, :B2], start=False, stop=False)
            mm(cn[:, :B2], sb_WJx[:], cxy_s[:, :B2], start=False, stop=False)
            mm(cn[:, :B2], sb_I11[:], CXCS[s][:, :B2], start=False, stop=True)
            mm(cn[:, B2:], sb_J2y2[:], lxy_new[:, B2:], start=True, stop=False)
            mm(cn[:, B2:], sb_J2yn[:], lxy_s[:, B2:], start=False, stop=False)
            mm(cn[:, B2:], sb_WJy[:], cxy_s[:, B2:], start=False, stop=False)
            mm(cn[:, B2:], sb_I11[:], CXCS[s][:, B2:], start=False, stop=True)

            cxy_new = lam.tile([NVAR, 2 * B2], FP, tag=f"cxy{s}", name=f"cxyn_{s}")
            nc.scalar.activation(cxy_new[:], cn[:], AF.Copy)
            cxyS[s] = cxy_new
            lxyS[s] = lxy_new

        for it in range(maxiter):
            qp_iter(0)
            qp_iter(1)

        for s in (0, 1):
            nc.sync.dma_start(
                out=out_t[:].rearrange("(h k) b -> k h b", h=2)[
                    :, :, s * B2 : (s + 1) * B2
                ],
                in_=cxyS[s][:].rearrange("p (h b) -> p h b", h=2),
            )

        psp.release()
        lam.release()
        act.release()
        cst.release()

    lp.__exit__(None, None, None)
    if split_waits:
        _split_sync_waits(nc)
    return nc


# ---------------------------------------------------------------------------
# host preprocessing
# ---------------------------------------------------------------------------

def _kkt_inv(cost, A_eq):
    m = A_eq.shape[0]
    n = cost.shape[0]
    M = np.zeros((n + m, n + m), np.float64)
    M[:n, :n] = cost
    M[:n, n:] = A_eq.T
    M[n:, :n] = A_eq
    return np.linalg.inv(M).astype(f32)


def host_prep(inputs):
    """Returns per-core input dicts for run_bass_kernel_spmd."""
    inp = np.ascontiguousarray(np.asarray(inputs["inp"], f32))
    P = np.asarray(inputs["P"], f32)
    Pd = np.asarray(inputs["Pdot"], f32)
    Pdd = np.asarray(inputs["Pddot"], f32)
    B = inp.shape[0]
    t = np.linspace(0.0, 15.0, NUM).astype(f32)

    A_eq_x = np.stack([P[0], Pd[0], Pdd[0]])
    A_eq_y = np.stack([P[0], Pd[0], Pdd[0], Pd[-1]])
    K_D = f32(2.0 * np.sqrt(np.float32(20.0)))
    A_pd = Pdd - f32(20.0) * P - K_D * Pd
    A_vd = Pdd - f32(20.0) * Pd
    cs = Pdd.T @ Pdd
    inv1_x = _kkt_inv(cs + A_vd.T @ A_vd, A_eq_x)
    inv1_y = _kkt_inv(cs + A_pd.T @ A_pd, A_eq_y)
    PtP = P.T @ P
    PdTPd = Pd.T @ Pd
    PddTPdd = Pdd.T @ Pdd
    I = np.eye(NVAR, dtype=f32)
    cost2_x = RP * I + RO * 10.0 * PtP + RI * PddTPdd + RI * PdTPd
    cost2_y = cost2_x + RL * 2.0 * PtP
    inv2_x = _kkt_inv(cost2_x, A_eq_x)
    inv2_y = _kkt_inv(cost2_y, A_eq_y)

    J1x = inv1_x[:NVAR, :NVAR].T
    K1x = inv1_x[:NVAR, NVAR:].T
    J1y = inv1_y[:NVAR, :NVAR].T
    K1y = inv1_y[:NVAR, NVAR:].T
    J2x = inv2_x[:NVAR, :NVAR].T
    K2x = inv2_x[:NVAR, NVAR:].T
    J2y = inv2_y[:NVAR, :NVAR].T
    K2y = inv2_y[:NVAR, NVAR:].T

    Avd_bs = A_vd.reshape(4, 25, NVAR).sum(1)
    Apd_bs = A_pd.reshape(4, 25, NVAR).sum(1)
    Cx4 = (f32(-20.0) * (Avd_bs @ J1x)).astype(f32)
    Cy4 = (f32(-20.0) * (Apd_bs @ J1y)).astype(f32)
    Wx = (f32(10.0 * RO) * PtP + f32(RI) * (PddTPdd + PdTPd)).astype(f32)
    Wy = (Wx + f32(2.0 * RL) * PtP).astype(f32)

    consts = {
        "PTp": np.ascontiguousarray(P.T.astype(f32)),
        "PdT": np.ascontiguousarray(Pd.T.astype(f32)),
        "PddT": np.ascontiguousarray(Pdd.T.astype(f32)),
        "Pdk": np.ascontiguousarray(Pd.astype(bf16)),
        "Pddk": np.ascontiguousarray(Pdd.astype(bf16)),
        "Pres": np.ascontiguousarray((-RO * P).astype(bf16)),
        "Plane": np.ascontiguousarray((RL * P).astype(bf16)),
        "J2x2": np.ascontiguousarray((2.0 * J2x).astype(f32)),
        "J2xn": np.ascontiguousarray((-J2x).astype(f32)),
        "WJx": np.ascontiguousarray((Wx @ J2x).astype(f32)),
        "J2y2": np.ascontiguousarray((2.0 * J2y).astype(f32)),
        "J2yn": np.ascontiguousarray((-J2y).astype(f32)),
        "WJy": np.ascontiguousarray((Wy @ J2y).astype(f32)),
        "Cx4": Cx4,
        "Cy4": Cy4,
        "Cx4J": np.ascontiguousarray((Cx4 @ J2x).astype(f32)),
        "Cy4J": np.ascontiguousarray((Cy4 @ J2y).astype(f32)),
        "I11": I,
        "w1": np.ascontiguousarray(np.asarray(inputs["enc_w1"], f32).astype(bf16)),
        "b1": np.ascontiguousarray(np.asarray(inputs["enc_b1"], f32)),
        "w2": np.ascontiguousarray(np.asarray(inputs["enc_w2"], f32).astype(bf16)),
        "b2": np.ascontiguousarray(np.asarray(inputs["enc_b2"], f32)),
        "wml": np.ascontiguousarray(
            np.concatenate(
                [np.asarray(inputs["enc_wmu"], f32), np.asarray(inputs["enc_wlv"], f32)],
                axis=1,
            ).astype(bf16)
        ),
        "bmu": np.ascontiguousarray(np.asarray(inputs["enc_bmu"], f32)),
        "blvh": np.ascontiguousarray((0.5 * np.asarray(inputs["enc_blv"], f32))),
        "dw1": np.ascontiguousarray(np.asarray(inputs["dec_w1"], f32).astype(bf16)),
        "db1": np.ascontiguousarray(np.asarray(inputs["dec_b1"], f32)),
        "dw2": np.ascontiguousarray(np.asarray(inputs["dec_w2"], f32).astype(bf16)),
        "db2": np.ascontiguousarray(np.asarray(inputs["dec_b2"], f32)),
        "dw3": np.ascontiguousarray(np.asarray(inputs["dec_w3"], f32).astype(bf16)),
        "db3": np.ascontiguousarray(np.asarray(inputs["dec_b3"], f32)),
    }

    # batch-dependent host work
    ise = np.asarray(inputs["initial_state_ego"], f32)
    zb = np.zeros((B,), f32)
    b_eq_x = np.stack([zb, ise[:, 2], zb], 1)
    b_eq_y = np.stack([zb, ise[:, 3], zb, zb], 1)
    beq1x = b_eq_x @ K1x
    beq1y = b_eq_y @ K1y
    beqCx = b_eq_x @ (K1x @ J2x + K2x)
    beqCy = b_eq_y @ (K1y @ J2y + K2y)

    x_obs, y_obs = inp[:, 5::5], inp[:, 6::5]
    vx_obs, vy_obs = inp[:, 7::5], inp[:, 8::5]
    xo = x_obs[:, :, None] + vx_obs[:, :, None] * t        # (B, 10, 100)
    yo = y_obs[:, :, None] + vy_obs[:, :, None] * t
    # device obstacle layout [t, a, o, b], unscaled
    xyobs_full = np.empty((NUM, 2, NOBS, B), bf16)
    xyobs_full[:, 0] = xo.transpose(2, 1, 0).astype(bf16)
    xyobs_full[:, 1] = yo.transpose(2, 1, 0).astype(bf16)

    inp_mean = np.asarray(inputs["inp_mean"], f32)
    inp_std = np.asarray(inputs["inp_std"], f32)
    inp_n = (inp - inp_mean) / inp_std
    enc_in = np.concatenate(
        [inp_n, np.asarray(inputs["traj_gt"], f32)], axis=1
    ).T.astype(bf16)                                       # (255, B)
    inp_n_t = inp_n.T.astype(bf16)                         # (55, B)
    eps_t = np.asarray(inputs["eps"], f32).T               # (2, B)

    y_ub = np.asarray(inputs["y_ub"], f32)
    y_lb = np.asarray(inputs["y_lb"], f32)

    in_maps = []
    for c in range(NCORES):
        s = slice(c * BPC, (c + 1) * BPC)
        m = dict(consts)
        m["enc_in"] = np.ascontiguousarray(enc_in[:, s])
        m["inp_n_t"] = np.ascontiguousarray(inp_n_t[:, s])
        m["eps_t"] = np.ascontiguousarray(eps_t[:, s])
        m["beq1xy"] = np.ascontiguousarray(
            np.concatenate([beq1x[s].T, beq1y[s].T], axis=1).astype(f32)
        )
        m["beqCxy"] = np.ascontiguousarray(
            np.concatenate([beqCx[s].T, beqCy[s].T], axis=1).astype(f32)
        )
        blk = xyobs_full[:, :, :, s]                       # (100, 2, 10, 128)
        m["xyobs"] = np.ascontiguousarray(
            np.stack([blk[:, :, :, 0:64], blk[:, :, :, 64:128]], axis=1).reshape(
                NUM, 2 * NOBS * BPC
            )
        )
        m["ylbc"] = np.ascontiguousarray(
            np.broadcast_to(y_lb[s].astype(bf16), (NUM, BPC))
        )
        m["yubc"] = np.ascontiguousarray(
            np.broadcast_to(y_ub[s].astype(bf16), (NUM, BPC))
        )
        in_maps.append(m)
    return in_maps


_NC = None


def kernel(**inputs):
    global _NC
    if _NC is None:
        _NC = build_module()
    in_maps = host_prep(inputs)
    res = run_bass_kernel_spmd(_NC, in_maps, list(range(NCORES)))
    outs = [np.asarray(r["out_t"], f32).T for r in res.results]  # (128, 22) each
    return np.ascontiguousarray(np.concatenate(outs, axis=0))



# revision 10
# speedup vs baseline: 15.0846x; 15.0846x over previous
"""Trainium2 Bass kernel for nn_Beta_cVAE (cVAE encoder/decoder + 2-layer QP
trajectory projection, 20 ADMM iterations).

Strategy: pure data-parallel over 8 NeuronCores (128 batch rows each).
On-device layout is fully transposed: batch on the free dim, time/NVAR on
partitions, so every matmul uses weights/basis as the stationary operand and
no transposes appear anywhere.  All arctan2/cos/sin of the reference are
eliminated algebraically (cos(atan2(a,b)) = b/hypot(a,b)): the obstacle
ellipse projection becomes a residual scaling  resid = w*min(r-AB,0)/r  that
is *exactly zero* for inactive obstacles (no large-term cancellation, which
makes the bf16 obstacle pipeline safe), velocity/accel projections become
clip-scalings, and the lane term reduces to y - clip(y, lb, ub).  Obstacle
sums over the 10 obstacles fold into PE matmul accumulations with a shared
stationary operand.  MLP runs in bf16 (fp32 PSUM accumulation).

Host/runtime side: the expensive parts of a call (jit trace/lower, NEFF
load, 44 MB weight upload over the axon tunnel) are all cached across calls:
the jitted shard_map executable is built once, the replicated weights /
QP constants are device-resident keyed by a crc32 of the weight arrays, and
per-call traffic is only the ~0.8 MB of batch-dependent inputs (encoder
input, eps, initial velocities, lane bounds, obstacle states — obstacle
*trajectories* and boundary-condition expansions are computed on device).
The output is fetched with copy_to_host_async so the D2H ride shares the
dispatch round-trip.
"""
import sys
import os
import zlib

sys.path.insert(0, "/opt/trn_rl_repo")

import numpy as np
import ml_dtypes
import jax

import concourse.bass as bass
import concourse.mybir as mybir
from concourse.tile import TileContext, ScopedClock
from concourse.bass2jax import (
    _bass_exec_p,
    install_neuronx_cc_hook,
    partition_id_tensor,
)
from jax.experimental.shard_map import shard_map
from jax.sharding import Mesh, NamedSharding, PartitionSpec

f32 = np.float32
bf16 = ml_dtypes.bfloat16
FP = mybir.dt.float32
FR = mybir.dt.float32r
BF = mybir.dt.bfloat16
AF = mybir.ActivationFunctionType
OP = mybir.AluOpType

NCORES = 8
BPC = 128          # batch rows per core
NUM, NVAR, NOBS = 100, 11, 10
NB = NOBS * BPC    # 1280
B2 = BPC // 2      # 64 (per interleaved stream)
NB2 = NOBS * B2    # 640
A_OBS, B_OBS = 8.0, 4.2
AB = A_OBS * B_OBS
RO, RI, RL, RP = 100.0, 100.0, 100.0, 1.0
VMIN, VMAX, AMAX = 0.1, 30.0, 8.0
MAXITER = 20
HID = 1024
T_FIN = 15.0


class _TC(TileContext):
    """TileContext whose tail drain splits sem waits: this walrus build's
    TPB_CTRL codegen accepts only one sync wait per instruction."""

    def _drain_and_barrier(self, tick_clock, wait_clock):
        drain_inst = self.nc.sync.drain()
        wait_clock.add_sem_waits(
            drain_inst.ins, ScopedClock({None: tick_clock.global_clock})
        )
        si = drain_inst.ins.sync_info
        waits = list(si.on_wait) if si is not None else []
        if len(waits) > 1:
            drain_inst.ins.sync_info = mybir.SyncInfo(
                on_wait=waits[:1], on_update=list(si.on_update)
            )
            for w in waits[1:]:
                nop = self.nc.sync.nop(nofuse=True, hint="split_drain_wait")
                nop.ins.sync_info = mybir.SyncInfo(on_wait=[w], on_update=[])

        self.nc.all_engine_barrier()
        assert self.sems is not None
        popped = self.nc._tile_sem_poison_stack.pop()
        assert popped is self._sem_poison
        self.nc.clear_and_free_semaphores(list(self.sems.allocated().values()))
        self.nc.all_engine_barrier()


def _split_sync_waits(nc, max_waits=1):
    """Walrus's CoreV3 codegen accepts few sync waits per instruction; hoist
    surplus waits onto same-engine nops inserted just before the offender."""
    ctr = [0]
    for f in nc.m.functions:
        for bb in f.blocks:
            insts = list(bb.instructions)
            out = []
            for inst in insts:
                si = getattr(inst, "sync_info", None)
                if si is not None and len(si.on_wait) > max_waits:
                    waits = list(si.on_wait)
                    keep = waits[-max_waits:]
                    rest = waits[: len(waits) - max_waits]
                    for i in range(0, len(rest), max_waits):
                        ctr[0] += 1
                        nop = mybir.InstNoOp(
                            name=f"swsplit_{ctr[0]}",
                            engine=inst.engine,
                            bass_nofuse=True,
                            sync_info=mybir.SyncInfo(
                                on_wait=rest[i : i + max_waits], on_update=[]
                            ),
                        )
                        out.append(nop)
                    inst.sync_info = mybir.SyncInfo(
                        on_wait=keep, on_update=list(si.on_update)
                    )
                out.append(inst)
            bb.instructions = out


# ---------------------------------------------------------------------------
# module construction
# ---------------------------------------------------------------------------

def build_module(maxiter=MAXITER, split_waits=True):
    nc = bass.Bass()
    dp = nc.declare_dram_parameter

    # ---- per-core batch-dependent inputs (small; uploaded every call) ----
    enc_in = dp("enc_in", [255, BPC], BF, isOutput=False)
    eps_t = dp("eps_t", [2, BPC], FP, isOutput=False)
    vxy0 = dp("vxy0", [1, 2 * BPC], FP, isOutput=False)  # [vx0 | vy0]
    ylu = dp("ylu", [1, 2 * BPC], FP, isOutput=False)    # [y_lb | y_ub]
    obsst = dp("obsst", [4, 2 * NB2], FP, isOutput=False)  # rows x,vx,y,vy; free=(s,o,b)

    # ---- replicated weights (bf16; device-cached across calls) ----
    w1 = dp("w1", [255, HID], BF, isOutput=False)
    b1 = dp("b1", [HID], FP, isOutput=False)
    w2 = dp("w2", [HID, HID], BF, isOutput=False)
    b2 = dp("b2", [HID], FP, isOutput=False)
    wml = dp("wml", [HID, 4], BF, isOutput=False)
    bmu = dp("bmu", [2], FP, isOutput=False)
    blvh = dp("blvh", [2], FP, isOutput=False)
    dw1 = dp("dw1", [57, HID], BF, isOutput=False)
    db1 = dp("db1", [HID], FP, isOutput=False)
    dw2 = dp("dw2", [HID, HID], BF, isOutput=False)
    db2 = dp("db2", [HID], FP, isOutput=False)
    dw3 = dp("dw3", [HID, 8], BF, isOutput=False)
    db3 = dp("db3", [8], FP, isOutput=False)

    # ---- replicated QP constants ----
    PTp = dp("PTp", [NVAR, NUM], FP, isOutput=False)          # P.T
    PdT = dp("PdT", [NVAR, NUM], FP, isOutput=False)
    PddT = dp("PddT", [NVAR, NUM], FP, isOutput=False)
    Pdk = dp("Pdk", [NUM, NVAR], BF, isOutput=False)
    Pddk = dp("Pddk", [NUM, NVAR], BF, isOutput=False)
    Pres = dp("Pres", [NUM, NVAR], BF, isOutput=False)        # -RO*P
    Plane = dp("Plane", [NUM, NVAR], BF, isOutput=False)      # RL*P
    J2x2 = dp("J2x2", [NVAR, NVAR], FP, isOutput=False)
    J2xn = dp("J2xn", [NVAR, NVAR], FP, isOutput=False)
    WJx = dp("WJx", [NVAR, NVAR], FP, isOutput=False)
    J2y2 = dp("J2y2", [NVAR, NVAR], FP, isOutput=False)
    J2yn = dp("J2yn", [NVAR, NVAR], FP, isOutput=False)
    WJy = dp("WJy", [NVAR, NVAR], FP, isOutput=False)
    Cx4 = dp("Cx4", [4, NVAR], FP, isOutput=False)
    Cy4 = dp("Cy4", [4, NVAR], FP, isOutput=False)
    Cx4J = dp("Cx4J", [4, NVAR], FP, isOutput=False)
    Cy4J = dp("Cy4J", [4, NVAR], FP, isOutput=False)
    I11 = dp("I11", [NVAR, NVAR], FP, isOutput=False)
    T2 = dp("T2", [2, NUM], FP, isOutput=False)               # rows: ones, t
    K4 = dp("K4", [1, 4 * NVAR], FP, isOutput=False)          # beq rank-1 rows

    out_t = dp("out_t", [2 * NVAR, BPC], FP, isOutput=True)

    mm = nc.tensor.matmul

    lp = nc.allow_low_precision(reason="bf16 obstacle chain is within tolerance")
    lp.__enter__()
    with _TC(nc) as tc:
        cst = tc.alloc_tile_pool(name="cst", bufs=1)
        act = tc.alloc_tile_pool(name="act", bufs=2)
        lam = tc.alloc_tile_pool(name="lam", bufs=3)
        psp = tc.alloc_tile_pool(name="ps", bufs=3, space="PSUM")

        dmae = [nc.sync, nc.gpsimd, nc.scalar]

        def cload(ap, shape, dtype=FP, tag=None, q=0):
            t = cst.tile(shape, dtype, tag=tag or ap.tensor.name)
            dmae[q % len(dmae)].dma_start(out=t[:], in_=ap)
            return t

        # ---------- constants into SBUF ----------
        sb_PTp = cload(PTp[:], [NVAR, NUM])
        sb_PdT = cload(PdT[:], [NVAR, NUM])
        sb_PddT = cload(PddT[:], [NVAR, NUM])
        sb_Pdk = cload(Pdk[:], [NUM, NVAR], BF)
        sb_Pddk = cload(Pddk[:], [NUM, NVAR], BF)
        sb_Pres = cload(Pres[:], [NUM, NVAR], BF)
        sb_Plane = cload(Plane[:], [NUM, NVAR], BF)
        sb_J2x2 = cload(J2x2[:], [NVAR, NVAR])
        sb_J2xn = cload(J2xn[:], [NVAR, NVAR])
        sb_WJx = cload(WJx[:], [NVAR, NVAR])
        sb_J2y2 = cload(J2y2[:], [NVAR, NVAR])
        sb_J2yn = cload(J2yn[:], [NVAR, NVAR])
        sb_WJy = cload(WJy[:], [NVAR, NVAR])
        sb_Cx4 = cload(Cx4[:], [4, NVAR])
        sb_Cy4 = cload(Cy4[:], [4, NVAR])
        sb_Cx4J = cload(Cx4J[:], [4, NVAR])
        sb_Cy4J = cload(Cy4J[:], [4, NVAR])
        sb_I11 = cload(I11[:], [NVAR, NVAR])
        sb_T2 = cload(T2[:], [2, NUM])
        sb_K4 = cload(K4[:], [1, 4 * NVAR])
        sb_vxy0 = cload(vxy0[:], [1, 2 * BPC], q=1)
        sb_ylu = cload(ylu[:], [1, 2 * BPC], q=1)
        sb_obsx = cload(obsst[0:2, :], [2, 2 * NB2], tag="obsx", q=2)
        sb_obsy = cload(obsst[2:4, :], [2, 2 * NB2], tag="obsy", q=2)

        # ---------- MLP weights into SBUF (spread across DMA queues) ----------
        sb_w1a = cload(w1[0:128, :], [128, HID], BF, tag="w1a", q=1)
        sb_w1b = cload(w1[128:255, :], [127, HID], BF, tag="w1b", q=2)
        sb_w2 = cst.tile([128, 8 * HID], BF, tag="w2")
        for kc in range(8):
            dmae[kc % 3].dma_start(
                out=sb_w2[:, kc * HID : (kc + 1) * HID],
                in_=w2[kc * 128 : (kc + 1) * 128, :],
            )
        sb_wml = cst.tile([128, 8 * 4], BF, tag="wml")
        for kc in range(8):
            nc.sync.dma_start(
                out=sb_wml[:, kc * 4 : (kc + 1) * 4],
                in_=wml[kc * 128 : (kc + 1) * 128, :],
            )
        sb_dw1 = cload(dw1[:], [57, HID], BF, q=3)
        sb_dw2 = cst.tile([128, 8 * HID], BF, tag="dw2")
        for kc in range(8):
            dmae[kc % 3].dma_start(
                out=sb_dw2[:, kc * HID : (kc + 1) * HID],
                in_=dw2[kc * 128 : (kc + 1) * 128, :],
            )
        sb_dw3 = cst.tile([128, 8 * 8], BF, tag="dw3")
        for kc in range(8):
            nc.gpsimd.dma_start(
                out=sb_dw3[:, kc * 8 : (kc + 1) * 8],
                in_=dw3[kc * 128 : (kc + 1) * 128, :],
            )
        sb_b1 = cload(b1[:].rearrange("(c p) -> c p", p=128).rearrange("c p -> p c"), [128, 8], q=3)
        sb_b2 = cload(b2[:].rearrange("(c p) -> c p", p=128).rearrange("c p -> p c"), [128, 8], q=3)
        sb_db1 = cload(db1[:].rearrange("(c p) -> c p", p=128).rearrange("c p -> p c"), [128, 8], q=3)
        sb_db2 = cload(db2[:].rearrange("(c p) -> c p", p=128).rearrange("c p -> p c"), [128, 8], q=3)
        sb_bmu = cload(bmu[:, None], [2, 1], q=3)
        sb_blvh = cload(blvh[:, None], [2, 1], q=3)
        sb_db3v = cload(db3[0:4][:, None], [4, 1], tag="db3v", q=3)
        sb_db3y = cload(db3[4:8][:, None], [4, 1], tag="db3y", q=3)

        sb_x1a = cload(enc_in[0:128, :], [128, BPC], BF, tag="x1a", q=1)
        sb_x1b = cload(enc_in[128:255, :], [127, BPC], BF, tag="x1b", q=1)
        sb_eps = cload(eps_t[:], [2, BPC], q=1)

        # ---------- obstacle trajectories + lane bounds on device ----------
        # xo[t, (s,o,b)] = x + t*vx  via K=2 matmul with stationary [ones; t]
        sb_xyobs = cst.tile([NUM, 2 * NB], BF, tag="xyobs")
        H2 = NB2 // 2  # 320, fits the psum_a slot
        for s in (0, 1):
            for a in (0, 1):
                for h in (0, 1):
                    ps = psp.tile([NUM, H2], FP, tag="psum_a", bufs=4,
                                  name=f"obs{s}{a}{h}")
                    src = sb_obsx if a == 0 else sb_obsy
                    mov = src[:, s * NB2 + h * H2 : s * NB2 + (h + 1) * H2]
                    mm(ps[:], sb_T2[:], mov, start=True, stop=True)
                    dst = (2 * s + a) * NB2 + h * H2
                    nc.scalar.activation(
                        sb_xyobs[:, dst : dst + H2], ps[:], AF.Copy
                    )
        # lane bounds broadcast along partitions: ones[1,100]^T @ lb[1,B]
        lane_ps = psp.tile([NUM, 2 * BPC], FP, tag="psum_b", bufs=4, name="lane_ps")
        mm(lane_ps[:, :BPC], sb_T2[0:1, :], sb_ylu[0:1, :BPC], start=True, stop=True,
           skip_group_check=True)
        mm(lane_ps[:, BPC:], sb_T2[0:1, :], sb_ylu[0:1, BPC:], start=True, stop=True,
           skip_group_check=True)
        sb_ylbc = cst.tile([NUM, BPC], BF, tag="ylbc")
        nc.scalar.activation(sb_ylbc[:], lane_ps[:, :BPC], AF.Copy)
        sb_yubc = cst.tile([NUM, BPC], BF, tag="yubc")
        nc.scalar.activation(sb_yubc[:], lane_ps[:, BPC:], AF.Copy)

        # ---------- MLP (bf16 weights/acts, fp32 PSUM) ----------
        h1 = act.tile([128, HID], BF, tag="h1", bufs=1)
        for jc in range(8):
            ps = psp.tile([128, BPC], FP, tag="psum_a", bufs=4)
            js = slice(jc * 128, (jc + 1) * 128)
            mm(ps[:], sb_w1a[:, js], sb_x1a[:], start=True, stop=False)
            mm(ps[:], sb_w1b[:, js], sb_x1b[:], start=False, stop=True)
            nc.scalar.activation(h1[:, js], ps[:], AF.Relu, bias=sb_b1[:, jc : jc + 1])
        h2 = act.tile([128, HID], BF, tag="h2", bufs=1)
        for jc in range(8):
            ps = psp.tile([128, BPC], FP, tag="psum_a", bufs=4)
            js = slice(jc * 128, (jc + 1) * 128)
            for kc in range(8):
                mm(
                    ps[:],
                    sb_w2[:, kc * HID + jc * 128 : kc * HID + (jc + 1) * 128],
                    h1[:, kc * 128 : (kc + 1) * 128],
                    start=(kc == 0),
                    stop=(kc == 7),
                )
            nc.scalar.activation(h2[:, js], ps[:], AF.Relu, bias=sb_b2[:, jc : jc + 1])
        mu_ps = psp.tile([2, BPC], FP, tag="psum_b", bufs=4)
        lv_ps = psp.tile([2, BPC], FP, tag="psum_b", bufs=4)
        for kc in range(8):
            mm(
                mu_ps[:],
                sb_wml[:, kc * 4 : kc * 4 + 2],
                h2[:, kc * 128 : (kc + 1) * 128],
                start=(kc == 0),
                stop=(kc == 7),
            )
            mm(
                lv_ps[:],
                sb_wml[:, kc * 4 + 2 : kc * 4 + 4],
                h2[:, kc * 128 : (kc + 1) * 128],
                start=(kc == 0),
                stop=(kc == 7),
            )
        # z = mu + bmu + exp(0.5*lv + 0.5*blv)*eps  -> rows 0:2 of dec_in (bf16)
        dec_in = act.tile([57, BPC], BF, tag="dec_in", bufs=1)
        nc.sync.dma_start(out=dec_in[2:57, :], in_=sb_x1a[0:55, :])
        std = act.tile([2, BPC], FP, tag="std", bufs=1)
        nc.scalar.activation(std[:], lv_ps[:], AF.Exp, bias=sb_blvh[:], scale=0.5)
        zz = act.tile([2, BPC], FP, tag="zz", bufs=1)
        nc.vector.tensor_mul(out=zz[:], in0=std[:], in1=sb_eps[:])
        zmu = act.tile([2, BPC], FP, tag="zmu", bufs=1)
        nc.vector.tensor_add(out=zmu[:], in0=zz[:], in1=mu_ps[:])
        nc.vector.tensor_scalar(
            out=dec_in[0:2, :], in0=zmu[:], scalar1=sb_bmu[:], scalar2=None,
            op0=OP.add,
        )
        g1 = act.tile([128, HID], BF, tag="g1", bufs=1)
        for jc in range(8):
            ps = psp.tile([128, BPC], FP, tag="psum_a", bufs=4)
            js = slice(jc * 128, (jc + 1) * 128)
            mm(ps[:], sb_dw1[:, js], dec_in[:], start=True, stop=True)
            nc.scalar.activation(g1[:, js], ps[:], AF.Relu, bias=sb_db1[:, jc : jc + 1])
        g2 = act.tile([128, HID], BF, tag="g2", bufs=1)
        for jc in range(8):
            ps = psp.tile([128, BPC], FP, tag="psum_a", bufs=4)
            js = slice(jc * 128, (jc + 1) * 128)
            for kc in range(8):
                mm(
                    ps[:],
                    sb_dw2[:, kc * HID + jc * 128 : kc * HID + (jc + 1) * 128],
                    g1[:, kc * 128 : (kc + 1) * 128],
                    start=(kc == 0),
                    stop=(kc == 7),
                )
            nc.scalar.activation(g2[:, js], ps[:], AF.Relu, bias=sb_db2[:, jc : jc + 1])
        nnv_ps = psp.tile([4, BPC], FP, tag="psum_b", bufs=4)
        nny_ps = psp.tile([4, BPC], FP, tag="psum_b", bufs=4)
        for kc in range(8):
            mm(
                nnv_ps[:],
                sb_dw3[:, kc * 8 : kc * 8 + 4],
                g2[:, kc * 128 : (kc + 1) * 128],
                start=(kc == 0),
                stop=(kc == 7),
            )
            mm(
                nny_ps[:],
                sb_dw3[:, kc * 8 + 4 : kc * 8 + 8],
                g2[:, kc * 128 : (kc + 1) * 128],
                start=(kc == 0),
                stop=(kc == 7),
            )
        nn_v = act.tile([4, BPC], FP, tag="nn_v", bufs=1)
        nc.scalar.activation(nn_v[:], nnv_ps[:], AF.Identity, bias=sb_db3v[:])
        nn_y = act.tile([4, BPC], FP, tag="nn_y", bufs=1)
        nc.scalar.activation(nn_y[:], nny_ps[:], AF.Identity, bias=sb_db3y[:])

        # ---------- QP setup ----------
        # b_eq terms are rank-1 (only vx0/vy0 nonzero): K4 rows are the
        # corresponding KKT-inverse rows, applied as K=1 matmuls.
        st_ps = psp.tile([NVAR, 2 * BPC], FP, tag="psum_b", bufs=4)
        mm(st_ps[:, :BPC], sb_Cx4[:], nn_v[:], start=True, stop=False)
        mm(st_ps[:, :BPC], sb_K4[0:1, 0:NVAR], sb_vxy0[0:1, :BPC],
           start=False, stop=True)
        mm(st_ps[:, BPC:], sb_Cy4[:], nn_y[:], start=True, stop=False)
        mm(st_ps[:, BPC:], sb_K4[0:1, NVAR : 2 * NVAR], sb_vxy0[0:1, BPC:],
           start=False, stop=True)
        cxy = lam.tile([NVAR, 2 * BPC], FP, tag="cxy")
        nc.scalar.activation(cxy[:], st_ps[:], AF.Copy)

        sC_ps = psp.tile([NVAR, 2 * BPC], FP, tag="psum_b", bufs=4)
        mm(sC_ps[:, :BPC], sb_Cx4J[:], nn_v[:], start=True, stop=False)
        mm(sC_ps[:, :BPC], sb_K4[0:1, 2 * NVAR : 3 * NVAR], sb_vxy0[0:1, :BPC],
           start=False, stop=True)
        mm(sC_ps[:, BPC:], sb_Cy4J[:], nn_y[:], start=True, stop=False)
        mm(sC_ps[:, BPC:], sb_K4[0:1, 3 * NVAR : 4 * NVAR], sb_vxy0[0:1, BPC:],
           start=False, stop=True)
        CXC = cst.tile([NVAR, 2 * BPC], FP, tag="CXC")
        nc.scalar.activation(CXC[:], sC_ps[:], AF.Copy)

        sb_tiny = cst.tile([128, 1], FP, tag="tiny")
        nc.vector.memset(sb_tiny[:], 1e-30)
        sb_negone = cst.tile([128, 1], FP, tag="negone")
        nc.vector.memset(sb_negone[:], -1.0)

        # ---------- split into two independent 64-row streams ----------
        cxyS = []
        CXCS = []
        lxyS = []
        for s in (0, 1):
            cs = lam.tile([NVAR, 2 * B2], FP, tag=f"cxy{s}", name=f"cxyS{s}")
            nc.scalar.activation(
                cs[:].rearrange("p (a b) -> p a b", a=2),
                cxy[:].rearrange("p (a b) -> p a b", a=2)[:, :, s * B2 : (s + 1) * B2],
                AF.Copy,
            )
            cxyS.append(cs)
            xc = cst.tile([NVAR, 2 * B2], FP, tag=f"CXC{s}", name=f"CXCS{s}")
            nc.scalar.activation(
                xc[:].rearrange("p (a b) -> p a b", a=2),
                CXC[:].rearrange("p (a b) -> p a b", a=2)[:, :, s * B2 : (s + 1) * B2],
                AF.Copy,
            )
            CXCS.append(xc)
            lx = lam.tile([NVAR, 2 * B2], FP, tag=f"lxy{s}", name=f"lxyS{s}")
            nc.vector.memset(lx[:], 0.0)
            lxyS.append(lx)

        # ---------- QP iterations (two interleaved streams) ----------
        def qp_iter(s):
            cxy_s = cxyS[s]
            lxy_s = lxyS[s]
            obs = sb_xyobs[:, s * 2 * NB2 : (s + 1) * 2 * NB2]
            fwd = psp.tile([NUM, 6 * B2], FP, tag="psum_a", bufs=4, name=f"fwd{s}")
            xy = fwd[:, 0 : 2 * B2]
            xyd = fwd[:, 2 * B2 : 4 * B2]
            xydd = fwd[:, 4 * B2 : 6 * B2]
            mm(xy, sb_PTp[:], cxy_s[:], start=True, stop=True, skip_group_check=True)
            mm(xyd, sb_PdT[:], cxy_s[:], start=True, stop=True, skip_group_check=True)
            mm(xydd, sb_PddT[:], cxy_s[:], start=True, stop=True, skip_group_check=True)

            xys = act.tile([NUM, 2 * B2], BF, tag=f"xys{s}", name=f"xys_{s}")
            nc.scalar.activation(xys[:], xy, AF.Copy)
            w_s = act.tile([NUM, 2 * NB2], BF, tag=f"w_s{s}", name=f"w_s_{s}")
            nc.vector.tensor_tensor(
                out=w_s[:].rearrange("p (a o b) -> p a o b", a=2, o=NOBS),
                in0=xys[:]
                .rearrange("p (a b) -> p a b", a=2)[:, :, None, :]
                .to_broadcast((NUM, 2, NOBS, B2)),
                in1=obs.rearrange("p (a o b) -> p a o b", a=2, o=NOBS),
                op=OP.subtract,
            )

            sqw = act.tile([NUM, 2 * NB2], BF, tag=f"sqw{s}", name=f"sqw_{s}")
            nc.scalar.activation(sqw[:, :NB2], w_s[:, :NB2], AF.Square, scale=B_OBS)
            nc.scalar.activation(sqw[:, NB2:], w_s[:, NB2:], AF.Square, scale=A_OBS)
            sqd = act.tile([NUM, 2 * B2], BF, tag=f"sqd{s}", name=f"sqd_{s}")
            nc.scalar.activation(sqd[:], xyd, AF.Square)
            sqdd = act.tile([NUM, 2 * B2], BF, tag=f"sqdd{s}", name=f"sqdd_{s}")
            nc.scalar.activation(sqdd[:], xydd, AF.Square)

            r2all = act.tile([NUM, NB2 + 2 * B2], BF, tag=f"r2all{s}", name=f"r2all_{s}")
            nc.vector.tensor_add(out=r2all[:, :NB2], in0=sqw[:, :NB2], in1=sqw[:, NB2:])
            nc.vector.tensor_add(
                out=r2all[:, NB2 : NB2 + B2], in0=sqd[:, :B2], in1=sqd[:, B2:]
            )
            nc.vector.tensor_add(
                out=r2all[:, NB2 + B2 :], in0=sqdd[:, :B2], in1=sqdd[:, B2:]
            )
            r_all = act.tile([NUM, NB2 + 2 * B2], BF, tag=f"r_all{s}", name=f"r_all_{s}")
            nc.scalar.activation(r_all[:], r2all[:], AF.Sqrt, bias=sb_tiny[0:NUM, :])
            rinv = act.tile([NUM, NB2 + 2 * B2], BF, tag=f"rinv{s}", name=f"rinv_{s}")
            nc.vector.reciprocal(out=rinv[:], in_=r_all[:])

            qf = act.tile([NUM, NB2], BF, tag=f"qf{s}", name=f"qf_{s}")
            nc.scalar.activation(
                qf[:], rinv[:, :NB2], AF.Relu, bias=sb_negone[0:NUM, :], scale=AB
            )
            e_s = act.tile([NUM, 2 * NB2], BF, tag=f"e_s{s}", name=f"e_s_{s}")
            nc.vector.tensor_mul(out=e_s[:, :NB2], in0=qf[:], in1=w_s[:, :NB2])
            nc.vector.tensor_mul(out=e_s[:, NB2:], in0=qf[:], in1=w_s[:, NB2:])

            hva = act.tile([NUM, 2 * B2], FP, tag=f"hva{s}", name=f"hva_{s}")
            nc.vector.tensor_scalar(
                out=hva[:, :B2], in0=r_all[:, NB2 : NB2 + B2],
                scalar1=VMIN, scalar2=VMAX, op0=OP.max, op1=OP.min,
            )
            nc.vector.tensor_scalar(
                out=hva[:, B2:], in0=r_all[:, NB2 + B2 :],
                scalar1=AMAX, scalar2=None, op0=OP.min,
            )
            gva = act.tile([NUM, 2 * B2], FP, tag=f"gva{s}", name=f"gva_{s}")
            nc.vector.tensor_mul(out=gva[:], in0=hva[:], in1=rinv[:, NB2:])
            gvam = act.tile([NUM, 2 * B2], FP, tag=f"gvam{s}", name=f"gvam_{s}")
            nc.vector.tensor_scalar(
                out=gvam[:], in0=gva[:], scalar1=-RI, scalar2=RI,
                op0=OP.mult, op1=OP.add,
            )

            ys = xys[:, B2:]
            lt1 = act.tile([NUM, B2], BF, tag=f"lt1{s}", name=f"lt1_{s}")
            nc.vector.tensor_max(
                out=lt1[:], in0=ys, in1=sb_ylbc[:, s * B2 : (s + 1) * B2]
            )
            lt2 = act.tile([NUM, B2], BF, tag=f"lt2{s}", name=f"lt2_{s}")
            nc.vector.tensor_tensor(
                out=lt2[:], in0=lt1[:], in1=sb_yubc[:, s * B2 : (s + 1) * B2], op=OP.min
            )
            d12 = act.tile([NUM, B2], BF, tag=f"d12{s}", name=f"d12_{s}")
            nc.vector.tensor_sub(out=d12[:], in0=ys, in1=lt2[:])

            rvxy = act.tile([NUM, 2 * B2], BF, tag=f"rvxy{s}", name=f"rvxy_{s}")
            nc.vector.tensor_tensor(
                out=rvxy[:].rearrange("p (a b) -> p a b", a=2),
                in0=gvam[:, 0:B2][:, None, :].to_broadcast((NUM, 2, B2)),
                in1=xyd.rearrange("p (a b) -> p a b", a=2),
                op=OP.mult,
            )
            raxy = act.tile([NUM, 2 * B2], BF, tag=f"raxy{s}", name=f"raxy_{s}")
            nc.vector.tensor_tensor(
                out=raxy[:].rearrange("p (a b) -> p a b", a=2),
                in0=gvam[:, B2:][:, None, :].to_broadcast((NUM, 2, B2)),
                in1=xydd.rearrange("p (a b) -> p a b", a=2),
                op=OP.mult,
            )

            Dx = psp.tile([NVAR, B2], FP, tag="psum_b", bufs=4, name=f"Dx{s}")
            for o in range(NOBS):
                mm(
                    Dx[:],
                    sb_Pres[:],
                    e_s[:, o * B2 : (o + 1) * B2],
                    start=(o == 0),
                    stop=False,
                )
            mm(Dx[:], sb_Pddk[:], raxy[:, :B2], start=False, stop=False)
            mm(Dx[:], sb_Pdk[:], rvxy[:, :B2], start=False, stop=True)

            Dy = psp.tile([NVAR, B2], FP, tag="psum_b", bufs=4, name=f"Dy{s}")
            for o in range(NOBS):
                mm(
                    Dy[:],
                    sb_Pres[:],
                    e_s[:, NB2 + o * B2 : NB2 + (o + 1) * B2],
                    start=(o == 0),
                    stop=False,
                )
            mm(Dy[:], sb_Pddk[:], raxy[:, B2:], start=False, stop=False)
            mm(Dy[:], sb_Pdk[:], rvxy[:, B2:], start=False, stop=False)
            mm(Dy[:], sb_Plane[:], d12[:], start=False, stop=True)

            lxy_new = lam.tile([NVAR, 2 * B2], FP, tag=f"lxy{s}", name=f"lxyn_{s}")
            nc.vector.tensor_sub(out=lxy_new[:, :B2], in0=lxy_s[:, :B2], in1=Dx[:])
            nc.vector.tensor_sub(out=lxy_new[:, B2:], in0=lxy_s[:, B2:], in1=Dy[:])

            cn = psp.tile([NVAR, 2 * B2], FP, tag="psum_b", bufs=4, name=f"cn{s}")
            mm(cn[:, :B2], sb_J2x2[:], lxy_new[:, :B2], start=True, stop=False)
            mm(cn[:, :B2], sb_J2xn[:], lxy_s[:, :B2], start=False, stop=False)
            mm(cn[:, :B2], sb_WJx[:], cxy_s[:, :B2], start=False, stop=False)
            mm(cn[:, :B2], sb_I11[:], CXCS[s][:, :B2], start=False, stop=True)
            mm(cn[:, B2:], sb_J2y2[:], lxy_new[:, B2:], start=True, stop=False)
            mm(cn[:, B2:], sb_J2yn[:], lxy_s[:, B2:], start=False, stop=False)
            mm(cn[:, B2:], sb_WJy[:], cxy_s[:, B2:], start=False, stop=False)
            mm(cn[:, B2:], sb_I11[:], CXCS[s][:, B2:], start=False, stop=True)

            cxy_new = lam.tile([NVAR, 2 * B2], FP, tag=f"cxy{s}", name=f"cxyn_{s}")
            nc.scalar.activation(cxy_new[:], cn[:], AF.Copy)
            cxyS[s] = cxy_new
            lxyS[s] = lxy_new

        for it in range(maxiter):
            qp_iter(0)
            qp_iter(1)

        for s in (0, 1):
            nc.sync.dma_start(
                out=out_t[:].rearrange("(h k) b -> k h b", h=2)[
                    :, :, s * B2 : (s + 1) * B2
                ],
                in_=cxyS[s][:].rearrange("p (h b) -> p h b", h=2),
            )

        psp.release()
        lam.release()
        act.release()
        cst.release()

    lp.__exit__(None, None, None)
    if split_waits:
        _split_sync_waits(nc)
    return nc


# ---------------------------------------------------------------------------
# host preprocessing
# ---------------------------------------------------------------------------

# inputs whose values flow into the device-cached constant tensors
_CONST_SRC = (
    "P", "Pdot", "Pddot",
    "enc_w1", "enc_b1", "enc_w2", "enc_b2",
    "enc_wmu", "enc_bmu", "enc_wlv", "enc_blv",
    "dec_w1", "dec_b1", "dec_w2", "dec_b2", "dec_w3", "dec_b3",
)


def _kkt_inv(cost, A_eq):
    m = A_eq.shape[0]
    n = cost.shape[0]
    M = np.zeros((n + m, n + m), np.float64)
    M[:n, :n] = cost
    M[:n, n:] = A_eq.T
    M[n:, :n] = A_eq
    return np.linalg.inv(M).astype(f32)


def prep_consts(inputs):
    """Per-core (replicated) constant tensors, as numpy arrays."""
    P = np.asarray(inputs["P"], f32)
    Pd = np.asarray(inputs["Pdot"], f32)
    Pdd = np.asarray(inputs["Pddot"], f32)
    t = np.linspace(0.0, T_FIN, NUM).astype(f32)

    A_eq_x = np.stack([P[0], Pd[0], Pdd[0]])
    A_eq_y = np.stack([P[0], Pd[0], Pdd[0], Pd[-1]])
    K_D = f32(2.0 * np.sqrt(np.float32(20.0)))
    A_pd = Pdd - f32(20.0) * P - K_D * Pd
    A_vd = Pdd - f32(20.0) * Pd
    cs = Pdd.T @ Pdd
    inv1_x = _kkt_inv(cs + A_vd.T @ A_vd, A_eq_x)
    inv1_y = _kkt_inv(cs + A_pd.T @ A_pd, A_eq_y)
    PtP = P.T @ P
    PdTPd = Pd.T @ Pd
    PddTPdd = Pdd.T @ Pdd
    I = np.eye(NVAR, dtype=f32)
    cost2_x = RP * I + RO * 10.0 * PtP + RI * PddTPdd + RI * PdTPd
    cost2_y = cost2_x + RL * 2.0 * PtP
    inv2_x = _kkt_inv(cost2_x, A_eq_x)
    inv2_y = _kkt_inv(cost2_y, A_eq_y)

    J1x = inv1_x[:NVAR, :NVAR].T
    K1x = inv1_x[:NVAR, NVAR:].T
    J1y = inv1_y[:NVAR, :NVAR].T
    K1y = inv1_y[:NVAR, NVAR:].T
    J2x = inv2_x[:NVAR, :NVAR].T
    K2x = inv2_x[:NVAR, NVAR:].T
    J2y = inv2_y[:NVAR, :NVAR].T
    K2y = inv2_y[:NVAR, NVAR:].T

    Avd_bs = A_vd.reshape(4, 25, NVAR).sum(1)
    Apd_bs = A_pd.reshape(4, 25, NVAR).sum(1)
    Cx4 = (f32(-20.0) * (Avd_bs @ J1x)).astype(f32)
    Cy4 = (f32(-20.0) * (Apd_bs @ J1y)).astype(f32)
    Wx = (f32(10.0 * RO) * PtP + f32(RI) * (PddTPdd + PdTPd)).astype(f32)
    Wy = (Wx + f32(2.0 * RL) * PtP).astype(f32)

    # rank-1 b_eq rows: b_eq_x = [0, vx0, 0], b_eq_y = [0, vy0, 0, 0]
    K4 = np.concatenate(
        [
            K1x[1],
            K1y[1],
            (K1x @ J2x + K2x)[1],
            (K1y @ J2y + K2y)[1],
        ]
    ).astype(f32)[None, :]
    T2 = np.stack([np.ones(NUM, f32), t]).astype(f32)

    return {
        "PTp": np.ascontiguousarray(P.T.astype(f32)),
        "PdT": np.ascontiguousarray(Pd.T.astype(f32)),
        "PddT": np.ascontiguousarray(Pdd.T.astype(f32)),
        "Pdk": np.ascontiguousarray(Pd.astype(bf16)),
        "Pddk": np.ascontiguousarray(Pdd.astype(bf16)),
        "Pres": np.ascontiguousarray((-RO * P).astype(bf16)),
        "Plane": np.ascontiguousarray((RL * P).astype(bf16)),
        "J2x2": np.ascontiguousarray((2.0 * J2x).astype(f32)),
        "J2xn": np.ascontiguousarray((-J2x).astype(f32)),
        "WJx": np.ascontiguousarray((Wx @ J2x).astype(f32)),
        "J2y2": np.ascontiguousarray((2.0 * J2y).astype(f32)),
        "J2yn": np.ascontiguousarray((-J2y).astype(f32)),
        "WJy": np.ascontiguousarray((Wy @ J2y).astype(f32)),
        "Cx4": Cx4,
        "Cy4": Cy4,
        "Cx4J": np.ascontiguousarray((Cx4 @ J2x).astype(f32)),
        "Cy4J": np.ascontiguousarray((Cy4 @ J2y).astype(f32)),
        "I11": I,
        "T2": T2,
        "K4": K4,
        "w1": np.ascontiguousarray(np.asarray(inputs["enc_w1"], f32).astype(bf16)),
        "b1": np.ascontiguousarray(np.asarray(inputs["enc_b1"], f32)),
        "w2": np.ascontiguousarray(np.asarray(inputs["enc_w2"], f32).astype(bf16)),
        "b2": np.ascontiguousarray(np.asarray(inputs["enc_b2"], f32)),
        "wml": np.ascontiguousarray(
            np.concatenate(
                [np.asarray(inputs["enc_wmu"], f32), np.asarray(inputs["enc_wlv"], f32)],
                axis=1,
            ).astype(bf16)
        ),
        "bmu": np.ascontiguousarray(np.asarray(inputs["enc_bmu"], f32)),
        "blvh": np.ascontiguousarray((0.5 * np.asarray(inputs["enc_blv"], f32))),
        "dw1": np.ascontiguousarray(np.asarray(inputs["dec_w1"], f32).astype(bf16)),
        "db1": np.ascontiguousarray(np.asarray(inputs["dec_b1"], f32)),
        "dw2": np.ascontiguousarray(np.asarray(inputs["dec_w2"], f32).astype(bf16)),
        "db2": np.ascontiguousarray(np.asarray(inputs["dec_b2"], f32)),
        "dw3": np.ascontiguousarray(np.asarray(inputs["dec_w3"], f32).astype(bf16)),
        "db3": np.ascontiguousarray(np.asarray(inputs["dec_b3"], f32)),
    }


def prep_batch_global(inputs):
    """Batch-dependent inputs as concatenated (ncores*rows, cols) arrays."""
    inp = np.asarray(inputs["inp"], f32)
    B = inp.shape[0]

    inp_mean = np.asarray(inputs["inp_mean"], f32)
    inp_std = np.asarray(inputs["inp_std"], f32)
    inp_n = (inp - inp_mean) / inp_std
    enc_full = np.concatenate(
        [inp_n, np.asarray(inputs["traj_gt"], f32)], axis=1
    ).T.astype(bf16)                                         # (255, B)
    enc_in = np.ascontiguousarray(
        enc_full.reshape(255, NCORES, BPC).transpose(1, 0, 2).reshape(
            NCORES * 255, BPC
        )
    )

    def core_rows(a):
        # (r, B) -> (ncores*r, BPC) grouping batch columns per core
        r = a.shape[0]
        return np.ascontiguousarray(
            a.reshape(r, NCORES, BPC).transpose(1, 0, 2).reshape(NCORES * r, BPC)
        )

    eps_t = core_rows(np.asarray(inputs["eps"], f32).T)
    ise = np.asarray(inputs["initial_state_ego"], f32)
    # per-core single row: [vx0 (BPC) | vy0 (BPC)]
    vxy0 = np.ascontiguousarray(
        ise[:, 2:4].T.reshape(2, NCORES, BPC).transpose(1, 0, 2).reshape(
            NCORES, 2 * BPC
        )
    )
    ylu = np.ascontiguousarray(
        np.stack([np.asarray(inputs["y_lb"], f32), np.asarray(inputs["y_ub"], f32)])
        .reshape(2, NCORES, BPC).transpose(1, 0, 2).reshape(NCORES, 2 * BPC)
    )

    # obstacle states: rows (x, vx, y, vy), free = (stream, obstacle, b64)
    X4 = np.stack(
        [inp[:, 5::5], inp[:, 7::5], inp[:, 6::5], inp[:, 8::5]]
    )                                                        # (4, B, NOBS)
    obsst = np.ascontiguousarray(
        X4.reshape(4, NCORES, 2, B2, NOBS)
        .transpose(1, 0, 2, 4, 3)                            # (c, 4, s, o, b)
        .reshape(NCORES * 4, 2 * NB2)
    )
    return {
        "enc_in": enc_in,
        "eps_t": eps_t,
        "vxy0": vxy0,
        "ylu": ylu,
        "obsst": obsst,
    }


def host_prep(inputs):
    """Per-core input dicts (CoreSim / debugging path)."""
    consts = prep_consts(inputs)
    gb = prep_batch_global(inputs)
    in_maps = []
    for c in range(NCORES):
        m = dict(consts)
        for k, g in gb.items():
            r = g.shape[0] // NCORES
            m[k] = np.ascontiguousarray(g[c * r : (c + 1) * r])
        in_maps.append(m)
    return in_maps


# ---------------------------------------------------------------------------
# persistent runtime: jitted executable + device-resident constants
# ---------------------------------------------------------------------------

class _Runtime:
    def __init__(self):
        install_neuronx_cc_hook()
        nc = build_module()
        self.nc = nc
        assert nc.dbg_addr is None or not nc.dbg_callbacks

        partition_name = (
            nc.partition_id_tensor.name if nc.partition_id_tensor else None
        )
        in_names, out_names, out_avals, zero_shapes = [], [], [], []
        for alloc in nc.m.functions[0].allocations:
            if not isinstance(alloc, mybir.MemoryLocationSet):
                continue
            name = alloc.memorylocations[0].name
            if alloc.kind == "ExternalInput":
                if name != partition_name:
                    in_names.append(name)
            elif alloc.kind == "ExternalOutput":
                shape = tuple(alloc.tensor_shape)
                dtype = mybir.dt.np(alloc.dtype)
                out_avals.append(jax.core.ShapedArray(shape, dtype))
                zero_shapes.append((shape, dtype))
                out_names.append(name)
        self.in_names = in_names
        self.out_names = out_names
        self.zero_shapes = zero_shapes
        n_params = len(in_names)
        n_outs = len(out_names)
        in_names_full = in_names + out_names + (
            [partition_name] if partition_name else []
        )

        def _body(*args):
            operands = list(args)
            if partition_name is not None:
                operands.append(partition_id_tensor())
            outs = _bass_exec_p.bind(
                *operands,
                out_avals=tuple(out_avals),
                in_names=tuple(in_names_full),
                out_names=tuple(out_names),
                lowering_input_output_aliases=(),
                sim_require_finite=True,
                sim_require_nnan=True,
                nc=nc,
            )
            return tuple(outs)

        devices = jax.devices()[:NCORES]
        assert len(devices) == NCORES
        self.mesh = Mesh(np.asarray(devices), ("core",))
        self.sharding = NamedSharding(self.mesh, PartitionSpec("core"))
        donate = tuple(range(n_params, n_params + n_outs))
        self.fn = jax.jit(
            shard_map(
                _body,
                mesh=self.mesh,
                in_specs=(PartitionSpec("core"),) * (n_params + n_outs),
                out_specs=(PartitionSpec("core"),) * n_outs,
                check_rep=False,
            ),
            donate_argnums=donate,
            keep_unused=True,
        )

        self.const_ids = None
        self.const_crc = None
        self.dev_consts = None

    def _const_key_fast(self, inputs):
        return tuple(id(inputs[k]) for k in _CONST_SRC)

    def _const_crc(self, inputs):
        h = 0
        for k in _CONST_SRC:
            a = np.ascontiguousarray(np.asarray(inputs[k]))
            h = zlib.crc32(a, h)
        return h

    def get_consts(self, inputs):
        ids = self._const_key_fast(inputs)
        if self.dev_consts is not None and ids == self.const_ids:
            return self.dev_consts
        crc = self._const_crc(inputs)
        if self.dev_consts is not None and crc == self.const_crc:
            self.const_ids = ids
            return self.dev_consts
        consts = prep_consts(inputs)
        dev = {}
        for name, a in consts.items():
            g = np.ascontiguousarray(
                np.broadcast_to(a, (NCORES,) + a.shape).reshape(
                    NCORES * a.shape[0], *a.shape[1:]
                )
            )
            dev[name] = jax.device_put(g, self.sharding)
        for v in dev.values():
            v.block_until_ready()
        self.const_ids = ids
        self.const_crc = crc
        self.dev_consts = dev
        return dev


_RT = None


def kernel(**inputs):
    global _RT
    if _RT is None:
        _RT = _Runtime()
    rt = _RT
    dev_consts = rt.get_consts(inputs)
    batch = prep_batch_global(inputs)
    args = [
        dev_consts[n] if n in dev_consts else batch[n] for n in rt.in_names
    ]
    zeros = [np.zeros((NCORES * s[0], *s[1:]), d) for s, d in rt.zero_shapes]
    outs = rt.fn(*args, *zeros)
    out = outs[0]
    try:
        out.copy_to_host_async()
    except Exception:
        pass
    a = np.asarray(out)                                  # (8*22, BPC)
    # rows per core: [h=0 (x) vars 0..10, h=1 (y) vars 0..10]; cols = batch
    res = a.reshape(NCORES, 2 * NVAR, BPC).transpose(0, 2, 1).reshape(
        NCORES * BPC, 2 * NVAR
    )
    return np.ascontiguousarray(res)


# revision 11
# speedup vs baseline: 28.4698x; 1.8873x over previous
"""Trainium2 Bass kernel for nn_Beta_cVAE (cVAE encoder/decoder + 2-layer QP
trajectory projection, 20 ADMM iterations).

Strategy: pure data-parallel over 8 NeuronCores (128 batch rows each).
On-device layout is fully transposed: batch on the free dim, time/NVAR on
partitions, so every matmul uses weights/basis as the stationary operand and
no transposes appear anywhere.  All arctan2/cos/sin of the reference are
eliminated algebraically (cos(atan2(a,b)) = b/hypot(a,b)): the obstacle
ellipse projection becomes a residual scaling  resid = w*min(r-AB,0)/r  that
is *exactly zero* for inactive obstacles (no large-term cancellation, which
makes the bf16 obstacle pipeline safe), velocity/accel projections become
clip-scalings, and the lane term reduces to y - clip(y, lb, ub).  Obstacle
sums over the 10 obstacles fold into PE matmul accumulations with a shared
stationary operand.  MLP runs in bf16 (fp32 PSUM accumulation).

Host/runtime side: the expensive parts of a call (jit trace/lower, NEFF
load, 44 MB weight upload over the axon tunnel) are all cached across calls:
the jitted shard_map executable is built once, the replicated weights /
QP constants are device-resident keyed by a crc32 of the weight arrays, and
per-call traffic is only the ~0.8 MB of batch-dependent inputs (encoder
input, eps, initial velocities, lane bounds, obstacle states — obstacle
*trajectories* and boundary-condition expansions are computed on device).
The output is fetched with copy_to_host_async so the D2H ride shares the
dispatch round-trip.
"""
import sys
import os
import zlib

sys.path.insert(0, "/opt/trn_rl_repo")

import numpy as np
import ml_dtypes
import jax

import concourse.bass as bass
import concourse.mybir as mybir
from concourse.tile import TileContext, ScopedClock
from concourse.bass2jax import (
    _bass_exec_p,
    install_neuronx_cc_hook,
    partition_id_tensor,
)
from jax.experimental.shard_map import shard_map
from jax.sharding import Mesh, NamedSharding, PartitionSpec

f32 = np.float32
bf16 = ml_dtypes.bfloat16
FP = mybir.dt.float32
FR = mybir.dt.float32r
BF = mybir.dt.bfloat16
AF = mybir.ActivationFunctionType
OP = mybir.AluOpType

NCORES = 8
BPC = 128          # batch rows per core
NUM, NVAR, NOBS = 100, 11, 10
NB = NOBS * BPC    # 1280
B2 = BPC // 2      # 64 (per interleaved stream)
NB2 = NOBS * B2    # 640
A_OBS, B_OBS = 8.0, 4.2
AB = A_OBS * B_OBS
RO, RI, RL, RP = 100.0, 100.0, 100.0, 1.0
VMIN, VMAX, AMAX = 0.1, 30.0, 8.0
MAXITER = 20
HID = 1024
T_FIN = 15.0


class _TC(TileContext):
    """TileContext whose tail drain splits sem waits: this walrus build's
    TPB_CTRL codegen accepts only one sync wait per instruction."""

    def _drain_and_barrier(self, tick_clock, wait_clock):
        drain_inst = self.nc.sync.drain()
        wait_clock.add_sem_waits(
            drain_inst.ins, ScopedClock({None: tick_clock.global_clock})
        )
        si = drain_inst.ins.sync_info
        waits = list(si.on_wait) if si is not None else []
        if len(waits) > 1:
            drain_inst.ins.sync_info = mybir.SyncInfo(
                on_wait=waits[:1], on_update=list(si.on_update)
            )
            for w in waits[1:]:
                nop = self.nc.sync.nop(nofuse=True, hint="split_drain_wait")
                nop.ins.sync_info = mybir.SyncInfo(on_wait=[w], on_update=[])

        self.nc.all_engine_barrier()
        assert self.sems is not None
        popped = self.nc._tile_sem_poison_stack.pop()
        assert popped is self._sem_poison
        self.nc.clear_and_free_semaphores(list(self.sems.allocated().values()))
        self.nc.all_engine_barrier()


def _split_sync_waits(nc, max_waits=1):
    """Walrus's CoreV3 codegen accepts few sync waits per instruction; hoist
    surplus waits onto same-engine nops inserted just before the offender."""
    ctr = [0]
    for f in nc.m.functions:
        for bb in f.blocks:
            insts = list(bb.instructions)
            out = []
            for inst in insts:
                si = getattr(inst, "sync_info", None)
                if si is not None and len(si.on_wait) > max_waits:
                    waits = list(si.on_wait)
                    keep = waits[-max_waits:]
                    rest = waits[: len(waits) - max_waits]
                    for i in range(0, len(rest), max_waits):
                        ctr[0] += 1
                        nop = mybir.InstNoOp(
                            name=f"swsplit_{ctr[0]}",
                            engine=inst.engine,
                            bass_nofuse=True,
                            sync_info=mybir.SyncInfo(
                                on_wait=rest[i : i + max_waits], on_update=[]
                            ),
                        )
                        out.append(nop)
                    inst.sync_info = mybir.SyncInfo(
                        on_wait=keep, on_update=list(si.on_update)
                    )
                out.append(inst)
            bb.instructions = out


# ---------------------------------------------------------------------------
# module construction
# ---------------------------------------------------------------------------

def build_module(maxiter=MAXITER, split_waits=True):
    nc = bass.Bass()
    dp = nc.declare_dram_parameter

    # ---- per-core batch-dependent inputs (small; uploaded every call) ----
    enc_in = dp("enc_in", [255, BPC], BF, isOutput=False)
    eps_t = dp("eps_t", [2, BPC], FP, isOutput=False)
    vxy0 = dp("vxy0", [1, 2 * BPC], FP, isOutput=False)  # [vx0 | vy0]
    ylu = dp("ylu", [1, 2 * BPC], FP, isOutput=False)    # [y_lb | y_ub]
    obsst = dp("obsst", [4, 2 * NB2], FP, isOutput=False)  # rows x,vx,y,vy; free=(s,o,b)

    # ---- replicated weights (bf16; device-cached across calls) ----
    w1 = dp("w1", [255, HID], BF, isOutput=False)
    b1 = dp("b1", [HID], FP, isOutput=False)
    w2 = dp("w2", [HID, HID], BF, isOutput=False)
    b2 = dp("b2", [HID], FP, isOutput=False)
    wml = dp("wml", [HID, 4], BF, isOutput=False)
    bmu = dp("bmu", [2], FP, isOutput=False)
    blvh = dp("blvh", [2], FP, isOutput=False)
    dw1 = dp("dw1", [57, HID], BF, isOutput=False)
    db1 = dp("db1", [HID], FP, isOutput=False)
    dw2 = dp("dw2", [HID, HID], BF, isOutput=False)
    db2 = dp("db2", [HID], FP, isOutput=False)
    dw3 = dp("dw3", [HID, 8], BF, isOutput=False)
    db3 = dp("db3", [8], FP, isOutput=False)

    # ---- replicated QP constants ----
    PTp = dp("PTp", [NVAR, NUM], FP, isOutput=False)          # P.T
    PdT = dp("PdT", [NVAR, NUM], FP, isOutput=False)
    PddT = dp("PddT", [NVAR, NUM], FP, isOutput=False)
    Pdk = dp("Pdk", [NUM, NVAR], BF, isOutput=False)
    Pddk = dp("Pddk", [NUM, NVAR], BF, isOutput=False)
    Pres = dp("Pres", [NUM, NVAR], BF, isOutput=False)        # -RO*P
    Plane = dp("Plane", [NUM, NVAR], BF, isOutput=False)      # RL*P
    J2x2 = dp("J2x2", [NVAR, NVAR], FP, isOutput=False)
    J2xn = dp("J2xn", [NVAR, NVAR], FP, isOutput=False)
    WJx = dp("WJx", [NVAR, NVAR], FP, isOutput=False)
    J2y2 = dp("J2y2", [NVAR, NVAR], FP, isOutput=False)
    J2yn = dp("J2yn", [NVAR, NVAR], FP, isOutput=False)
    WJy = dp("WJy", [NVAR, NVAR], FP, isOutput=False)
    Cx4 = dp("Cx4", [4, NVAR], FP, isOutput=False)
    Cy4 = dp("Cy4", [4, NVAR], FP, isOutput=False)
    Cx4J = dp("Cx4J", [4, NVAR], FP, isOutput=False)
    Cy4J = dp("Cy4J", [4, NVAR], FP, isOutput=False)
    I11 = dp("I11", [NVAR, NVAR], FP, isOutput=False)
    T2 = dp("T2", [2, NUM], FP, isOutput=False)               # rows: ones, t
    K4 = dp("K4", [1, 4 * NVAR], FP, isOutput=False)          # beq rank-1 rows

    out_t = dp("out_t", [2 * NVAR, BPC], FP, isOutput=True)

    mm = nc.tensor.matmul

    lp = nc.allow_low_precision(reason="bf16 obstacle chain is within tolerance")
    lp.__enter__()
    with _TC(nc) as tc:
        cst = tc.alloc_tile_pool(name="cst", bufs=1)
        act = tc.alloc_tile_pool(name="act", bufs=2)
        lam = tc.alloc_tile_pool(name="lam", bufs=3)
        psp = tc.alloc_tile_pool(name="ps", bufs=3, space="PSUM")

        dmae = [nc.sync, nc.gpsimd, nc.scalar]

        def cload(ap, shape, dtype=FP, tag=None, q=0):
            t = cst.tile(shape, dtype, tag=tag or ap.tensor.name)
            dmae[q % len(dmae)].dma_start(out=t[:], in_=ap)
            return t

        # ---------- constants into SBUF ----------
        sb_PTp = cload(PTp[:], [NVAR, NUM])
        sb_PdT = cload(PdT[:], [NVAR, NUM])
        sb_PddT = cload(PddT[:], [NVAR, NUM])
        sb_Pdk = cload(Pdk[:], [NUM, NVAR], BF)
        sb_Pddk = cload(Pddk[:], [NUM, NVAR], BF)
        sb_Pres = cload(Pres[:], [NUM, NVAR], BF)
        sb_Plane = cload(Plane[:], [NUM, NVAR], BF)
        sb_J2x2 = cload(J2x2[:], [NVAR, NVAR])
        sb_J2xn = cload(J2xn[:], [NVAR, NVAR])
        sb_WJx = cload(WJx[:], [NVAR, NVAR])
        sb_J2y2 = cload(J2y2[:], [NVAR, NVAR])
        sb_J2yn = cload(J2yn[:], [NVAR, NVAR])
        sb_WJy = cload(WJy[:], [NVAR, NVAR])
        sb_Cx4 = cload(Cx4[:], [4, NVAR])
        sb_Cy4 = cload(Cy4[:], [4, NVAR])
        sb_Cx4J = cload(Cx4J[:], [4, NVAR])
        sb_Cy4J = cload(Cy4J[:], [4, NVAR])
        sb_I11 = cload(I11[:], [NVAR, NVAR])
        sb_T2 = cload(T2[:], [2, NUM])
        sb_K4 = cload(K4[:], [1, 4 * NVAR])
        sb_vxy0 = cload(vxy0[:], [1, 2 * BPC], q=1)
        sb_ylu = cload(ylu[:], [1, 2 * BPC], q=1)
        sb_obsx = cload(obsst[0:2, :], [2, 2 * NB2], tag="obsx", q=2)
        sb_obsy = cload(obsst[2:4, :], [2, 2 * NB2], tag="obsy", q=2)

        # ---------- MLP weights into SBUF (spread across DMA queues) ----------
        sb_w1a = cload(w1[0:128, :], [128, HID], BF, tag="w1a", q=1)
        sb_w1b = cload(w1[128:255, :], [127, HID], BF, tag="w1b", q=2)
        sb_w2 = cst.tile([128, 8 * HID], BF, tag="w2")
        for kc in range(8):
            dmae[kc % 3].dma_start(
                out=sb_w2[:, kc * HID : (kc + 1) * HID],
                in_=w2[kc * 128 : (kc + 1) * 128, :],
            )
        sb_wml = cst.tile([128, 8 * 4], BF, tag="wml")
        for kc in range(8):
            nc.sync.dma_start(
                out=sb_wml[:, kc * 4 : (kc + 1) * 4],
                in_=wml[kc * 128 : (kc + 1) * 128, :],
            )
        sb_dw1 = cload(dw1[:], [57, HID], BF, q=3)
        sb_dw2 = cst.tile([128, 8 * HID], BF, tag="dw2")
        for kc in range(8):
            dmae[kc % 3].dma_start(
                out=sb_dw2[:, kc * HID : (kc + 1) * HID],
                in_=dw2[kc * 128 : (kc + 1) * 128, :],
            )
        sb_dw3 = cst.tile([128, 8 * 8], BF, tag="dw3")
        for kc in range(8):
            nc.gpsimd.dma_start(
                out=sb_dw3[:, kc * 8 : (kc + 1) * 8],
                in_=dw3[kc * 128 : (kc + 1) * 128, :],
            )
        sb_b1 = cload(b1[:].rearrange("(c p) -> c p", p=128).rearrange("c p -> p c"), [128, 8], q=3)
        sb_b2 = cload(b2[:].rearrange("(c p) -> c p", p=128).rearrange("c p -> p c"), [128, 8], q=3)
        sb_db1 = cload(db1[:].rearrange("(c p) -> c p", p=128).rearrange("c p -> p c"), [128, 8], q=3)
        sb_db2 = cload(db2[:].rearrange("(c p) -> c p", p=128).rearrange("c p -> p c"), [128, 8], q=3)
        sb_bmu = cload(bmu[:, None], [2, 1], q=3)
        sb_blvh = cload(blvh[:, None], [2, 1], q=3)
        sb_db3v = cload(db3[0:4][:, None], [4, 1], tag="db3v", q=3)
        sb_db3y = cload(db3[4:8][:, None], [4, 1], tag="db3y", q=3)

        sb_x1a = cload(enc_in[0:128, :], [128, BPC], BF, tag="x1a", q=1)
        sb_x1b = cload(enc_in[128:255, :], [127, BPC], BF, tag="x1b", q=1)
        sb_eps = cload(eps_t[:], [2, BPC], q=1)

        # ---------- obstacle trajectories + lane bounds on device ----------
        # xo[t, (s,o,b)] = x + t*vx  via K=2 matmul with stationary [ones; t]
        sb_xyobs = cst.tile([NUM, 2 * NB], BF, tag="xyobs")
        H2 = NB2 // 2  # 320, fits the psum_a slot
        for s in (0, 1):
            for a in (0, 1):
                for h in (0, 1):
                    ps = psp.tile([NUM, H2], FP, tag="psum_a", bufs=4,
                                  name=f"obs{s}{a}{h}")
                    src = sb_obsx if a == 0 else sb_obsy
                    mov = src[:, s * NB2 + h * H2 : s * NB2 + (h + 1) * H2]
                    mm(ps[:], sb_T2[:], mov, start=True, stop=True)
                    dst = (2 * s + a) * NB2 + h * H2
                    nc.scalar.activation(
                        sb_xyobs[:, dst : dst + H2], ps[:], AF.Copy
                    )
        # lane bounds broadcast along partitions: ones[1,100]^T @ lb[1,B]
        lane_ps = psp.tile([NUM, 2 * BPC], FP, tag="psum_b", bufs=4, name="lane_ps")
        mm(lane_ps[:, :BPC], sb_T2[0:1, :], sb_ylu[0:1, :BPC], start=True, stop=True,
           skip_group_check=True)
        mm(lane_ps[:, BPC:], sb_T2[0:1, :], sb_ylu[0:1, BPC:], start=True, stop=True,
           skip_group_check=True)
        sb_ylbc = cst.tile([NUM, BPC], BF, tag="ylbc")
        nc.scalar.activation(sb_ylbc[:], lane_ps[:, :BPC], AF.Copy)
        sb_yubc = cst.tile([NUM, BPC], BF, tag="yubc")
        nc.scalar.activation(sb_yubc[:], lane_ps[:, BPC:], AF.Copy)

        # ---------- MLP (bf16 weights/acts, fp32 PSUM) ----------
        h1 = act.tile([128, HID], BF, tag="h1", bufs=1)
        for jc in range(8):
            ps = psp.tile([128, BPC], FP, tag="psum_a", bufs=4)
            js = slice(jc * 128, (jc + 1) * 128)
            mm(ps[:], sb_w1a[:, js], sb_x1a[:], start=True, stop=False)
            mm(ps[:], sb_w1b[:, js], sb_x1b[:], start=False, stop=True)
            nc.scalar.activation(h1[:, js], ps[:], AF.Relu, bias=sb_b1[:, jc : jc + 1])
        h2 = act.tile([128, HID], BF, tag="h2", bufs=1)
        for jc in range(8):
            ps = psp.tile([128, BPC], FP, tag="psum_a", bufs=4)
            js = slice(jc * 128, (jc + 1) * 128)
            for kc in range(8):
                mm(
                    ps[:],
                    sb_w2[:, kc * HID + jc * 128 : kc * HID + (jc + 1) * 128],
                    h1[:, kc * 128 : (kc + 1) * 128],
                    start=(kc == 0),
                    stop=(kc == 7),
                )
            nc.scalar.activation(h2[:, js], ps[:], AF.Relu, bias=sb_b2[:, jc : jc + 1])
        mu_ps = psp.tile([2, BPC], FP, tag="psum_b", bufs=4)
        lv_ps = psp.tile([2, BPC], FP, tag="psum_b", bufs=4)
        for kc in range(8):
            mm(
                mu_ps[:],
                sb_wml[:, kc * 4 : kc * 4 + 2],
                h2[:, kc * 128 : (kc + 1) * 128],
                start=(kc == 0),
                stop=(kc == 7),
            )
            mm(
                lv_ps[:],
                sb_wml[:, kc * 4 + 2 : kc * 4 + 4],
                h2[:, kc * 128 : (kc + 1) * 128],
                start=(kc == 0),
                stop=(kc == 7),
            )
        # z = mu + bmu + exp(0.5*lv + 0.5*blv)*eps  -> rows 0:2 of dec_in (bf16)
        dec_in = act.tile([57, BPC], BF, tag="dec_in", bufs=1)
        nc.sync.dma_start(out=dec_in[2:57, :], in_=sb_x1a[0:55, :])
        std = act.tile([2, BPC], FP, tag="std", bufs=1)
        nc.scalar.activation(std[:], lv_ps[:], AF.Exp, bias=sb_blvh[:], scale=0.5)
        zz = act.tile([2, BPC], FP, tag="zz", bufs=1)
        nc.vector.tensor_mul(out=zz[:], in0=std[:], in1=sb_eps[:])
        zmu = act.tile([2, BPC], FP, tag="zmu", bufs=1)
        nc.vector.tensor_add(out=zmu[:], in0=zz[:], in1=mu_ps[:])
        nc.vector.tensor_scalar(
            out=dec_in[0:2, :], in0=zmu[:], scalar1=sb_bmu[:], scalar2=None,
            op0=OP.add,
        )
        g1 = act.tile([128, HID], BF, tag="g1", bufs=1)
        for jc in range(8):
            ps = psp.tile([128, BPC], FP, tag="psum_a", bufs=4)
            js = slice(jc * 128, (jc + 1) * 128)
            mm(ps[:], sb_dw1[:, js], dec_in[:], start=True, stop=True)
            nc.scalar.activation(g1[:, js], ps[:], AF.Relu, bias=sb_db1[:, jc : jc + 1])
        g2 = act.tile([128, HID], BF, tag="g2", bufs=1)
        for jc in range(8):
            ps = psp.tile([128, BPC], FP, tag="psum_a", bufs=4)
            js = slice(jc * 128, (jc + 1) * 128)
            for kc in range(8):
                mm(
                    ps[:],
                    sb_dw2[:, kc * HID + jc * 128 : kc * HID + (jc + 1) * 128],
                    g1[:, kc * 128 : (kc + 1) * 128],
                    start=(kc == 0),
                    stop=(kc == 7),
                )
            nc.scalar.activation(g2[:, js], ps[:], AF.Relu, bias=sb_db2[:, jc : jc + 1])
        nnv_ps = psp.tile([4, BPC], FP, tag="psum_b", bufs=4)
        nny_ps = psp.tile([4, BPC], FP, tag="psum_b", bufs=4)
        for kc in range(8):
            mm(
                nnv_ps[:],
                sb_dw3[:, kc * 8 : kc * 8 + 4],
                g2[:, kc * 128 : (kc + 1) * 128],
                start=(kc == 0),
                stop=(kc == 7),
            )
            mm(
                nny_ps[:],
                sb_dw3[:, kc * 8 + 4 : kc * 8 + 8],
                g2[:, kc * 128 : (kc + 1) * 128],
                start=(kc == 0),
                stop=(kc == 7),
            )
        nn_v = act.tile([4, BPC], FP, tag="nn_v", bufs=1)
        nc.scalar.activation(nn_v[:], nnv_ps[:], AF.Identity, bias=sb_db3v[:])
        nn_y = act.tile([4, BPC], FP, tag="nn_y", bufs=1)
        nc.scalar.activation(nn_y[:], nny_ps[:], AF.Identity, bias=sb_db3y[:])

        # ---------- QP setup ----------
        # b_eq terms are rank-1 (only vx0/vy0 nonzero): K4 rows are the
        # corresponding KKT-inverse rows, applied as K=1 matmuls.
        st_ps = psp.tile([NVAR, 2 * BPC], FP, tag="psum_b", bufs=4)
        mm(st_ps[:, :BPC], sb_Cx4[:], nn_v[:], start=True, stop=False)
        mm(st_ps[:, :BPC], sb_K4[0:1, 0:NVAR], sb_vxy0[0:1, :BPC],
           start=False, stop=True)
        mm(st_ps[:, BPC:], sb_Cy4[:], nn_y[:], start=True, stop=False)
        mm(st_ps[:, BPC:], sb_K4[0:1, NVAR : 2 * NVAR], sb_vxy0[0:1, BPC:],
           start=False, stop=True)
        cxy = lam.tile([NVAR, 2 * BPC], FP, tag="cxy")
        nc.scalar.activation(cxy[:], st_ps[:], AF.Copy)

        sC_ps = psp.tile([NVAR, 2 * BPC], FP, tag="psum_b", bufs=4)
        mm(sC_ps[:, :BPC], sb_Cx4J[:], nn_v[:], start=True, stop=False)
        mm(sC_ps[:, :BPC], sb_K4[0:1, 2 * NVAR : 3 * NVAR], sb_vxy0[0:1, :BPC],
           start=False, stop=True)
        mm(sC_ps[:, BPC:], sb_Cy4J[:], nn_y[:], start=True, stop=False)
        mm(sC_ps[:, BPC:], sb_K4[0:1, 3 * NVAR : 4 * NVAR], sb_vxy0[0:1, BPC:],
           start=False, stop=True)
        CXC = cst.tile([NVAR, 2 * BPC], FP, tag="CXC")
        nc.scalar.activation(CXC[:], sC_ps[:], AF.Copy)

        sb_tiny = cst.tile([128, 1], FP, tag="tiny")
        nc.vector.memset(sb_tiny[:], 1e-30)
        sb_negone = cst.tile([128, 1], FP, tag="negone")
        nc.vector.memset(sb_negone[:], -1.0)

        # ---------- split into two independent 64-row streams ----------
        cxyS = []
        CXCS = []
        lxyS = []
        for s in (0, 1):
            cs = lam.tile([NVAR, 2 * B2], FP, tag=f"cxy{s}", name=f"cxyS{s}")
            nc.scalar.activation(
                cs[:].rearrange("p (a b) -> p a b", a=2),
                cxy[:].rearrange("p (a b) -> p a b", a=2)[:, :, s * B2 : (s + 1) * B2],
                AF.Copy,
            )
            cxyS.append(cs)
            xc = cst.tile([NVAR, 2 * B2], FP, tag=f"CXC{s}", name=f"CXCS{s}")
            nc.scalar.activation(
                xc[:].rearrange("p (a b) -> p a b", a=2),
                CXC[:].rearrange("p (a b) -> p a b", a=2)[:, :, s * B2 : (s + 1) * B2],
                AF.Copy,
            )
            CXCS.append(xc)
            lx = lam.tile([NVAR, 2 * B2], FP, tag=f"lxy{s}", name=f"lxyS{s}")
            nc.vector.memset(lx[:], 0.0)
            lxyS.append(lx)

        # ---------- QP iterations (two interleaved streams) ----------
        def qp_iter(s):
            cxy_s = cxyS[s]
            lxy_s = lxyS[s]
            obs = sb_xyobs[:, s * 2 * NB2 : (s + 1) * 2 * NB2]
            fwd = psp.tile([NUM, 6 * B2], FP, tag="psum_a", bufs=4, name=f"fwd{s}")
            xy = fwd[:, 0 : 2 * B2]
            xyd = fwd[:, 2 * B2 : 4 * B2]
            xydd = fwd[:, 4 * B2 : 6 * B2]
            mm(xy, sb_PTp[:], cxy_s[:], start=True, stop=True, skip_group_check=True)
            mm(xyd, sb_PdT[:], cxy_s[:], start=True, stop=True, skip_group_check=True)
            mm(xydd, sb_PddT[:], cxy_s[:], start=True, stop=True, skip_group_check=True)

            xys = act.tile([NUM, 2 * B2], BF, tag=f"xys{s}", name=f"xys_{s}")
            nc.scalar.activation(xys[:], xy, AF.Copy)
            w_s = act.tile([NUM, 2 * NB2], BF, tag=f"w_s{s}", name=f"w_s_{s}")
            nc.vector.tensor_tensor(
                out=w_s[:].rearrange("p (a o b) -> p a o b", a=2, o=NOBS),
                in0=xys[:]
                .rearrange("p (a b) -> p a b", a=2)[:, :, None, :]
                .to_broadcast((NUM, 2, NOBS, B2)),
                in1=obs.rearrange("p (a o b) -> p a o b", a=2, o=NOBS),
                op=OP.subtract,
            )

            sqw = act.tile([NUM, 2 * NB2], BF, tag=f"sqw{s}", name=f"sqw_{s}")
            nc.scalar.activation(sqw[:, :NB2], w_s[:, :NB2], AF.Square, scale=B_OBS)
            nc.scalar.activation(sqw[:, NB2:], w_s[:, NB2:], AF.Square, scale=A_OBS)
            sqd = act.tile([NUM, 2 * B2], BF, tag=f"sqd{s}", name=f"sqd_{s}")
            nc.scalar.activation(sqd[:], xyd, AF.Square)
            sqdd = act.tile([NUM, 2 * B2], BF, tag=f"sqdd{s}", name=f"sqdd_{s}")
            nc.scalar.activation(sqdd[:], xydd, AF.Square)

            r2all = act.tile([NUM, NB2 + 2 * B2], BF, tag=f"r2all{s}", name=f"r2all_{s}")
            nc.vector.tensor_add(out=r2all[:, :NB2], in0=sqw[:, :NB2], in1=sqw[:, NB2:])
            nc.vector.tensor_add(
                out=r2all[:, NB2 : NB2 + B2], in0=sqd[:, :B2], in1=sqd[:, B2:]
            )
            nc.vector.tensor_add(
                out=r2all[:, NB2 + B2 :], in0=sqdd[:, :B2], in1=sqdd[:, B2:]
            )
            r_all = act.tile([NUM, NB2 + 2 * B2], BF, tag=f"r_all{s}", name=f"r_all_{s}")
            nc.scalar.activation(r_all[:], r2all[:], AF.Sqrt, bias=sb_tiny[0:NUM, :])
            rinv = act.tile([NUM, NB2 + 2 * B2], BF, tag=f"rinv{s}", name=f"rinv_{s}")
            nc.vector.reciprocal(out=rinv[:], in_=r_all[:])

            qf = act.tile([NUM, NB2], BF, tag=f"qf{s}", name=f"qf_{s}")
            nc.scalar.activation(
                qf[:], rinv[:, :NB2], AF.Relu, bias=sb_negone[0:NUM, :], scale=AB
            )
            e_s = act.tile([NUM, 2 * NB2], BF, tag=f"e_s{s}", name=f"e_s_{s}")
            nc.vector.tensor_mul(out=e_s[:, :NB2], in0=qf[:], in1=w_s[:, :NB2])
            nc.vector.tensor_mul(out=e_s[:, NB2:], in0=qf[:], in1=w_s[:, NB2:])

            hva = act.tile([NUM, 2 * B2], FP, tag=f"hva{s}", name=f"hva_{s}")
            nc.vector.tensor_scalar(
                out=hva[:, :B2], in0=r_all[:, NB2 : NB2 + B2],
                scalar1=VMIN, scalar2=VMAX, op0=OP.max, op1=OP.min,
            )
            nc.vector.tensor_scalar(
                out=hva[:, B2:], in0=r_all[:, NB2 + B2 :],
                scalar1=AMAX, scalar2=None, op0=OP.min,
            )
            gva = act.tile([NUM, 2 * B2], FP, tag=f"gva{s}", name=f"gva_{s}")
            nc.vector.tensor_mul(out=gva[:], in0=hva[:], in1=rinv[:, NB2:])
            gvam = act.tile([NUM, 2 * B2], FP, tag=f"gvam{s}", name=f"gvam_{s}")
            nc.vector.tensor_scalar(
                out=gvam[:], in0=gva[:], scalar1=-RI, scalar2=RI,
                op0=OP.mult, op1=OP.add,
            )

            ys = xys[:, B2:]
            lt1 = act.tile([NUM, B2], BF, tag=f"lt1{s}", name=f"lt1_{s}")
            nc.vector.tensor_max(
                out=lt1[:], in0=ys, in1=sb_ylbc[:, s * B2 : (s + 1) * B2]
            )
            lt2 = act.tile([NUM, B2], BF, tag=f"lt2{s}", name=f"lt2_{s}")
            nc.vector.tensor_tensor(
                out=lt2[:], in0=lt1[:], in1=sb_yubc[:, s * B2 : (s + 1) * B2], op=OP.min
            )
            d12 = act.tile([NUM, B2], BF, tag=f"d12{s}", name=f"d12_{s}")
            nc.vector.tensor_sub(out=d12[:], in0=ys, in1=lt2[:])

            rvxy = act.tile([NUM, 2 * B2], BF, tag=f"rvxy{s}", name=f"rvxy_{s}")
            nc.vector.tensor_tensor(
                out=rvxy[:].rearrange("p (a b) -> p a b", a=2),
                in0=gvam[:, 0:B2][:, None, :].to_broadcast((NUM, 2, B2)),
                in1=xyd.rearrange("p (a b) -> p a b", a=2),
                op=OP.mult,
            )
            raxy = act.tile([NUM, 2 * B2], BF, tag=f"raxy{s}", name=f"raxy_{s}")
            nc.vector.tensor_tensor(
                out=raxy[:].rearrange("p (a b) -> p a b", a=2),
                in0=gvam[:, B2:][:, None, :].to_broadcast((NUM, 2, B2)),
                in1=xydd.rearrange("p (a b) -> p a b", a=2),
                op=OP.mult,
            )

            Dx = psp.tile([NVAR, B2], FP, tag="psum_b", bufs=4, name=f"Dx{s}")
            for o in range(NOBS):
                mm(
                    Dx[:],
                    sb_Pres[:],
                    e_s[:, o * B2 : (o + 1) * B2],
                    start=(o == 0),
                    stop=False,
                )
            mm(Dx[:], sb_Pddk[:], raxy[:, :B2], start=False, stop=False)
            mm(Dx[:], sb_Pdk[:], rvxy[:, :B2], start=False, stop=True)

            Dy = psp.tile([NVAR, B2], FP, tag="psum_b", bufs=4, name=f"Dy{s}")
            for o in range(NOBS):
                mm(
                    Dy[:],
                    sb_Pres[:],
                    e_s[:, NB2 + o * B2 : NB2 + (o + 1) * B2],
                    start=(o == 0),
                    stop=False,
                )
            mm(Dy[:], sb_Pddk[:], raxy[:, B2:], start=False, stop=False)
            mm(Dy[:], sb_Pdk[:], rvxy[:, B2:], start=False, stop=False)
            mm(Dy[:], sb_Plane[:], d12[:], start=False, stop=True)

            lxy_new = lam.tile([NVAR, 2 * B2], FP, tag=f"lxy{s}", name=f"lxyn_{s}")
            nc.vector.tensor_sub(out=lxy_new[:, :B2], in0=lxy_s[:, :B2], in1=Dx[:])
            nc.vector.tensor_sub(out=lxy_new[:, B2:], in0=lxy_s[:, B2:], in1=Dy[:])

            cn = psp.tile([NVAR, 2 * B2], FP, tag="psum_b", bufs=4, name=f"cn{s}")
            mm(cn[:, :B2], sb_J2x2[:], lxy_new[:, :B2], start=True, stop=False)
            mm(cn[:, :B2], sb_J2xn[:], lxy_s[:, :B2], start=False, stop=False)
            mm(cn[:, :B2], sb_WJx[:], cxy_s[:, :B2], start=False, stop=False)
            mm(cn[:, :B2], sb_I11[:], CXCS[s][:, :B2], start=False, stop=True)
            mm(cn[:, B2:], sb_J2y2[:], lxy_new[:, B2:], start=True, stop=False)
            mm(cn[:, B2:], sb_J2yn[:], lxy_s[:, B2:], start=False, stop=False)
            mm(cn[:, B2:], sb_WJy[:], cxy_s[:, B2:], start=False, stop=False)
            mm(cn[:, B2:], sb_I11[:], CXCS[s][:, B2:], start=False, stop=True)

            cxy_new = lam.tile([NVAR, 2 * B2], FP, tag=f"cxy{s}", name=f"cxyn_{s}")
            nc.scalar.activation(cxy_new[:], cn[:], AF.Copy)
            cxyS[s] = cxy_new
            lxyS[s] = lxy_new

        for it in range(maxiter):
            qp_iter(0)
            qp_iter(1)

        for s in (0, 1):
            nc.sync.dma_start(
                out=out_t[:].rearrange("(h k) b -> k h b", h=2)[
                    :, :, s * B2 : (s + 1) * B2
                ],
                in_=cxyS[s][:].rearrange("p (h b) -> p h b", h=2),
            )

        psp.release()
        lam.release()
        act.release()
        cst.release()

    lp.__exit__(None, None, None)
    if split_waits:
        _split_sync_waits(nc)
    return nc


# ---------------------------------------------------------------------------
# host preprocessing
# ---------------------------------------------------------------------------

# inputs whose values flow into the device-cached constant tensors
_CONST_SRC = (
    "P", "Pdot", "Pddot",
    "enc_w1", "enc_b1", "enc_w2", "enc_b2",
    "enc_wmu", "enc_bmu", "enc_wlv", "enc_blv",
    "dec_w1", "dec_b1", "dec_w2", "dec_b2", "dec_w3", "dec_b3",
)


def _kkt_inv(cost, A_eq):
    m = A_eq.shape[0]
    n = cost.shape[0]
    M = np.zeros((n + m, n + m), np.float64)
    M[:n, :n] = cost
    M[:n, n:] = A_eq.T
    M[n:, :n] = A_eq
    return np.linalg.inv(M).astype(f32)


def prep_consts(inputs):
    """Per-core (replicated) constant tensors, as numpy arrays."""
    P = np.asarray(inputs["P"], f32)
    Pd = np.asarray(inputs["Pdot"], f32)
    Pdd = np.asarray(inputs["Pddot"], f32)
    t = np.linspace(0.0, T_FIN, NUM).astype(f32)

    A_eq_x = np.stack([P[0], Pd[0], Pdd[0]])
    A_eq_y = np.stack([P[0], Pd[0], Pdd[0], Pd[-1]])
    K_D = f32(2.0 * np.sqrt(np.float32(20.0)))
    A_pd = Pdd - f32(20.0) * P - K_D * Pd
    A_vd = Pdd - f32(20.0) * Pd
    cs = Pdd.T @ Pdd
    inv1_x = _kkt_inv(cs + A_vd.T @ A_vd, A_eq_x)
    inv1_y = _kkt_inv(cs + A_pd.T @ A_pd, A_eq_y)
    PtP = P.T @ P
    PdTPd = Pd.T @ Pd
    PddTPdd = Pdd.T @ Pdd
    I = np.eye(NVAR, dtype=f32)
    cost2_x = RP * I + RO * 10.0 * PtP + RI * PddTPdd + RI * PdTPd
    cost2_y = cost2_x + RL * 2.0 * PtP
    inv2_x = _kkt_inv(cost2_x, A_eq_x)
    inv2_y = _kkt_inv(cost2_y, A_eq_y)

    J1x = inv1_x[:NVAR, :NVAR].T
    K1x = inv1_x[:NVAR, NVAR:].T
    J1y = inv1_y[:NVAR, :NVAR].T
    K1y = inv1_y[:NVAR, NVAR:].T
    J2x = inv2_x[:NVAR, :NVAR].T
    K2x = inv2_x[:NVAR, NVAR:].T
    J2y = inv2_y[:NVAR, :NVAR].T
    K2y = inv2_y[:NVAR, NVAR:].T

    Avd_bs = A_vd.reshape(4, 25, NVAR).sum(1)
    Apd_bs = A_pd.reshape(4, 25, NVAR).sum(1)
    Cx4 = (f32(-20.0) * (Avd_bs @ J1x)).astype(f32)
    Cy4 = (f32(-20.0) * (Apd_bs @ J1y)).astype(f32)
    Wx = (f32(10.0 * RO) * PtP + f32(RI) * (PddTPdd + PdTPd)).astype(f32)
    Wy = (Wx + f32(2.0 * RL) * PtP).astype(f32)

    # rank-1 b_eq rows: b_eq_x = [0, vx0, 0], b_eq_y = [0, vy0, 0, 0]
    K4 = np.concatenate(
        [
            K1x[1],
            K1y[1],
            (K1x @ J2x + K2x)[1],
            (K1y @ J2y + K2y)[1],
        ]
    ).astype(f32)[None, :]
    T2 = np.stack([np.ones(NUM, f32), t]).astype(f32)

    return {
        "PTp": np.ascontiguousarray(P.T.astype(f32)),
        "PdT": np.ascontiguousarray(Pd.T.astype(f32)),
        "PddT": np.ascontiguousarray(Pdd.T.astype(f32)),
        "Pdk": np.ascontiguousarray(Pd.astype(bf16)),
        "Pddk": np.ascontiguousarray(Pdd.astype(bf16)),
        "Pres": np.ascontiguousarray((-RO * P).astype(bf16)),
        "Plane": np.ascontiguousarray((RL * P).astype(bf16)),
        "J2x2": np.ascontiguousarray((2.0 * J2x).astype(f32)),
        "J2xn": np.ascontiguousarray((-J2x).astype(f32)),
        "WJx": np.ascontiguousarray((Wx @ J2x).astype(f32)),
        "J2y2": np.ascontiguousarray((2.0 * J2y).astype(f32)),
        "J2yn": np.ascontiguousarray((-J2y).astype(f32)),
        "WJy": np.ascontiguousarray((Wy @ J2y).astype(f32)),
        "Cx4": Cx4,
        "Cy4": Cy4,
        "Cx4J": np.ascontiguousarray((Cx4 @ J2x).astype(f32)),
        "Cy4J": np.ascontiguousarray((Cy4 @ J2y).astype(f32)),
        "I11": I,
        "T2": T2,
        "K4": K4,
        "w1": np.ascontiguousarray(np.asarray(inputs["enc_w1"], f32).astype(bf16)),
        "b1": np.ascontiguousarray(np.asarray(inputs["enc_b1"], f32)),
        "w2": np.ascontiguousarray(np.asarray(inputs["enc_w2"], f32).astype(bf16)),
        "b2": np.ascontiguousarray(np.asarray(inputs["enc_b2"], f32)),
        "wml": np.ascontiguousarray(
            np.concatenate(
                [np.asarray(inputs["enc_wmu"], f32), np.asarray(inputs["enc_wlv"], f32)],
                axis=1,
            ).astype(bf16)
        ),
        "bmu": np.ascontiguousarray(np.asarray(inputs["enc_bmu"], f32)),
        "blvh": np.ascontiguousarray((0.5 * np.asarray(inputs["enc_blv"], f32))),
        "dw1": np.ascontiguousarray(np.asarray(inputs["dec_w1"], f32).astype(bf16)),
        "db1": np.ascontiguousarray(np.asarray(inputs["dec_b1"], f32)),
        "dw2": np.ascontiguousarray(np.asarray(inputs["dec_w2"], f32).astype(bf16)),
        "db2": np.ascontiguousarray(np.asarray(inputs["dec_b2"], f32)),
        "dw3": np.ascontiguousarray(np.asarray(inputs["dec_w3"], f32).astype(bf16)),
        "db3": np.ascontiguousarray(np.asarray(inputs["dec_b3"], f32)),
    }


def prep_batch_global(inputs):
    """Batch-dependent inputs as concatenated (ncores*rows, cols) arrays."""
    inp = np.asarray(inputs["inp"], f32)
    B = inp.shape[0]

    inp_mean = np.asarray(inputs["inp_mean"], f32)
    inp_std = np.asarray(inputs["inp_std"], f32)
    inp_n = (inp - inp_mean) / inp_std
    enc_full = np.concatenate(
        [inp_n, np.asarray(inputs["traj_gt"], f32)], axis=1
    ).T.astype(bf16)                                         # (255, B)
    enc_in = np.ascontiguousarray(
        enc_full.reshape(255, NCORES, BPC).transpose(1, 0, 2).reshape(
            NCORES * 255, BPC
        )
    )

    def core_rows(a):
        # (r, B) -> (ncores*r, BPC) grouping batch columns per core
        r = a.shape[0]
        return np.ascontiguousarray(
            a.reshape(r, NCORES, BPC).transpose(1, 0, 2).reshape(NCORES * r, BPC)
        )

    eps_t = core_rows(np.asarray(inputs["eps"], f32).T)
    ise = np.asarray(inputs["initial_state_ego"], f32)
    # per-core single row: [vx0 (BPC) | vy0 (BPC)]
    vxy0 = np.ascontiguousarray(
        ise[:, 2:4].T.reshape(2, NCORES, BPC).transpose(1, 0, 2).reshape(
            NCORES, 2 * BPC
        )
    )
    ylu = np.ascontiguousarray(
        np.stack([np.asarray(inputs["y_lb"], f32), np.asarray(inputs["y_ub"], f32)])
        .reshape(2, NCORES, BPC).transpose(1, 0, 2).reshape(NCORES, 2 * BPC)
    )

    # obstacle states: rows (x, vx, y, vy), free = (stream, obstacle, b64)
    X4 = np.stack(
        [inp[:, 5::5], inp[:, 7::5], inp[:, 6::5], inp[:, 8::5]]
    )                                                        # (4, B, NOBS)
    obsst = np.ascontiguousarray(
        X4.reshape(4, NCORES, 2, B2, NOBS)
        .transpose(1, 0, 2, 4, 3)                            # (c, 4, s, o, b)
        .reshape(NCORES * 4, 2 * NB2)
    )
    return {
        "enc_in": enc_in,
        "eps_t": eps_t,
        "vxy0": vxy0,
        "ylu": ylu,
        "obsst": obsst,
    }


def host_prep(inputs):
    """Per-core input dicts (CoreSim / debugging path)."""
    consts = prep_consts(inputs)
    gb = prep_batch_global(inputs)
    in_maps = []
    for c in range(NCORES):
        m = dict(consts)
        for k, g in gb.items():
            r = g.shape[0] // NCORES
            m[k] = np.ascontiguousarray(g[c * r : (c + 1) * r])
        in_maps.append(m)
    return in_maps


# ---------------------------------------------------------------------------
# persistent runtime: jitted executable + device-resident constants
# ---------------------------------------------------------------------------

class _Runtime:
    def __init__(self):
        install_neuronx_cc_hook()
        nc = build_module()
        self.nc = nc
        assert nc.dbg_addr is None or not nc.dbg_callbacks

        partition_name = (
            nc.partition_id_tensor.name if nc.partition_id_tensor else None
        )
        in_names, out_names, out_avals, zero_shapes = [], [], [], []
        for alloc in nc.m.functions[0].allocations:
            if not isinstance(alloc, mybir.MemoryLocationSet):
                continue
            name = alloc.memorylocations[0].name
            if alloc.kind == "ExternalInput":
                if name != partition_name:
                    in_names.append(name)
            elif alloc.kind == "ExternalOutput":
                shape = tuple(alloc.tensor_shape)
                dtype = mybir.dt.np(alloc.dtype)
                out_avals.append(jax.core.ShapedArray(shape, dtype))
                zero_shapes.append((shape, dtype))
                out_names.append(name)
        self.in_names = in_names
        self.out_names = out_names
        self.zero_shapes = zero_shapes
        n_params = len(in_names)
        n_outs = len(out_names)
        in_names_full = in_names + out_names + (
            [partition_name] if partition_name else []
        )

        def _body(*args):
            operands = list(args)
            if partition_name is not None:
                operands.append(partition_id_tensor())
            outs = _bass_exec_p.bind(
                *operands,
                out_avals=tuple(out_avals),
                in_names=tuple(in_names_full),
                out_names=tuple(out_names),
                lowering_input_output_aliases=(),
                sim_require_finite=True,
                sim_require_nnan=True,
                nc=nc,
            )
            return tuple(outs)

        devices = jax.devices()[:NCORES]
        assert len(devices) == NCORES
        self.mesh = Mesh(np.asarray(devices), ("core",))
        self.sharding = NamedSharding(self.mesh, PartitionSpec("core"))
        donate = tuple(range(n_params, n_params + n_outs))
        self.fn = jax.jit(
            shard_map(
                _body,
                mesh=self.mesh,
                in_specs=(PartitionSpec("core"),) * (n_params + n_outs),
                out_specs=(PartitionSpec("core"),) * n_outs,
                check_rep=False,
            ),
            donate_argnums=donate,
            keep_unused=True,
        )

        self.const_ids = None
        self.const_crc = None
        self.dev_consts = None

    def _const_key_fast(self, inputs):
        return tuple(id(inputs[k]) for k in _CONST_SRC)

    def _const_crc(self, inputs):
        h = 0
        for k in _CONST_SRC:
            a = np.ascontiguousarray(np.asarray(inputs[k]))
            h = zlib.crc32(a, h)
        return h

    def get_consts(self, inputs):
        ids = self._const_key_fast(inputs)
        if self.dev_consts is not None and ids == self.const_ids:
            return self.dev_consts
        crc = self._const_crc(inputs)
        if self.dev_consts is not None and crc == self.const_crc:
            self.const_ids = ids
            return self.dev_consts
        consts = prep_consts(inputs)
        dev = {}
        for name, a in consts.items():
            g = np.ascontiguousarray(
                np.broadcast_to(a, (NCORES,) + a.shape).reshape(
                    NCORES * a.shape[0], *a.shape[1:]
                )
            )
            dev[name] = jax.device_put(g, self.sharding)
        for v in dev.values():
            v.block_until_ready()
        self.const_ids = ids
        self.const_crc = crc
        self.dev_consts = dev
        return dev


_RT = None


def _run_once(rt, inputs):
    dev_consts = rt.get_consts(inputs)
    batch = prep_batch_global(inputs)
    args = [
        dev_consts[n] if n in dev_consts else batch[n] for n in rt.in_names
    ]
    zeros = [np.zeros((NCORES * s[0], *s[1:]), d) for s, d in rt.zero_shapes]
    outs = rt.fn(*args, *zeros)
    out = outs[0]
    try:
        out.copy_to_host_async()
    except Exception:
        pass
    a = np.asarray(out)                                  # (8*22, BPC)
    # rows per core: [h=0 (x) vars 0..10, h=1 (y) vars 0..10]; cols = batch
    res = a.reshape(NCORES, 2 * NVAR, BPC).transpose(0, 2, 1).reshape(
        NCORES * BPC, 2 * NVAR
    )
    return np.ascontiguousarray(res)


def kernel(**inputs):
    global _RT
    first = _RT is None
    if first:
        _RT = _Runtime()
    if first:
        # warm the dispatch fastpath / remote buffer pools on the cold call
        _run_once(_RT, inputs)
    return _run_once(_RT, inputs)


# revision 23
# speedup vs baseline: 28.9702x; 1.0176x over previous
"""Trainium2 Bass kernel for nn_Beta_cVAE (cVAE encoder/decoder + 2-layer QP
trajectory projection, 20 ADMM iterations).

Strategy: pure data-parallel over 8 NeuronCores (128 batch rows each).
On-device layout is fully transposed: batch on the free dim, time/NVAR on
partitions, so every matmul uses weights/basis as the stationary operand and
no transposes appear anywhere.  All arctan2/cos/sin of the reference are
eliminated algebraically (cos(atan2(a,b)) = b/hypot(a,b)): the obstacle
ellipse projection becomes a residual scaling  resid = w*min(r-AB,0)/r  that
is *exactly zero* for inactive obstacles (no large-term cancellation, which
makes the bf16 obstacle pipeline safe), velocity/accel projections become
clip-scalings, and the lane term reduces to y - clip(y, lb, ub).  Obstacle
sums over the 10 obstacles fold into PE matmul accumulations with a shared
stationary operand.  MLP runs in bf16 (fp32 PSUM accumulation).

Host/runtime side: the expensive parts of a call (jit trace/lower, NEFF
load, 44 MB weight upload over the axon tunnel) are all cached across calls:
the jitted shard_map executable is built once, the replicated weights /
QP constants are device-resident keyed by a crc32 of the weight arrays, and
per-call traffic is only the ~0.8 MB of batch-dependent inputs (encoder
input, eps, initial velocities, lane bounds, obstacle states — obstacle
*trajectories* and boundary-condition expansions are computed on device).
The output is fetched with copy_to_host_async so the D2H ride shares the
dispatch round-trip.
"""
import sys
import os
import zlib

sys.path.insert(0, "/opt/trn_rl_repo")

import numpy as np
import ml_dtypes
import jax

import concourse.bass as bass
import concourse.mybir as mybir
from concourse.tile import TileContext, ScopedClock
from concourse.bass2jax import (
    _bass_exec_p,
    install_neuronx_cc_hook,
    partition_id_tensor,
)
from jax.experimental.shard_map import shard_map
from jax.sharding import Mesh, NamedSharding, PartitionSpec

f32 = np.float32
bf16 = ml_dtypes.bfloat16
FP = mybir.dt.float32
FR = mybir.dt.float32r
BF = mybir.dt.bfloat16
AF = mybir.ActivationFunctionType
OP = mybir.AluOpType

NCORES = 8
BPC = 128          # batch rows per core
NUM, NVAR, NOBS = 100, 11, 10
NB = NOBS * BPC    # 1280
B2 = BPC // 2      # 64 (per interleaved stream)
NB2 = NOBS * B2    # 640
A_OBS, B_OBS = 8.0, 4.2
AB = A_OBS * B_OBS
RO, RI, RL, RP = 100.0, 100.0, 100.0, 1.0
VMIN, VMAX, AMAX = 0.1, 30.0, 8.0
MAXITER = 20
HID = 1024
T_FIN = 15.0


class _TC(TileContext):
    """TileContext whose tail drain splits sem waits: this walrus build's
    TPB_CTRL codegen accepts only one sync wait per instruction."""

    def _drain_and_barrier(self, tick_clock, wait_clock):
        drain_inst = self.nc.sync.drain()
        wait_clock.add_sem_waits(
            drain_inst.ins, ScopedClock({None: tick_clock.global_clock})
        )
        si = drain_inst.ins.sync_info
        waits = list(si.on_wait) if si is not None else []
        if len(waits) > 1:
            drain_inst.ins.sync_info = mybir.SyncInfo(
                on_wait=waits[:1], on_update=list(si.on_update)
            )
            for w in waits[1:]:
                nop = self.nc.sync.nop(nofuse=True, hint="split_drain_wait")
                nop.ins.sync_info = mybir.SyncInfo(on_wait=[w], on_update=[])

        self.nc.all_engine_barrier()
        assert self.sems is not None
        popped = self.nc._tile_sem_poison_stack.pop()
        assert popped is self._sem_poison
        self.nc.clear_and_free_semaphores(list(self.sems.allocated().values()))
        self.nc.all_engine_barrier()


def _split_sync_waits(nc, max_waits=1):
    """Walrus's CoreV3 codegen accepts few sync waits per instruction; hoist
    surplus waits onto same-engine nops inserted just before the offender."""
    ctr = [0]
    for f in nc.m.functions:
        for bb in f.blocks:
            insts = list(bb.instructions)
            out = []
            for inst in insts:
                si = getattr(inst, "sync_info", None)
                if si is not None and len(si.on_wait) > max_waits:
                    waits = list(si.on_wait)
                    keep = waits[-max_waits:]
                    rest = waits[: len(waits) - max_waits]
                    for i in range(0, len(rest), max_waits):
                        ctr[0] += 1
                        nop = mybir.InstNoOp(
                            name=f"swsplit_{ctr[0]}",
                            engine=inst.engine,
                            bass_nofuse=True,
                            sync_info=mybir.SyncInfo(
                                on_wait=rest[i : i + max_waits], on_update=[]
                            ),
                        )
                        out.append(nop)
                    inst.sync_info = mybir.SyncInfo(
                        on_wait=keep, on_update=list(si.on_update)
                    )
                out.append(inst)
            bb.instructions = out


# ---------------------------------------------------------------------------
# module construction
# ---------------------------------------------------------------------------

def build_module(maxiter=MAXITER, split_waits=True):
    nc = bass.Bass()
    dp = nc.declare_dram_parameter

    # ---- per-core batch-dependent inputs (small; uploaded every call) ----
    enc_in = dp("enc_in", [255, BPC], BF, isOutput=False)
    eps_t = dp("eps_t", [2, BPC], FP, isOutput=False)
    vxy0 = dp("vxy0", [1, 2 * BPC], FP, isOutput=False)  # [vx0 | vy0]
    ylu = dp("ylu", [1, 2 * BPC], FP, isOutput=False)    # [y_lb | y_ub]
    obsst = dp("obsst", [4, 2 * NB2], FP, isOutput=False)  # rows x,vx,y,vy; free=(s,o,b)

    # ---- replicated weights (bf16; device-cached across calls) ----
    w1 = dp("w1", [255, HID], BF, isOutput=False)
    b1 = dp("b1", [HID], FP, isOutput=False)
    w2 = dp("w2", [HID, HID], BF, isOutput=False)
    b2 = dp("b2", [HID], FP, isOutput=False)
    wml = dp("wml", [HID, 4], BF, isOutput=False)
    bmu = dp("bmu", [2], FP, isOutput=False)
    blvh = dp("blvh", [2], FP, isOutput=False)
    dw1 = dp("dw1", [57, HID], BF, isOutput=False)
    db1 = dp("db1", [HID], FP, isOutput=False)
    dw2 = dp("dw2", [HID, HID], BF, isOutput=False)
    db2 = dp("db2", [HID], FP, isOutput=False)
    dw3 = dp("dw3", [HID, 8], BF, isOutput=False)
    db3 = dp("db3", [8], FP, isOutput=False)

    # ---- replicated QP constants ----
    PTp = dp("PTp", [NVAR, NUM], FP, isOutput=False)          # P.T
    PdT = dp("PdT", [NVAR, NUM], FP, isOutput=False)
    PddT = dp("PddT", [NVAR, NUM], FP, isOutput=False)
    Pdk = dp("Pdk", [NUM, NVAR], BF, isOutput=False)
    Pddk = dp("Pddk", [NUM, NVAR], BF, isOutput=False)
    Pres = dp("Pres", [NUM, NVAR], BF, isOutput=False)        # -RO*P
    Plane = dp("Plane", [NUM, NVAR], BF, isOutput=False)      # RL*P
    J2x2 = dp("J2x2", [NVAR, NVAR], FP, isOutput=False)
    J2xn = dp("J2xn", [NVAR, NVAR], FP, isOutput=False)
    WJx = dp("WJx", [NVAR, NVAR], FP, isOutput=False)
    J2y2 = dp("J2y2", [NVAR, NVAR], FP, isOutput=False)
    J2yn = dp("J2yn", [NVAR, NVAR], FP, isOutput=False)
    WJy = dp("WJy", [NVAR, NVAR], FP, isOutput=False)
    Cx4 = dp("Cx4", [4, NVAR], FP, isOutput=False)
    Cy4 = dp("Cy4", [4, NVAR], FP, isOutput=False)
    Cx4J = dp("Cx4J", [4, NVAR], FP, isOutput=False)
    Cy4J = dp("Cy4J", [4, NVAR], FP, isOutput=False)
    I11 = dp("I11", [NVAR, NVAR], FP, isOutput=False)
    T2 = dp("T2", [2, NUM], FP, isOutput=False)               # rows: ones, t
    K4 = dp("K4", [1, 4 * NVAR], FP, isOutput=False)          # beq rank-1 rows

    out_t = dp("out_t", [2 * NVAR, BPC], FP, isOutput=True)

    mm = nc.tensor.matmul

    lp = nc.allow_low_precision(reason="bf16 obstacle chain is within tolerance")
    lp.__enter__()
    with _TC(nc) as tc:
        cst = tc.alloc_tile_pool(name="cst", bufs=1)
        act = tc.alloc_tile_pool(name="act", bufs=2)
        lam = tc.alloc_tile_pool(name="lam", bufs=3)
        psp = tc.alloc_tile_pool(name="ps", bufs=3, space="PSUM")

        dmae = [nc.sync, nc.gpsimd, nc.scalar]

        def cload(ap, shape, dtype=FP, tag=None, q=0):
            t = cst.tile(shape, dtype, tag=tag or ap.tensor.name)
            dmae[q % len(dmae)].dma_start(out=t[:], in_=ap)
            return t

        # ---------- constants into SBUF ----------
        sb_PTp = cload(PTp[:], [NVAR, NUM])
        sb_PdT = cload(PdT[:], [NVAR, NUM])
        sb_PddT = cload(PddT[:], [NVAR, NUM])
        sb_Pdk = cload(Pdk[:], [NUM, NVAR], BF)
        sb_Pddk = cload(Pddk[:], [NUM, NVAR], BF)
        sb_Pres = cload(Pres[:], [NUM, NVAR], BF)
        sb_Plane = cload(Plane[:], [NUM, NVAR], BF)
        sb_J2x2 = cload(J2x2[:], [NVAR, NVAR])
        sb_J2xn = cload(J2xn[:], [NVAR, NVAR])
        sb_WJx = cload(WJx[:], [NVAR, NVAR])
        sb_J2y2 = cload(J2y2[:], [NVAR, NVAR])
        sb_J2yn = cload(J2yn[:], [NVAR, NVAR])
        sb_WJy = cload(WJy[:], [NVAR, NVAR])
        sb_Cx4 = cload(Cx4[:], [4, NVAR])
        sb_Cy4 = cload(Cy4[:], [4, NVAR])
        sb_Cx4J = cload(Cx4J[:], [4, NVAR])
        sb_Cy4J = cload(Cy4J[:], [4, NVAR])
        sb_I11 = cload(I11[:], [NVAR, NVAR])
        sb_T2 = cload(T2[:], [2, NUM])
        sb_K4 = cload(K4[:], [1, 4 * NVAR])
        sb_vxy0 = cload(vxy0[:], [1, 2 * BPC], q=1)
        sb_ylu = cload(ylu[:], [1, 2 * BPC], q=1)
        sb_obsx = cload(obsst[0:2, :], [2, 2 * NB2], tag="obsx", q=2)
        sb_obsy = cload(obsst[2:4, :], [2, 2 * NB2], tag="obsy", q=2)

        # ---------- MLP weights into SBUF (spread across DMA queues) ----------
        sb_w1a = cload(w1[0:128, :], [128, HID], BF, tag="w1a", q=1)
        sb_w1b = cload(w1[128:255, :], [127, HID], BF, tag="w1b", q=2)
        sb_w2 = cst.tile([128, 8 * HID], BF, tag="w2")
        for kc in range(8):
            dmae[kc % 3].dma_start(
                out=sb_w2[:, kc * HID : (kc + 1) * HID],
                in_=w2[kc * 128 : (kc + 1) * 128, :],
            )
        sb_wml = cst.tile([128, 8 * 4], BF, tag="wml")
        for kc in range(8):
            nc.sync.dma_start(
                out=sb_wml[:, kc * 4 : (kc + 1) * 4],
                in_=wml[kc * 128 : (kc + 1) * 128, :],
            )
        sb_dw1 = cload(dw1[:], [57, HID], BF, q=3)
        sb_dw2 = cst.tile([128, 8 * HID], BF, tag="dw2")
        for kc in range(8):
            dmae[kc % 3].dma_start(
                out=sb_dw2[:, kc * HID : (kc + 1) * HID],
                in_=dw2[kc * 128 : (kc + 1) * 128, :],
            )
        sb_dw3 = cst.tile([128, 8 * 8], BF, tag="dw3")
        for kc in range(8):
            nc.gpsimd.dma_start(
                out=sb_dw3[:, kc * 8 : (kc + 1) * 8],
                in_=dw3[kc * 128 : (kc + 1) * 128, :],
            )
        sb_b1 = cload(b1[:].rearrange("(c p) -> c p", p=128).rearrange("c p -> p c"), [128, 8], q=3)
        sb_b2 = cload(b2[:].rearrange("(c p) -> c p", p=128).rearrange("c p -> p c"), [128, 8], q=3)
        sb_db1 = cload(db1[:].rearrange("(c p) -> c p", p=128).rearrange("c p -> p c"), [128, 8], q=3)
        sb_db2 = cload(db2[:].rearrange("(c p) -> c p", p=128).rearrange("c p -> p c"), [128, 8], q=3)
        sb_bmu = cload(bmu[:, None], [2, 1], q=3)
        sb_blvh = cload(blvh[:, None], [2, 1], q=3)
        sb_db3v = cload(db3[0:4][:, None], [4, 1], tag="db3v", q=3)
        sb_db3y = cload(db3[4:8][:, None], [4, 1], tag="db3y", q=3)

        sb_x1a = cload(enc_in[0:128, :], [128, BPC], BF, tag="x1a", q=1)
        sb_x1b = cload(enc_in[128:255, :], [127, BPC], BF, tag="x1b", q=1)
        sb_eps = cload(eps_t[:], [2, BPC], q=1)

        # ---------- obstacle trajectories + lane bounds on device ----------
        # xo[t, (s,o,b)] = x + t*vx  via K=2 matmul with stationary [ones; t]
        sb_xyobs = cst.tile([NUM, 2 * NB], BF, tag="xyobs")
        H2 = NB2 // 2  # 320, fits the psum_a slot
        for s in (0, 1):
            for a in (0, 1):
                for h in (0, 1):
                    ps = psp.tile([NUM, H2], FP, tag="psum_a", bufs=4,
                                  name=f"obs{s}{a}{h}")
                    src = sb_obsx if a == 0 else sb_obsy
                    mov = src[:, s * NB2 + h * H2 : s * NB2 + (h + 1) * H2]
                    mm(ps[:], sb_T2[:], mov, start=True, stop=True)
                    dst = (2 * s + a) * NB2 + h * H2
                    nc.scalar.activation(
                        sb_xyobs[:, dst : dst + H2], ps[:], AF.Copy
                    )
        # lane bounds broadcast along partitions: ones[1,100]^T @ lb[1,B]
        lane_ps = psp.tile([NUM, 2 * BPC], FP, tag="psum_b", bufs=4, name="lane_ps")
        mm(lane_ps[:, :BPC], sb_T2[0:1, :], sb_ylu[0:1, :BPC], start=True, stop=True,
           skip_group_check=True)
        mm(lane_ps[:, BPC:], sb_T2[0:1, :], sb_ylu[0:1, BPC:], start=True, stop=True,
           skip_group_check=True)
        sb_ylbc = cst.tile([NUM, BPC], BF, tag="ylbc")
        nc.scalar.activation(sb_ylbc[:], lane_ps[:, :BPC], AF.Copy)
        sb_yubc = cst.tile([NUM, BPC], BF, tag="yubc")
        nc.scalar.activation(sb_yubc[:], lane_ps[:, BPC:], AF.Copy)

        # ---------- MLP (bf16 weights/acts, fp32 PSUM) ----------
        h1 = act.tile([128, HID], BF, tag="h1", bufs=1)
        for jc in range(8):
            ps = psp.tile([128, BPC], FP, tag="psum_a", bufs=4)
            js = slice(jc * 128, (jc + 1) * 128)
            mm(ps[:], sb_w1a[:, js], sb_x1a[:], start=True, stop=False)
            mm(ps[:], sb_w1b[:, js], sb_x1b[:], start=False, stop=True)
            nc.scalar.activation(h1[:, js], ps[:], AF.Relu, bias=sb_b1[:, jc : jc + 1])
        h2 = act.tile([128, HID], BF, tag="h2", bufs=1)
        for jc in range(8):
            ps = psp.tile([128, BPC], FP, tag="psum_a", bufs=4)
            js = slice(jc * 128, (jc + 1) * 128)
            for kc in range(8):
                mm(
                    ps[:],
                    sb_w2[:, kc * HID + jc * 128 : kc * HID + (jc + 1) * 128],
                    h1[:, kc * 128 : (kc + 1) * 128],
                    start=(kc == 0),
                    stop=(kc == 7),
                )
            nc.scalar.activation(h2[:, js], ps[:], AF.Relu, bias=sb_b2[:, jc : jc + 1])
        mu_ps = psp.tile([2, BPC], FP, tag="psum_b", bufs=4)
        lv_ps = psp.tile([2, BPC], FP, tag="psum_b", bufs=4)
        for kc in range(8):
            mm(
                mu_ps[:],
                sb_wml[:, kc * 4 : kc * 4 + 2],
                h2[:, kc * 128 : (kc + 1) * 128],
                start=(kc == 0),
                stop=(kc == 7),
            )
            mm(
                lv_ps[:],
                sb_wml[:, kc * 4 + 2 : kc * 4 + 4],
                h2[:, kc * 128 : (kc + 1) * 128],
                start=(kc == 0),
                stop=(kc == 7),
            )
        # z = mu + bmu + exp(0.5*lv + 0.5*blv)*eps  -> rows 0:2 of dec_in (bf16)
        dec_in = act.tile([57, BPC], BF, tag="dec_in", bufs=1)
        nc.sync.dma_start(out=dec_in[2:57, :], in_=sb_x1a[0:55, :])
        std = act.tile([2, BPC], FP, tag="std", bufs=1)
        nc.scalar.activation(std[:], lv_ps[:], AF.Exp, bias=sb_blvh[:], scale=0.5)
        zz = act.tile([2, BPC], FP, tag="zz", bufs=1)
        nc.vector.tensor_mul(out=zz[:], in0=std[:], in1=sb_eps[:])
        zmu = act.tile([2, BPC], FP, tag="zmu", bufs=1)
        nc.vector.tensor_add(out=zmu[:], in0=zz[:], in1=mu_ps[:])
        nc.vector.tensor_scalar(
            out=dec_in[0:2, :], in0=zmu[:], scalar1=sb_bmu[:], scalar2=None,
            op0=OP.add,
        )
        g1 = act.tile([128, HID], BF, tag="g1", bufs=1)
        for jc in range(8):
            ps = psp.tile([128, BPC], FP, tag="psum_a", bufs=4)
            js = slice(jc * 128, (jc + 1) * 128)
            mm(ps[:], sb_dw1[:, js], dec_in[:], start=True, stop=True)
            nc.scalar.activation(g1[:, js], ps[:], AF.Relu, bias=sb_db1[:, jc : jc + 1])
        g2 = act.tile([128, HID], BF, tag="g2", bufs=1)
        for jc in range(8):
            ps = psp.tile([128, BPC], FP, tag="psum_a", bufs=4)
            js = slice(jc * 128, (jc + 1) * 128)
            for kc in range(8):
                mm(
                    ps[:],
                    sb_dw2[:, kc * HID + jc * 128 : kc * HID + (jc + 1) * 128],
                    g1[:, kc * 128 : (kc + 1) * 128],
                    start=(kc == 0),
                    stop=(kc == 7),
                )
            nc.scalar.activation(g2[:, js], ps[:], AF.Relu, bias=sb_db2[:, jc : jc + 1])
        nnv_ps = psp.tile([4, BPC], FP, tag="psum_b", bufs=4)
        nny_ps = psp.tile([4, BPC], FP, tag="psum_b", bufs=4)
        for kc in range(8):
            mm(
                nnv_ps[:],
                sb_dw3[:, kc * 8 : kc * 8 + 4],
                g2[:, kc * 128 : (kc + 1) * 128],
                start=(kc == 0),
                stop=(kc == 7),
            )
            mm(
                nny_ps[:],
                sb_dw3[:, kc * 8 + 4 : kc * 8 + 8],
                g2[:, kc * 128 : (kc + 1) * 128],
                start=(kc == 0),
                stop=(kc == 7),
            )
        nn_v = act.tile([4, BPC], FP, tag="nn_v", bufs=1)
        nc.scalar.activation(nn_v[:], nnv_ps[:], AF.Identity, bias=sb_db3v[:])
        nn_y = act.tile([4, BPC], FP, tag="nn_y", bufs=1)
        nc.scalar.activation(nn_y[:], nny_ps[:], AF.Identity, bias=sb_db3y[:])

        # ---------- QP setup ----------
        # b_eq terms are rank-1 (only vx0/vy0 nonzero): K4 rows are the
        # corresponding KKT-inverse rows, applied as K=1 matmuls.
        st_ps = psp.tile([NVAR, 2 * BPC], FP, tag="psum_b", bufs=4)
        mm(st_ps[:, :BPC], sb_Cx4[:], nn_v[:], start=True, stop=False)
        mm(st_ps[:, :BPC], sb_K4[0:1, 0:NVAR], sb_vxy0[0:1, :BPC],
           start=False, stop=True)
        mm(st_ps[:, BPC:], sb_Cy4[:], nn_y[:], start=True, stop=False)
        mm(st_ps[:, BPC:], sb_K4[0:1, NVAR : 2 * NVAR], sb_vxy0[0:1, BPC:],
           start=False, stop=True)
        cxy = lam.tile([NVAR, 2 * BPC], FP, tag="cxy")
        nc.scalar.activation(cxy[:], st_ps[:], AF.Copy)

        sC_ps = psp.tile([NVAR, 2 * BPC], FP, tag="psum_b", bufs=4)
        mm(sC_ps[:, :BPC], sb_Cx4J[:], nn_v[:], start=True, stop=False)
        mm(sC_ps[:, :BPC], sb_K4[0:1, 2 * NVAR : 3 * NVAR], sb_vxy0[0:1, :BPC],
           start=False, stop=True)
        mm(sC_ps[:, BPC:], sb_Cy4J[:], nn_y[:], start=True, stop=False)
        mm(sC_ps[:, BPC:], sb_K4[0:1, 3 * NVAR : 4 * NVAR], sb_vxy0[0:1, BPC:],
           start=False, stop=True)
        CXC = cst.tile([NVAR, 2 * BPC], FP, tag="CXC")
        nc.scalar.activation(CXC[:], sC_ps[:], AF.Copy)

        sb_tiny = cst.tile([128, 1], FP, tag="tiny")
        nc.vector.memset(sb_tiny[:], 1e-30)
        sb_negone = cst.tile([128, 1], FP, tag="negone")
        nc.vector.memset(sb_negone[:], -1.0)

        # ---------- split into two independent 64-row streams ----------
        cxyS = []
        CXCS = []
        lxyS = []
        for s in (0, 1):
            cs = lam.tile([NVAR, 2 * B2], FP, tag=f"cxy{s}", name=f"cxyS{s}")
            nc.scalar.activation(
                cs[:].rearrange("p (a b) -> p a b", a=2),
                cxy[:].rearrange("p (a b) -> p a b", a=2)[:, :, s * B2 : (s + 1) * B2],
                AF.Copy,
            )
            cxyS.append(cs)
            xc = cst.tile([NVAR, 2 * B2], FP, tag=f"CXC{s}", name=f"CXCS{s}")
            nc.scalar.activation(
                xc[:].rearrange("p (a b) -> p a b", a=2),
                CXC[:].rearrange("p (a b) -> p a b", a=2)[:, :, s * B2 : (s + 1) * B2],
                AF.Copy,
            )
            CXCS.append(xc)
            lx = lam.tile([NVAR, 2 * B2], FP, tag=f"lxy{s}", name=f"lxyS{s}")
            nc.vector.memset(lx[:], 0.0)
            lxyS.append(lx)

        # ---------- QP iterations (two interleaved streams) ----------
        def qp_iter(s):
            cxy_s = cxyS[s]
            lxy_s = lxyS[s]
            obs = sb_xyobs[:, s * 2 * NB2 : (s + 1) * 2 * NB2]
            fwd = psp.tile([NUM, 6 * B2], FP, tag="psum_a", bufs=4, name=f"fwd{s}")
            xy = fwd[:, 0 : 2 * B2]
            xyd = fwd[:, 2 * B2 : 4 * B2]
            xydd = fwd[:, 4 * B2 : 6 * B2]
            mm(xy, sb_PTp[:], cxy_s[:], start=True, stop=True, skip_group_check=True)
            mm(xyd, sb_PdT[:], cxy_s[:], start=True, stop=True, skip_group_check=True)
            mm(xydd, sb_PddT[:], cxy_s[:], start=True, stop=True, skip_group_check=True)

            xys = act.tile([NUM, 2 * B2], BF, tag=f"xys{s}", name=f"xys_{s}")
            nc.scalar.activation(xys[:], xy, AF.Copy)
            w_s = act.tile([NUM, 2 * NB2], BF, tag=f"w_s{s}", name=f"w_s_{s}")
            nc.vector.tensor_tensor(
                out=w_s[:].rearrange("p (a o b) -> p a o b", a=2, o=NOBS),
                in0=xys[:]
                .rearrange("p (a b) -> p a b", a=2)[:, :, None, :]
                .to_broadcast((NUM, 2, NOBS, B2)),
                in1=obs.rearrange("p (a o b) -> p a o b", a=2, o=NOBS),
                op=OP.subtract,
            )

            sqw = act.tile([NUM, 2 * NB2], BF, tag=f"sqw{s}", name=f"sqw_{s}")
            nc.scalar.activation(sqw[:, :NB2], w_s[:, :NB2], AF.Square, scale=B_OBS)
            nc.scalar.activation(sqw[:, NB2:], w_s[:, NB2:], AF.Square, scale=A_OBS)
            sqd = act.tile([NUM, 2 * B2], BF, tag=f"sqd{s}", name=f"sqd_{s}")
            nc.scalar.activation(sqd[:], xyd, AF.Square)
            sqdd = act.tile([NUM, 2 * B2], BF, tag=f"sqdd{s}", name=f"sqdd_{s}")
            nc.scalar.activation(sqdd[:], xydd, AF.Square)

            r2all = act.tile([NUM, NB2 + 2 * B2], BF, tag=f"r2all{s}", name=f"r2all_{s}")
            nc.vector.tensor_add(out=r2all[:, :NB2], in0=sqw[:, :NB2], in1=sqw[:, NB2:])
            nc.vector.tensor_add(
                out=r2all[:, NB2 : NB2 + B2], in0=sqd[:, :B2], in1=sqd[:, B2:]
            )
            nc.vector.tensor_add(
                out=r2all[:, NB2 + B2 :], in0=sqdd[:, :B2], in1=sqdd[:, B2:]
            )
            r_all = act.tile([NUM, NB2 + 2 * B2], BF, tag=f"r_all{s}", name=f"r_all_{s}")
            nc.scalar.activation(r_all[:], r2all[:], AF.Sqrt, bias=sb_tiny[0:NUM, :])
            rinv = act.tile([NUM, NB2 + 2 * B2], BF, tag=f"rinv{s}", name=f"rinv_{s}")
            nc.vector.reciprocal(out=rinv[:], in_=r_all[:])

            qf = act.tile([NUM, NB2], BF, tag=f"qf{s}", name=f"qf_{s}")
            nc.scalar.activation(
                qf[:], rinv[:, :NB2], AF.Relu, bias=sb_negone[0:NUM, :], scale=AB
            )
            e_s = act.tile([NUM, 2 * NB2], BF, tag=f"e_s{s}", name=f"e_s_{s}")
            nc.vector.tensor_mul(out=e_s[:, :NB2], in0=qf[:], in1=w_s[:, :NB2])
            nc.vector.tensor_mul(out=e_s[:, NB2:], in0=qf[:], in1=w_s[:, NB2:])

            hva = act.tile([NUM, 2 * B2], FP, tag=f"hva{s}", name=f"hva_{s}")
            nc.vector.tensor_scalar(
                out=hva[:, :B2], in0=r_all[:, NB2 : NB2 + B2],
                scalar1=VMIN, scalar2=VMAX, op0=OP.max, op1=OP.min,
            )
            nc.vector.tensor_scalar(
                out=hva[:, B2:], in0=r_all[:, NB2 + B2 :],
                scalar1=AMAX, scalar2=None, op0=OP.min,
            )
            gva = act.tile([NUM, 2 * B2], FP, tag=f"gva{s}", name=f"gva_{s}")
            nc.vector.tensor_mul(out=gva[:], in0=hva[:], in1=rinv[:, NB2:])
            gvam = act.tile([NUM, 2 * B2], FP, tag=f"gvam{s}", name=f"gvam_{s}")
            nc.vector.tensor_scalar(
                out=gvam[:], in0=gva[:], scalar1=-RI, scalar2=RI,
                op0=OP.mult, op1=OP.add,
            )

            ys = xys[:, B2:]
            lt1 = act.tile([NUM, B2], BF, tag=f"lt1{s}", name=f"lt1_{s}")
            nc.vector.tensor_max(
                out=lt1[:], in0=ys, in1=sb_ylbc[:, s * B2 : (s + 1) * B2]
            )
            lt2 = act.tile([NUM, B2], BF, tag=f"lt2{s}", name=f"lt2_{s}")
            nc.vector.tensor_tensor(
                out=lt2[:], in0=lt1[:], in1=sb_yubc[:, s * B2 : (s + 1) * B2], op=OP.min
            )
            d12 = act.tile([NUM, B2], BF, tag=f"d12{s}", name=f"d12_{s}")
            nc.vector.tensor_sub(out=d12[:], in0=ys, in1=lt2[:])

            rvxy = act.tile([NUM, 2 * B2], BF, tag=f"rvxy{s}", name=f"rvxy_{s}")
            nc.vector.tensor_tensor(
                out=rvxy[:].rearrange("p (a b) -> p a b", a=2),
                in0=gvam[:, 0:B2][:, None, :].to_broadcast((NUM, 2, B2)),
                in1=xyd.rearrange("p (a b) -> p a b", a=2),
                op=OP.mult,
            )
            raxy = act.tile([NUM, 2 * B2], BF, tag=f"raxy{s}", name=f"raxy_{s}")
            nc.vector.tensor_tensor(
                out=raxy[:].rearrange("p (a b) -> p a b", a=2),
                in0=gvam[:, B2:][:, None, :].to_broadcast((NUM, 2, B2)),
                in1=xydd.rearrange("p (a b) -> p a b", a=2),
                op=OP.mult,
            )

            Dx = psp.tile([NVAR, B2], FP, tag="psum_b", bufs=4, name=f"Dx{s}")
            for o in range(NOBS):
                mm(
                    Dx[:],
                    sb_Pres[:],
                    e_s[:, o * B2 : (o + 1) * B2],
                    start=(o == 0),
                    stop=False,
                )
            mm(Dx[:], sb_Pddk[:], raxy[:, :B2], start=False, stop=False)
            mm(Dx[:], sb_Pdk[:], rvxy[:, :B2], start=False, stop=True)

            Dy = psp.tile([NVAR, B2], FP, tag="psum_b", bufs=4, name=f"Dy{s}")
            for o in range(NOBS):
                mm(
                    Dy[:],
                    sb_Pres[:],
                    e_s[:, NB2 + o * B2 : NB2 + (o + 1) * B2],
                    start=(o == 0),
                    stop=False,
                )
            mm(Dy[:], sb_Pddk[:], raxy[:, B2:], start=False, stop=False)
            mm(Dy[:], sb_Pdk[:], rvxy[:, B2:], start=False, stop=False)
            mm(Dy[:], sb_Plane[:], d12[:], start=False, stop=True)

            lxy_new = lam.tile([NVAR, 2 * B2], FP, tag=f"lxy{s}", name=f"lxyn_{s}")
            nc.vector.tensor_sub(out=lxy_new[:, :B2], in0=lxy_s[:, :B2], in1=Dx[:])
            nc.vector.tensor_sub(out=lxy_new[:, B2:], in0=lxy_s[:, B2:], in1=Dy[:])

            cn = psp.tile([NVAR, 2 * B2], FP, tag="psum_b", bufs=4, name=f"cn{s}")
            mm(cn[:, :B2], sb_J2x2[:], lxy_new[:, :B2], start=True, stop=False)
            mm(cn[:, :B2], sb_J2xn[:], lxy_s[:, :B2], start=False, stop=False)
            mm(cn[:, :B2], sb_WJx[:], cxy_s[:, :B2], start=False, stop=False)
            mm(cn[:, :B2], sb_I11[:], CXCS[s][:, :B2], start=False, stop=True)
            mm(cn[:, B2:], sb_J2y2[:], lxy_new[:, B2:], start=True, stop=False)
            mm(cn[:, B2:], sb_J2yn[:], lxy_s[:, B2:], start=False, stop=False)
            mm(cn[:, B2:], sb_WJy[:], cxy_s[:, B2:], start=False, stop=False)
            mm(cn[:, B2:], sb_I11[:], CXCS[s][:, B2:], start=False, stop=True)

            cxy_new = lam.tile([NVAR, 2 * B2], FP, tag=f"cxy{s}", name=f"cxyn_{s}")
            nc.scalar.activation(cxy_new[:], cn[:], AF.Copy)
            cxyS[s] = cxy_new
            lxyS[s] = lxy_new

        for it in range(maxiter):
            qp_iter(0)
            qp_iter(1)

        for s in (0, 1):
            nc.sync.dma_start(
                out=out_t[:].rearrange("(h k) b -> k h b", h=2)[
                    :, :, s * B2 : (s + 1) * B2
                ],
                in_=cxyS[s][:].rearrange("p (h b) -> p h b", h=2),
            )

        psp.release()
        lam.release()
        act.release()
        cst.release()

    lp.__exit__(None, None, None)
    if split_waits:
        _split_sync_waits(nc)
    return nc


# ---------------------------------------------------------------------------
# host preprocessing
# ---------------------------------------------------------------------------

# inputs whose values flow into the device-cached constant tensors
_CONST_SRC = (
    "P", "Pdot", "Pddot",
    "enc_w1", "enc_b1", "enc_w2", "enc_b2",
    "enc_wmu", "enc_bmu", "enc_wlv", "enc_blv",
    "dec_w1", "dec_b1", "dec_w2", "dec_b2", "dec_w3", "dec_b3",
)


def _kkt_inv(cost, A_eq):
    m = A_eq.shape[0]
    n = cost.shape[0]
    M = np.zeros((n + m, n + m), np.float64)
    M[:n, :n] = cost
    M[:n, n:] = A_eq.T
    M[n:, :n] = A_eq
    return np.linalg.inv(M).astype(f32)


def prep_consts(inputs):
    """Per-core (replicated) constant tensors, as numpy arrays."""
    P = np.asarray(inputs["P"], f32)
    Pd = np.asarray(inputs["Pdot"], f32)
    Pdd = np.asarray(inputs["Pddot"], f32)
    t = np.linspace(0.0, T_FIN, NUM).astype(f32)

    A_eq_x = np.stack([P[0], Pd[0], Pdd[0]])
    A_eq_y = np.stack([P[0], Pd[0], Pdd[0], Pd[-1]])
    K_D = f32(2.0 * np.sqrt(np.float32(20.0)))
    A_pd = Pdd - f32(20.0) * P - K_D * Pd
    A_vd = Pdd - f32(20.0) * Pd
    cs = Pdd.T @ Pdd
    inv1_x = _kkt_inv(cs + A_vd.T @ A_vd, A_eq_x)
    inv1_y = _kkt_inv(cs + A_pd.T @ A_pd, A_eq_y)
    PtP = P.T @ P
    PdTPd = Pd.T @ Pd
    PddTPdd = Pdd.T @ Pdd
    I = np.eye(NVAR, dtype=f32)
    cost2_x = RP * I + RO * 10.0 * PtP + RI * PddTPdd + RI * PdTPd
    cost2_y = cost2_x + RL * 2.0 * PtP
    inv2_x = _kkt_inv(cost2_x, A_eq_x)
    inv2_y = _kkt_inv(cost2_y, A_eq_y)

    J1x = inv1_x[:NVAR, :NVAR].T
    K1x = inv1_x[:NVAR, NVAR:].T
    J1y = inv1_y[:NVAR, :NVAR].T
    K1y = inv1_y[:NVAR, NVAR:].T
    J2x = inv2_x[:NVAR, :NVAR].T
    K2x = inv2_x[:NVAR, NVAR:].T
    J2y = inv2_y[:NVAR, :NVAR].T
    K2y = inv2_y[:NVAR, NVAR:].T

    Avd_bs = A_vd.reshape(4, 25, NVAR).sum(1)
    Apd_bs = A_pd.reshape(4, 25, NVAR).sum(1)
    Cx4 = (f32(-20.0) * (Avd_bs @ J1x)).astype(f32)
    Cy4 = (f32(-20.0) * (Apd_bs @ J1y)).astype(f32)
    Wx = (f32(10.0 * RO) * PtP + f32(RI) * (PddTPdd + PdTPd)).astype(f32)
    Wy = (Wx + f32(2.0 * RL) * PtP).astype(f32)

    # rank-1 b_eq rows: b_eq_x = [0, vx0, 0], b_eq_y = [0, vy0, 0, 0]
    K4 = np.concatenate(
        [
            K1x[1],
            K1y[1],
            (K1x @ J2x + K2x)[1],
            (K1y @ J2y + K2y)[1],
        ]
    ).astype(f32)[None, :]
    T2 = np.stack([np.ones(NUM, f32), t]).astype(f32)

    return {
        "PTp": np.ascontiguousarray(P.T.astype(f32)),
        "PdT": np.ascontiguousarray(Pd.T.astype(f32)),
        "PddT": np.ascontiguousarray(Pdd.T.astype(f32)),
        "Pdk": np.ascontiguousarray(Pd.astype(bf16)),
        "Pddk": np.ascontiguousarray(Pdd.astype(bf16)),
        "Pres": np.ascontiguousarray((-RO * P).astype(bf16)),
        "Plane": np.ascontiguousarray((RL * P).astype(bf16)),
        "J2x2": np.ascontiguousarray((2.0 * J2x).astype(f32)),
        "J2xn": np.ascontiguousarray((-J2x).astype(f32)),
        "WJx": np.ascontiguousarray((Wx @ J2x).astype(f32)),
        "J2y2": np.ascontiguousarray((2.0 * J2y).astype(f32)),
        "J2yn": np.ascontiguousarray((-J2y).astype(f32)),
        "WJy": np.ascontiguousarray((Wy @ J2y).astype(f32)),
        "Cx4": Cx4,
        "Cy4": Cy4,
        "Cx4J": np.ascontiguousarray((Cx4 @ J2x).astype(f32)),
        "Cy4J": np.ascontiguousarray((Cy4 @ J2y).astype(f32)),
        "I11": I,
        "T2": T2,
        "K4": K4,
        "w1": np.ascontiguousarray(np.asarray(inputs["enc_w1"], f32).astype(bf16)),
        "b1": np.ascontiguousarray(np.asarray(inputs["enc_b1"], f32)),
        "w2": np.ascontiguousarray(np.asarray(inputs["enc_w2"], f32).astype(bf16)),
        "b2": np.ascontiguousarray(np.asarray(inputs["enc_b2"], f32)),
        "wml": np.ascontiguousarray(
            np.concatenate(
                [np.asarray(inputs["enc_wmu"], f32), np.asarray(inputs["enc_wlv"], f32)],
                axis=1,
            ).astype(bf16)
        ),
        "bmu": np.ascontiguousarray(np.asarray(inputs["enc_bmu"], f32)),
        "blvh": np.ascontiguousarray((0.5 * np.asarray(inputs["enc_blv"], f32))),
        "dw1": np.ascontiguousarray(np.asarray(inputs["dec_w1"], f32).astype(bf16)),
        "db1": np.ascontiguousarray(np.asarray(inputs["dec_b1"], f32)),
        "dw2": np.ascontiguousarray(np.asarray(inputs["dec_w2"], f32).astype(bf16)),
        "db2": np.ascontiguousarray(np.asarray(inputs["dec_b2"], f32)),
        "dw3": np.ascontiguousarray(np.asarray(inputs["dec_w3"], f32).astype(bf16)),
        "db3": np.ascontiguousarray(np.asarray(inputs["dec_b3"], f32)),
    }


def prep_batch_global(inputs):
    """Batch-dependent inputs as concatenated (ncores*rows, cols) arrays."""
    inp = np.asarray(inputs["inp"], f32)
    B = inp.shape[0]

    inp_mean = np.asarray(inputs["inp_mean"], f32)
    inp_std = np.asarray(inputs["inp_std"], f32)
    inp_n = (inp - inp_mean) / inp_std
    enc_full = np.concatenate(
        [inp_n, np.asarray(inputs["traj_gt"], f32)], axis=1
    ).T.astype(bf16)                                         # (255, B)
    enc_in = np.ascontiguousarray(
        enc_full.reshape(255, NCORES, BPC).transpose(1, 0, 2).reshape(
            NCORES * 255, BPC
        )
    )

    def core_rows(a):
        # (r, B) -> (ncores*r, BPC) grouping batch columns per core
        r = a.shape[0]
        return np.ascontiguousarray(
            a.reshape(r, NCORES, BPC).transpose(1, 0, 2).reshape(NCORES * r, BPC)
        )

    eps_t = core_rows(np.asarray(inputs["eps"], f32).T)
    ise = np.asarray(inputs["initial_state_ego"], f32)
    # per-core single row: [vx0 (BPC) | vy0 (BPC)]
    vxy0 = np.ascontiguousarray(
        ise[:, 2:4].T.reshape(2, NCORES, BPC).transpose(1, 0, 2).reshape(
            NCORES, 2 * BPC
        )
    )
    ylu = np.ascontiguousarray(
        np.stack([np.asarray(inputs["y_lb"], f32), np.asarray(inputs["y_ub"], f32)])
        .reshape(2, NCORES, BPC).transpose(1, 0, 2).reshape(NCORES, 2 * BPC)
    )

    # obstacle states: rows (x, vx, y, vy), free = (stream, obstacle, b64)
    X4 = np.stack(
        [inp[:, 5::5], inp[:, 7::5], inp[:, 6::5], inp[:, 8::5]]
    )                                                        # (4, B, NOBS)
    obsst = np.ascontiguousarray(
        X4.reshape(4, NCORES, 2, B2, NOBS)
        .transpose(1, 0, 2, 4, 3)                            # (c, 4, s, o, b)
        .reshape(NCORES * 4, 2 * NB2)
    )
    return {
        "enc_in": enc_in,
        "eps_t": eps_t,
        "vxy0": vxy0,
        "ylu": ylu,
        "obsst": obsst,
    }


def host_prep(inputs):
    """Per-core input dicts (CoreSim / debugging path)."""
    consts = prep_consts(inputs)
    gb = prep_batch_global(inputs)
    in_maps = []
    for c in range(NCORES):
        m = dict(consts)
        for k, g in gb.items():
            r = g.shape[0] // NCORES
            m[k] = np.ascontiguousarray(g[c * r : (c + 1) * r])
        in_maps.append(m)
    return in_maps


# ---------------------------------------------------------------------------
# persistent runtime: jitted executable + device-resident constants
# ---------------------------------------------------------------------------

class _Runtime:
    def __init__(self):
        install_neuronx_cc_hook()
        nc = build_module()
        self.nc = nc
        assert nc.dbg_addr is None or not nc.dbg_callbacks

        partition_name = (
            nc.partition_id_tensor.name if nc.partition_id_tensor else None
        )
        in_names, out_names, out_avals, zero_shapes = [], [], [], []
        for alloc in nc.m.functions[0].allocations:
            if not isinstance(alloc, mybir.MemoryLocationSet):
                continue
            name = alloc.memorylocations[0].name
            if alloc.kind == "ExternalInput":
                if name != partition_name:
                    in_names.append(name)
            elif alloc.kind == "ExternalOutput":
                shape = tuple(alloc.tensor_shape)
                dtype = mybir.dt.np(alloc.dtype)
                out_avals.append(jax.core.ShapedArray(shape, dtype))
                zero_shapes.append((shape, dtype))
                out_names.append(name)
        self.in_names = in_names
        self.out_names = out_names
        self.zero_shapes = zero_shapes
        n_params = len(in_names)
        n_outs = len(out_names)
        in_names_full = in_names + out_names + (
            [partition_name] if partition_name else []
        )

        def _body(*args):
            operands = list(args)
            if partition_name is not None:
                operands.append(partition_id_tensor())
            outs = _bass_exec_p.bind(
                *operands,
                out_avals=tuple(out_avals),
                in_names=tuple(in_names_full),
                out_names=tuple(out_names),
                lowering_input_output_aliases=(),
                sim_require_finite=True,
                sim_require_nnan=True,
                nc=nc,
            )
            return tuple(outs)

        devices = jax.devices()[:NCORES]
        assert len(devices) == NCORES
        self.mesh = Mesh(np.asarray(devices), ("core",))
        self.sharding = NamedSharding(self.mesh, PartitionSpec("core"))
        donate = tuple(range(n_params, n_params + n_outs))
        self.fn = jax.jit(
            shard_map(
                _body,
                mesh=self.mesh,
                in_specs=(PartitionSpec("core"),) * (n_params + n_outs),
                out_specs=(PartitionSpec("core"),) * n_outs,
                check_rep=False,
            ),
            donate_argnums=donate,
            keep_unused=True,
        )

        self.const_ids = None
        self.const_crc = None
        self.dev_consts = None

    def _const_key_fast(self, inputs):
        return tuple(id(inputs[k]) for k in _CONST_SRC)

    def _const_crc(self, inputs):
        h = 0
        for k in _CONST_SRC:
            a = np.ascontiguousarray(np.asarray(inputs[k]))
            h = zlib.crc32(a, h)
        return h

    def get_consts(self, inputs):
        ids = self._const_key_fast(inputs)
        if self.dev_consts is not None and ids == self.const_ids:
            return self.dev_consts
        crc = self._const_crc(inputs)
        if self.dev_consts is not None and crc == self.const_crc:
            self.const_ids = ids
            return self.dev_consts
        consts = prep_consts(inputs)
        dev = {}
        for name, a in consts.items():
            g = np.ascontiguousarray(
                np.broadcast_to(a, (NCORES,) + a.shape).reshape(
                    NCORES * a.shape[0], *a.shape[1:]
                )
            )
            dev[name] = jax.device_put(g, self.sharding)
        for v in dev.values():
            v.block_until_ready()
        self.const_ids = ids
        self.const_crc = crc
        self.dev_consts = dev
        return dev


_RT = None


def _run_once(rt, inputs):
    dev_consts = rt.get_consts(inputs)
    batch = prep_batch_global(inputs)
    args = [
        dev_consts[n] if n in dev_consts else batch[n] for n in rt.in_names
    ]
    zeros = [np.zeros((NCORES * s[0], *s[1:]), d) for s, d in rt.zero_shapes]
    outs = rt.fn(*args, *zeros)
    out = outs[0]
    try:
        out.copy_to_host_async()
    except Exception:
        pass
    a = np.asarray(out)                                  # (8*22, BPC)
    # rows per core: [h=0 (x) vars 0..10, h=1 (y) vars 0..10]; cols = batch
    res = a.reshape(NCORES, 2 * NVAR, BPC).transpose(0, 2, 1).reshape(
        NCORES * BPC, 2 * NVAR
    )
    return np.ascontiguousarray(res)


def kernel(**inputs):
    global _RT
    first = _RT is None
    if first:
        _RT = _Runtime()
    if first:
        # warm the dispatch fastpath / remote buffer pools on the cold call
        _run_once(_RT, inputs)
    return _run_once(_RT, inputs)


# revision 32
# speedup vs baseline: 29.7605x; 1.0273x over previous
"""Trainium2 Bass kernel for nn_Beta_cVAE (cVAE encoder/decoder + 2-layer QP
trajectory projection, 20 ADMM iterations).

Strategy: pure data-parallel over 8 NeuronCores (128 batch rows each).
On-device layout is fully transposed: batch on the free dim, time/NVAR on
partitions, so every matmul uses weights/basis as the stationary operand and
no transposes appear anywhere.  All arctan2/cos/sin of the reference are
eliminated algebraically (cos(atan2(a,b)) = b/hypot(a,b)): the obstacle
ellipse projection becomes a residual scaling  resid = w*min(r-AB,0)/r  that
is *exactly zero* for inactive obstacles (no large-term cancellation, which
makes the bf16 obstacle pipeline safe), velocity/accel projections become
clip-scalings, and the lane term reduces to y - clip(y, lb, ub).  Obstacle
sums over the 10 obstacles fold into PE matmul accumulations with a shared
stationary operand.  MLP runs in bf16 (fp32 PSUM accumulation).

Host/runtime side: the expensive parts of a call (jit trace/lower, NEFF
load, 44 MB weight upload over the axon tunnel) are all cached across calls:
the jitted shard_map executable is built once, the replicated weights /
QP constants are device-resident keyed by a crc32 of the weight arrays, and
per-call traffic is only the ~0.8 MB of batch-dependent inputs (encoder
input, eps, initial velocities, lane bounds, obstacle states — obstacle
*trajectories* and boundary-condition expansions are computed on device).
The output is fetched with copy_to_host_async so the D2H ride shares the
dispatch round-trip.
"""
import sys
import os
import zlib

sys.path.insert(0, "/opt/trn_rl_repo")

import numpy as np
import ml_dtypes
import jax

import concourse.bass as bass
import concourse.mybir as mybir
from concourse.tile import TileContext, ScopedClock
from concourse.bass2jax import (
    _bass_exec_p,
    install_neuronx_cc_hook,
    partition_id_tensor,
)
from jax.experimental.shard_map import shard_map
from jax.sharding import Mesh, NamedSharding, PartitionSpec

f32 = np.float32
bf16 = ml_dtypes.bfloat16
FP = mybir.dt.float32
FR = mybir.dt.float32r
BF = mybir.dt.bfloat16
AF = mybir.ActivationFunctionType
OP = mybir.AluOpType

NCORES = 8
BPC = 128          # batch rows per core
NUM, NVAR, NOBS = 100, 11, 10
NB = NOBS * BPC    # 1280
B2 = BPC // 2      # 64 (per interleaved stream)
NB2 = NOBS * B2    # 640
A_OBS, B_OBS = 8.0, 4.2
AB = A_OBS * B_OBS
RO, RI, RL, RP = 100.0, 100.0, 100.0, 1.0
VMIN, VMAX, AMAX = 0.1, 30.0, 8.0
MAXITER = 20
HID = 1024
T_FIN = 15.0


class _TC(TileContext):
    """TileContext whose tail drain splits sem waits: this walrus build's
    TPB_CTRL codegen accepts only one sync wait per instruction."""

    def _drain_and_barrier(self, tick_clock, wait_clock):
        drain_inst = self.nc.sync.drain()
        wait_clock.add_sem_waits(
            drain_inst.ins, ScopedClock({None: tick_clock.global_clock})
        )
        si = drain_inst.ins.sync_info
        waits = list(si.on_wait) if si is not None else []
        if len(waits) > 1:
            drain_inst.ins.sync_info = mybir.SyncInfo(
                on_wait=waits[:1], on_update=list(si.on_update)
            )
            for w in waits[1:]:
                nop = self.nc.sync.nop(nofuse=True, hint="split_drain_wait")
                nop.ins.sync_info = mybir.SyncInfo(on_wait=[w], on_update=[])

        self.nc.all_engine_barrier()
        assert self.sems is not None
        popped = self.nc._tile_sem_poison_stack.pop()
        assert popped is self._sem_poison
        self.nc.clear_and_free_semaphores(list(self.sems.allocated().values()))
        self.nc.all_engine_barrier()


def _split_sync_waits(nc, max_waits=1):
    """Walrus's CoreV3 codegen accepts few sync waits per instruction; hoist
    surplus waits onto same-engine nops inserted just before the offender."""
    ctr = [0]
    for f in nc.m.functions:
        for bb in f.blocks:
            insts = list(bb.instructions)
            out = []
            for inst in insts:
                si = getattr(inst, "sync_info", None)
                if si is not None and len(si.on_wait) > max_waits:
                    waits = list(si.on_wait)
                    keep = waits[-max_waits:]
                    rest = waits[: len(waits) - max_waits]
                    for i in range(0, len(rest), max_waits):
                        ctr[0] += 1
                        nop = mybir.InstNoOp(
                            name=f"swsplit_{ctr[0]}",
                            engine=inst.engine,
                            bass_nofuse=True,
                            sync_info=mybir.SyncInfo(
                                on_wait=rest[i : i + max_waits], on_update=[]
                            ),
                        )
                        out.append(nop)
                    inst.sync_info = mybir.SyncInfo(
                        on_wait=keep, on_update=list(si.on_update)
                    )
                out.append(inst)
            bb.instructions = out


# ---------------------------------------------------------------------------
# module construction
# ---------------------------------------------------------------------------

def build_module(maxiter=MAXITER, split_waits=True):
    nc = bass.Bass()
    dp = nc.declare_dram_parameter

    # ---- per-core batch-dependent inputs (small; uploaded every call) ----
    enc_in = dp("enc_in", [255, BPC], BF, isOutput=False)
    eps_t = dp("eps_t", [2, BPC], FP, isOutput=False)
    vxy0 = dp("vxy0", [1, 2 * BPC], FP, isOutput=False)  # [vx0 | vy0]
    ylu = dp("ylu", [1, 2 * BPC], FP, isOutput=False)    # [y_lb | y_ub]
    obsst = dp("obsst", [4, 2 * NB2], FP, isOutput=False)  # rows x,vx,y,vy; free=(s,o,b)

    # ---- replicated weights (bf16; device-cached across calls) ----
    w1 = dp("w1", [255, HID], BF, isOutput=False)
    b1 = dp("b1", [HID], FP, isOutput=False)
    w2 = dp("w2", [HID, HID], BF, isOutput=False)
    b2 = dp("b2", [HID], FP, isOutput=False)
    wml = dp("wml", [HID, 4], BF, isOutput=False)
    bmu = dp("bmu", [2], FP, isOutput=False)
    blvh = dp("blvh", [2], FP, isOutput=False)
    dw1 = dp("dw1", [57, HID], BF, isOutput=False)
    db1 = dp("db1", [HID], FP, isOutput=False)
    dw2 = dp("dw2", [HID, HID], BF, isOutput=False)
    db2 = dp("db2", [HID], FP, isOutput=False)
    dw3 = dp("dw3", [HID, 8], BF, isOutput=False)
    db3 = dp("db3", [8], FP, isOutput=False)

    # ---- replicated QP constants ----
    PTp = dp("PTp", [NVAR, NUM], FP, isOutput=False)          # P.T
    PdT = dp("PdT", [NVAR, NUM], FP, isOutput=False)
    PddT = dp("PddT", [NVAR, NUM], FP, isOutput=False)
    Pdk = dp("Pdk", [NUM, NVAR], BF, isOutput=False)
    Pddk = dp("Pddk", [NUM, NVAR], BF, isOutput=False)
    Pres = dp("Pres", [NUM, NVAR], BF, isOutput=False)        # -RO*P
    Plane = dp("Plane", [NUM, NVAR], BF, isOutput=False)      # RL*P
    J2x2 = dp("J2x2", [NVAR, NVAR], FP, isOutput=False)
    J2xn = dp("J2xn", [NVAR, NVAR], FP, isOutput=False)
    WJx = dp("WJx", [NVAR, NVAR], FP, isOutput=False)
    J2y2 = dp("J2y2", [NVAR, NVAR], FP, isOutput=False)
    J2yn = dp("J2yn", [NVAR, NVAR], FP, isOutput=False)
    WJy = dp("WJy", [NVAR, NVAR], FP, isOutput=False)
    Cx4 = dp("Cx4", [4, NVAR], FP, isOutput=False)
    Cy4 = dp("Cy4", [4, NVAR], FP, isOutput=False)
    Cx4J = dp("Cx4J", [4, NVAR], FP, isOutput=False)
    Cy4J = dp("Cy4J", [4, NVAR], FP, isOutput=False)
    I11 = dp("I11", [NVAR, NVAR], FP, isOutput=False)
    T2 = dp("T2", [2, NUM], FP, isOutput=False)               # rows: ones, t
    K4 = dp("K4", [1, 4 * NVAR], FP, isOutput=False)          # beq rank-1 rows

    out_t = dp("out_t", [2 * NVAR, BPC], FP, isOutput=True)

    mm = nc.tensor.matmul

    lp = nc.allow_low_precision(reason="bf16 obstacle chain is within tolerance")
    lp.__enter__()
    with _TC(nc) as tc:
        cst = tc.alloc_tile_pool(name="cst", bufs=1)
        act = tc.alloc_tile_pool(name="act", bufs=2)
        lam = tc.alloc_tile_pool(name="lam", bufs=3)
        psp = tc.alloc_tile_pool(name="ps", bufs=3, space="PSUM")

        dmae = [nc.sync, nc.gpsimd, nc.scalar]

        def cload(ap, shape, dtype=FP, tag=None, q=0):
            t = cst.tile(shape, dtype, tag=tag or ap.tensor.name)
            dmae[q % len(dmae)].dma_start(out=t[:], in_=ap)
            return t

        # ---------- constants into SBUF ----------
        sb_PTp = cload(PTp[:], [NVAR, NUM])
        sb_PdT = cload(PdT[:], [NVAR, NUM])
        sb_PddT = cload(PddT[:], [NVAR, NUM])
        sb_Pdk = cload(Pdk[:], [NUM, NVAR], BF)
        sb_Pddk = cload(Pddk[:], [NUM, NVAR], BF)
        sb_Pres = cload(Pres[:], [NUM, NVAR], BF)
        sb_Plane = cload(Plane[:], [NUM, NVAR], BF)
        sb_J2x2 = cload(J2x2[:], [NVAR, NVAR])
        sb_J2xn = cload(J2xn[:], [NVAR, NVAR])
        sb_WJx = cload(WJx[:], [NVAR, NVAR])
        sb_J2y2 = cload(J2y2[:], [NVAR, NVAR])
        sb_J2yn = cload(J2yn[:], [NVAR, NVAR])
        sb_WJy = cload(WJy[:], [NVAR, NVAR])
        sb_Cx4 = cload(Cx4[:], [4, NVAR])
        sb_Cy4 = cload(Cy4[:], [4, NVAR])
        sb_Cx4J = cload(Cx4J[:], [4, NVAR])
        sb_Cy4J = cload(Cy4J[:], [4, NVAR])
        sb_I11 = cload(I11[:], [NVAR, NVAR])
        sb_T2 = cload(T2[:], [2, NUM])
        sb_K4 = cload(K4[:], [1, 4 * NVAR])
        sb_vxy0 = cload(vxy0[:], [1, 2 * BPC], q=1)
        sb_ylu = cload(ylu[:], [1, 2 * BPC], q=1)
        sb_obsx = cload(obsst[0:2, :], [2, 2 * NB2], tag="obsx", q=2)
        sb_obsy = cload(obsst[2:4, :], [2, 2 * NB2], tag="obsy", q=2)

        # ---------- MLP weights into SBUF (spread across DMA queues) ----------
        sb_w1a = cload(w1[0:128, :], [128, HID], BF, tag="w1a", q=1)
        sb_w1b = cload(w1[128:255, :], [127, HID], BF, tag="w1b", q=2)
        sb_w2 = cst.tile([128, 8 * HID], BF, tag="w2")
        for kc in range(8):
            dmae[kc % 3].dma_start(
                out=sb_w2[:, kc * HID : (kc + 1) * HID],
                in_=w2[kc * 128 : (kc + 1) * 128, :],
            )
        sb_wml = cst.tile([128, 8 * 4], BF, tag="wml")
        for kc in range(8):
            nc.sync.dma_start(
                out=sb_wml[:, kc * 4 : (kc + 1) * 4],
                in_=wml[kc * 128 : (kc + 1) * 128, :],
            )
        sb_dw1 = cload(dw1[:], [57, HID], BF, q=3)
        sb_dw2 = cst.tile([128, 8 * HID], BF, tag="dw2")
        for kc in range(8):
            dmae[kc % 3].dma_start(
                out=sb_dw2[:, kc * HID : (kc + 1) * HID],
                in_=dw2[kc * 128 : (kc + 1) * 128, :],
            )
        sb_dw3 = cst.tile([128, 8 * 8], BF, tag="dw3")
        for kc in range(8):
            nc.gpsimd.dma_start(
                out=sb_dw3[:, kc * 8 : (kc + 1) * 8],
                in_=dw3[kc * 128 : (kc + 1) * 128, :],
            )
        sb_b1 = cload(b1[:].rearrange("(c p) -> c p", p=128).rearrange("c p -> p c"), [128, 8], q=3)
        sb_b2 = cload(b2[:].rearrange("(c p) -> c p", p=128).rearrange("c p -> p c"), [128, 8], q=3)
        sb_db1 = cload(db1[:].rearrange("(c p) -> c p", p=128).rearrange("c p -> p c"), [128, 8], q=3)
        sb_db2 = cload(db2[:].rearrange("(c p) -> c p", p=128).rearrange("c p -> p c"), [128, 8], q=3)
        sb_bmu = cload(bmu[:, None], [2, 1], q=3)
        sb_blvh = cload(blvh[:, None], [2, 1], q=3)
        sb_db3v = cload(db3[0:4][:, None], [4, 1], tag="db3v", q=3)
        sb_db3y = cload(db3[4:8][:, None], [4, 1], tag="db3y", q=3)

        sb_x1a = cload(enc_in[0:128, :], [128, BPC], BF, tag="x1a", q=1)
        sb_x1b = cload(enc_in[128:255, :], [127, BPC], BF, tag="x1b", q=1)
        sb_eps = cload(eps_t[:], [2, BPC], q=1)

        # ---------- obstacle trajectories + lane bounds on device ----------
        # xo[t, (s,o,b)] = x + t*vx  via K=2 matmul with stationary [ones; t]
        sb_xyobs = cst.tile([NUM, 2 * NB], BF, tag="xyobs")
        H2 = NB2 // 2  # 320, fits the psum_a slot
        for s in (0, 1):
            for a in (0, 1):
                for h in (0, 1):
                    ps = psp.tile([NUM, H2], FP, tag="psum_a", bufs=4,
                                  name=f"obs{s}{a}{h}")
                    src = sb_obsx if a == 0 else sb_obsy
                    mov = src[:, s * NB2 + h * H2 : s * NB2 + (h + 1) * H2]
                    mm(ps[:], sb_T2[:], mov, start=True, stop=True)
                    dst = (2 * s + a) * NB2 + h * H2
                    nc.scalar.activation(
                        sb_xyobs[:, dst : dst + H2], ps[:], AF.Copy
                    )
        # lane bounds broadcast along partitions: ones[1,100]^T @ lb[1,B]
        lane_ps = psp.tile([NUM, 2 * BPC], FP, tag="psum_b", bufs=4, name="lane_ps")
        mm(lane_ps[:, :BPC], sb_T2[0:1, :], sb_ylu[0:1, :BPC], start=True, stop=True,
           skip_group_check=True)
        mm(lane_ps[:, BPC:], sb_T2[0:1, :], sb_ylu[0:1, BPC:], start=True, stop=True,
           skip_group_check=True)
        sb_ylbc = cst.tile([NUM, BPC], BF, tag="ylbc")
        nc.scalar.activation(sb_ylbc[:], lane_ps[:, :BPC], AF.Copy)
        sb_yubc = cst.tile([NUM, BPC], BF, tag="yubc")
        nc.scalar.activation(sb_yubc[:], lane_ps[:, BPC:], AF.Copy)

        # ---------- MLP (bf16 weights/acts, fp32 PSUM) ----------
        h1 = act.tile([128, HID], BF, tag="h1", bufs=1)
        for jc in range(8):
            ps = psp.tile([128, BPC], FP, tag="psum_a", bufs=4)
            js = slice(jc * 128, (jc + 1) * 128)
            mm(ps[:], sb_w1a[:, js], sb_x1a[:], start=True, stop=False)
            mm(ps[:], sb_w1b[:, js], sb_x1b[:], start=False, stop=True)
            nc.scalar.activation(h1[:, js], ps[:], AF.Relu, bias=sb_b1[:, jc : jc + 1])
        h2 = act.tile([128, HID], BF, tag="h2", bufs=1)
        for jc in range(8):
            ps = psp.tile([128, BPC], FP, tag="psum_a", bufs=4)
            js = slice(jc * 128, (jc + 1) * 128)
            for kc in range(8):
                mm(
                    ps[:],
                    sb_w2[:, kc * HID + jc * 128 : kc * HID + (jc + 1) * 128],
                    h1[:, kc * 128 : (kc + 1) * 128],
                    start=(kc == 0),
                    stop=(kc == 7),
                )
            nc.scalar.activation(h2[:, js], ps[:], AF.Relu, bias=sb_b2[:, jc : jc + 1])
        mu_ps = psp.tile([2, BPC], FP, tag="psum_b", bufs=4)
        lv_ps = psp.tile([2, BPC], FP, tag="psum_b", bufs=4)
        for kc in range(8):
            mm(
                mu_ps[:],
                sb_wml[:, kc * 4 : kc * 4 + 2],
                h2[:, kc * 128 : (kc + 1) * 128],
                start=(kc == 0),
                stop=(kc == 7),
            )
            mm(
                lv_ps[:],
                sb_wml[:, kc * 4 + 2 : kc * 4 + 4],
                h2[:, kc * 128 : (kc + 1) * 128],
                start=(kc == 0),
                stop=(kc == 7),
            )
        # z = mu + bmu + exp(0.5*lv + 0.5*blv)*eps  -> rows 0:2 of dec_in (bf16)
        dec_in = act.tile([57, BPC], BF, tag="dec_in", bufs=1)
        nc.sync.dma_start(out=dec_in[2:57, :], in_=sb_x1a[0:55, :])
        std = act.tile([2, BPC], FP, tag="std", bufs=1)
        nc.scalar.activation(std[:], lv_ps[:], AF.Exp, bias=sb_blvh[:], scale=0.5)
        zz = act.tile([2, BPC], FP, tag="zz", bufs=1)
        nc.vector.tensor_mul(out=zz[:], in0=std[:], in1=sb_eps[:])
        zmu = act.tile([2, BPC], FP, tag="zmu", bufs=1)
        nc.vector.tensor_add(out=zmu[:], in0=zz[:], in1=mu_ps[:])
        nc.vector.tensor_scalar(
            out=dec_in[0:2, :], in0=zmu[:], scalar1=sb_bmu[:], scalar2=None,
            op0=OP.add,
        )
        g1 = act.tile([128, HID], BF, tag="g1", bufs=1)
        for jc in range(8):
            ps = psp.tile([128, BPC], FP, tag="psum_a", bufs=4)
            js = slice(jc * 128, (jc + 1) * 128)
            mm(ps[:], sb_dw1[:, js], dec_in[:], start=True, stop=True)
            nc.scalar.activation(g1[:, js], ps[:], AF.Relu, bias=sb_db1[:, jc : jc + 1])
        g2 = act.tile([128, HID], BF, tag="g2", bufs=1)
        for jc in range(8):
            ps = psp.tile([128, BPC], FP, tag="psum_a", bufs=4)
            js = slice(jc * 128, (jc + 1) * 128)
            for kc in range(8):
                mm(
                    ps[:],
                    sb_dw2[:, kc * HID + jc * 128 : kc * HID + (jc + 1) * 128],
                    g1[:, kc * 128 : (kc + 1) * 128],
                    start=(kc == 0),
                    stop=(kc == 7),
                )
            nc.scalar.activation(g2[:, js], ps[:], AF.Relu, bias=sb_db2[:, jc : jc + 1])
        nnv_ps = psp.tile([4, BPC], FP, tag="psum_b", bufs=4)
        nny_ps = psp.tile([4, BPC], FP, tag="psum_b", bufs=4)
        for kc in range(8):
            mm(
                nnv_ps[:],
                sb_dw3[:, kc * 8 : kc * 8 + 4],
                g2[:, kc * 128 : (kc + 1) * 128],
                start=(kc == 0),
                stop=(kc == 7),
            )
            mm(
                nny_ps[:],
                sb_dw3[:, kc * 8 + 4 : kc * 8 + 8],
                g2[:, kc * 128 : (kc + 1) * 128],
                start=(kc == 0),
                stop=(kc == 7),
            )
        nn_v = act.tile([4, BPC], FP, tag="nn_v", bufs=1)
        nc.scalar.activation(nn_v[:], nnv_ps[:], AF.Identity, bias=sb_db3v[:])
        nn_y = act.tile([4, BPC], FP, tag="nn_y", bufs=1)
        nc.scalar.activation(nn_y[:], nny_ps[:], AF.Identity, bias=sb_db3y[:])

        # ---------- QP setup ----------
        # b_eq terms are rank-1 (only vx0/vy0 nonzero): K4 rows are the
        # corresponding KKT-inverse rows, applied as K=1 matmuls.
        st_ps = psp.tile([NVAR, 2 * BPC], FP, tag="psum_b", bufs=4)
        mm(st_ps[:, :BPC], sb_Cx4[:], nn_v[:], start=True, stop=False)
        mm(st_ps[:, :BPC], sb_K4[0:1, 0:NVAR], sb_vxy0[0:1, :BPC],
           start=False, stop=True)
        mm(st_ps[:, BPC:], sb_Cy4[:], nn_y[:], start=True, stop=False)
        mm(st_ps[:, BPC:], sb_K4[0:1, NVAR : 2 * NVAR], sb_vxy0[0:1, BPC:],
           start=False, stop=True)
        cxy = lam.tile([NVAR, 2 * BPC], FP, tag="cxy")
        nc.scalar.activation(cxy[:], st_ps[:], AF.Copy)

        sC_ps = psp.tile([NVAR, 2 * BPC], FP, tag="psum_b", bufs=4)
        mm(sC_ps[:, :BPC], sb_Cx4J[:], nn_v[:], start=True, stop=False)
        mm(sC_ps[:, :BPC], sb_K4[0:1, 2 * NVAR : 3 * NVAR], sb_vxy0[0:1, :BPC],
           start=False, stop=True)
        mm(sC_ps[:, BPC:], sb_Cy4J[:], nn_y[:], start=True, stop=False)
        mm(sC_ps[:, BPC:], sb_K4[0:1, 3 * NVAR : 4 * NVAR], sb_vxy0[0:1, BPC:],
           start=False, stop=True)
        CXC = cst.tile([NVAR, 2 * BPC], FP, tag="CXC")
        nc.scalar.activation(CXC[:], sC_ps[:], AF.Copy)

        sb_tiny = cst.tile([128, 1], FP, tag="tiny")
        nc.vector.memset(sb_tiny[:], 1e-30)
        sb_negone = cst.tile([128, 1], FP, tag="negone")
        nc.vector.memset(sb_negone[:], -1.0)

        # ---------- split into two independent 64-row streams ----------
        cxyS = []
        CXCS = []
        lxyS = []
        for s in (0, 1):
            cs = lam.tile([NVAR, 2 * B2], FP, tag=f"cxy{s}", name=f"cxyS{s}")
            nc.scalar.activation(
                cs[:].rearrange("p (a b) -> p a b", a=2),
                cxy[:].rearrange("p (a b) -> p a b", a=2)[:, :, s * B2 : (s + 1) * B2],
                AF.Copy,
            )
            cxyS.append(cs)
            xc = cst.tile([NVAR, 2 * B2], FP, tag=f"CXC{s}", name=f"CXCS{s}")
            nc.scalar.activation(
                xc[:].rearrange("p (a b) -> p a b", a=2),
                CXC[:].rearrange("p (a b) -> p a b", a=2)[:, :, s * B2 : (s + 1) * B2],
                AF.Copy,
            )
            CXCS.append(xc)
            lx = lam.tile([NVAR, 2 * B2], FP, tag=f"lxy{s}", name=f"lxyS{s}")
            nc.vector.memset(lx[:], 0.0)
            lxyS.append(lx)

        # ---------- QP iterations (two interleaved streams) ----------
        def qp_iter(s):
            cxy_s = cxyS[s]
            lxy_s = lxyS[s]
            obs = sb_xyobs[:, s * 2 * NB2 : (s + 1) * 2 * NB2]
            fwd = psp.tile([NUM, 6 * B2], FP, tag="psum_a", bufs=4, name=f"fwd{s}")
            xy = fwd[:, 0 : 2 * B2]
            xyd = fwd[:, 2 * B2 : 4 * B2]
            xydd = fwd[:, 4 * B2 : 6 * B2]
            mm(xy, sb_PTp[:], cxy_s[:], start=True, stop=True, skip_group_check=True)
            mm(xyd, sb_PdT[:], cxy_s[:], start=True, stop=True, skip_group_check=True)
            mm(xydd, sb_PddT[:], cxy_s[:], start=True, stop=True, skip_group_check=True)

            xys = act.tile([NUM, 2 * B2], BF, tag=f"xys{s}", name=f"xys_{s}")
            nc.scalar.activation(xys[:], xy, AF.Copy)
            w_s = act.tile([NUM, 2 * NB2], BF, tag=f"w_s{s}", name=f"w_s_{s}")
            nc.vector.tensor_tensor(
                out=w_s[:].rearrange("p (a o b) -> p a o b", a=2, o=NOBS),
                in0=xys[:]
                .rearrange("p (a b) -> p a b", a=2)[:, :, None, :]
                .to_broadcast((NUM, 2, NOBS, B2)),
                in1=obs.rearrange("p (a o b) -> p a o b", a=2, o=NOBS),
                op=OP.subtract,
            )

            sqw = act.tile([NUM, 2 * NB2], BF, tag=f"sqw{s}", name=f"sqw_{s}")
            nc.scalar.activation(sqw[:, :NB2], w_s[:, :NB2], AF.Square, scale=B_OBS)
            nc.scalar.activation(sqw[:, NB2:], w_s[:, NB2:], AF.Square, scale=A_OBS)
            sqd = act.tile([NUM, 2 * B2], BF, tag=f"sqd{s}", name=f"sqd_{s}")
            nc.scalar.activation(sqd[:], xyd, AF.Square)
            sqdd = act.tile([NUM, 2 * B2], BF, tag=f"sqdd{s}", name=f"sqdd_{s}")
            nc.scalar.activation(sqdd[:], xydd, AF.Square)

            r2all = act.tile([NUM, NB2 + 2 * B2], BF, tag=f"r2all{s}", name=f"r2all_{s}")
            nc.vector.tensor_add(out=r2all[:, :NB2], in0=sqw[:, :NB2], in1=sqw[:, NB2:])
            nc.vector.tensor_add(
                out=r2all[:, NB2 : NB2 + B2], in0=sqd[:, :B2], in1=sqd[:, B2:]
            )
            nc.vector.tensor_add(
                out=r2all[:, NB2 + B2 :], in0=sqdd[:, :B2], in1=sqdd[:, B2:]
            )
            r_all = act.tile([NUM, NB2 + 2 * B2], BF, tag=f"r_all{s}", name=f"r_all_{s}")
            nc.scalar.activation(r_all[:], r2all[:], AF.Sqrt, bias=sb_tiny[0:NUM, :])
            rinv = act.tile([NUM, NB2 + 2 * B2], BF, tag=f"rinv{s}", name=f"rinv_{s}")
            nc.vector.reciprocal(out=rinv[:], in_=r_all[:])

            qf = act.tile([NUM, NB2], BF, tag=f"qf{s}", name=f"qf_{s}")
            nc.scalar.activation(
                qf[:], rinv[:, :NB2], AF.Relu, bias=sb_negone[0:NUM, :], scale=AB
            )
            e_s = act.tile([NUM, 2 * NB2], BF, tag=f"e_s{s}", name=f"e_s_{s}")
            nc.vector.tensor_mul(out=e_s[:, :NB2], in0=qf[:], in1=w_s[:, :NB2])
            nc.vector.tensor_mul(out=e_s[:, NB2:], in0=qf[:], in1=w_s[:, NB2:])

            hva = act.tile([NUM, 2 * B2], FP, tag=f"hva{s}", name=f"hva_{s}")
            nc.vector.tensor_scalar(
                out=hva[:, :B2], in0=r_all[:, NB2 : NB2 + B2],
                scalar1=VMIN, scalar2=VMAX, op0=OP.max, op1=OP.min,
            )
            nc.vector.tensor_scalar(
                out=hva[:, B2:], in0=r_all[:, NB2 + B2 :],
                scalar1=AMAX, scalar2=None, op0=OP.min,
            )
            gva = act.tile([NUM, 2 * B2], FP, tag=f"gva{s}", name=f"gva_{s}")
            nc.vector.tensor_mul(out=gva[:], in0=hva[:], in1=rinv[:, NB2:])
            gvam = act.tile([NUM, 2 * B2], FP, tag=f"gvam{s}", name=f"gvam_{s}")
            nc.vector.tensor_scalar(
                out=gvam[:], in0=gva[:], scalar1=-RI, scalar2=RI,
                op0=OP.mult, op1=OP.add,
            )

            ys = xys[:, B2:]
            lt1 = act.tile([NUM, B2], BF, tag=f"lt1{s}", name=f"lt1_{s}")
            nc.vector.tensor_max(
                out=lt1[:], in0=ys, in1=sb_ylbc[:, s * B2 : (s + 1) * B2]
            )
            lt2 = act.tile([NUM, B2], BF, tag=f"lt2{s}", name=f"lt2_{s}")
            nc.vector.tensor_tensor(
                out=lt2[:], in0=lt1[:], in1=sb_yubc[:, s * B2 : (s + 1) * B2], op=OP.min
            )
            d12 = act.tile([NUM, B2], BF, tag=f"d12{s}", name=f"d12_{s}")
            nc.vector.tensor_sub(out=d12[:], in0=ys, in1=lt2[:])

            rvxy = act.tile([NUM, 2 * B2], BF, tag=f"rvxy{s}", name=f"rvxy_{s}")
            nc.vector.tensor_tensor(
                out=rvxy[:].rearrange("p (a b) -> p a b", a=2),
                in0=gvam[:, 0:B2][:, None, :].to_broadcast((NUM, 2, B2)),
                in1=xyd.rearrange("p (a b) -> p a b", a=2),
                op=OP.mult,
            )
            raxy = act.tile([NUM, 2 * B2], BF, tag=f"raxy{s}", name=f"raxy_{s}")
            nc.vector.tensor_tensor(
                out=raxy[:].rearrange("p (a b) -> p a b", a=2),
                in0=gvam[:, B2:][:, None, :].to_broadcast((NUM, 2, B2)),
                in1=xydd.rearrange("p (a b) -> p a b", a=2),
                op=OP.mult,
            )

            Dx = psp.tile([NVAR, B2], FP, tag="psum_b", bufs=4, name=f"Dx{s}")
            for o in range(NOBS):
                mm(
                    Dx[:],
                    sb_Pres[:],
                    e_s[:, o * B2 : (o + 1) * B2],
                    start=(o == 0),
                    stop=False,
                )
            mm(Dx[:], sb_Pddk[:], raxy[:, :B2], start=False, stop=False)
            mm(Dx[:], sb_Pdk[:], rvxy[:, :B2], start=False, stop=True)

            Dy = psp.tile([NVAR, B2], FP, tag="psum_b", bufs=4, name=f"Dy{s}")
            for o in range(NOBS):
                mm(
                    Dy[:],
                    sb_Pres[:],
                    e_s[:, NB2 + o * B2 : NB2 + (o + 1) * B2],
                    start=(o == 0),
                    stop=False,
                )
            mm(Dy[:], sb_Pddk[:], raxy[:, B2:], start=False, stop=False)
            mm(Dy[:], sb_Pdk[:], rvxy[:, B2:], start=False, stop=False)
            mm(Dy[:], sb_Plane[:], d12[:], start=False, stop=True)

            lxy_new = lam.tile([NVAR, 2 * B2], FP, tag=f"lxy{s}", name=f"lxyn_{s}")
            nc.vector.tensor_sub(out=lxy_new[:, :B2], in0=lxy_s[:, :B2], in1=Dx[:])
            nc.vector.tensor_sub(out=lxy_new[:, B2:], in0=lxy_s[:, B2:], in1=Dy[:])

            cn = psp.tile([NVAR, 2 * B2], FP, tag="psum_b", bufs=4, name=f"cn{s}")
            mm(cn[:, :B2], sb_J2x2[:], lxy_new[:, :B2], start=True, stop=False)
            mm(cn[:, :B2], sb_J2xn[:], lxy_s[:, :B2], start=False, stop=False)
            mm(cn[:, :B2], sb_WJx[:], cxy_s[:, :B2], start=False, stop=False)
            mm(cn[:, :B2], sb_I11[:], CXCS[s][:, :B2], start=False, stop=True)
            mm(cn[:, B2:], sb_J2y2[:], lxy_new[:, B2:], start=True, stop=False)
            mm(cn[:, B2:], sb_J2yn[:], lxy_s[:, B2:], start=False, stop=False)
            mm(cn[:, B2:], sb_WJy[:], cxy_s[:, B2:], start=False, stop=False)
            mm(cn[:, B2:], sb_I11[:], CXCS[s][:, B2:], start=False, stop=True)

            cxy_new = lam.tile([NVAR, 2 * B2], FP, tag=f"cxy{s}", name=f"cxyn_{s}")
            nc.scalar.activation(cxy_new[:], cn[:], AF.Copy)
            cxyS[s] = cxy_new
            lxyS[s] = lxy_new

        for it in range(maxiter):
            qp_iter(0)
            qp_iter(1)

        for s in (0, 1):
            nc.sync.dma_start(
                out=out_t[:].rearrange("(h k) b -> k h b", h=2)[
                    :, :, s * B2 : (s + 1) * B2
                ],
                in_=cxyS[s][:].rearrange("p (h b) -> p h b", h=2),
            )

        psp.release()
        lam.release()
        act.release()
        cst.release()

    lp.__exit__(None, None, None)
    if split_waits:
        _split_sync_waits(nc)
    return nc


# ---------------------------------------------------------------------------
# host preprocessing
# ---------------------------------------------------------------------------

# inputs whose values flow into the device-cached constant tensors
_CONST_SRC = (
    "P", "Pdot", "Pddot",
    "enc_w1", "enc_b1", "enc_w2", "enc_b2",
    "enc_wmu", "enc_bmu", "enc_wlv", "enc_blv",
    "dec_w1", "dec_b1", "dec_w2", "dec_b2", "dec_w3", "dec_b3",
)


def _kkt_inv(cost, A_eq):
    m = A_eq.shape[0]
    n = cost.shape[0]
    M = np.zeros((n + m, n + m), np.float64)
    M[:n, :n] = cost
    M[:n, n:] = A_eq.T
    M[n:, :n] = A_eq
    return np.linalg.inv(M).astype(f32)


def prep_consts(inputs):
    """Per-core (replicated) constant tensors, as numpy arrays."""
    P = np.asarray(inputs["P"], f32)
    Pd = np.asarray(inputs["Pdot"], f32)
    Pdd = np.asarray(inputs["Pddot"], f32)
    t = np.linspace(0.0, T_FIN, NUM).astype(f32)

    A_eq_x = np.stack([P[0], Pd[0], Pdd[0]])
    A_eq_y = np.stack([P[0], Pd[0], Pdd[0], Pd[-1]])
    K_D = f32(2.0 * np.sqrt(np.float32(20.0)))
    A_pd = Pdd - f32(20.0) * P - K_D * Pd
    A_vd = Pdd - f32(20.0) * Pd
    cs = Pdd.T @ Pdd
    inv1_x = _kkt_inv(cs + A_vd.T @ A_vd, A_eq_x)
    inv1_y = _kkt_inv(cs + A_pd.T @ A_pd, A_eq_y)
    PtP = P.T @ P
    PdTPd = Pd.T @ Pd
    PddTPdd = Pdd.T @ Pdd
    I = np.eye(NVAR, dtype=f32)
    cost2_x = RP * I + RO * 10.0 * PtP + RI * PddTPdd + RI * PdTPd
    cost2_y = cost2_x + RL * 2.0 * PtP
    inv2_x = _kkt_inv(cost2_x, A_eq_x)
    inv2_y = _kkt_inv(cost2_y, A_eq_y)

    J1x = inv1_x[:NVAR, :NVAR].T
    K1x = inv1_x[:NVAR, NVAR:].T
    J1y = inv1_y[:NVAR, :NVAR].T
    K1y = inv1_y[:NVAR, NVAR:].T
    J2x = inv2_x[:NVAR, :NVAR].T
    K2x = inv2_x[:NVAR, NVAR:].T
    J2y = inv2_y[:NVAR, :NVAR].T
    K2y = inv2_y[:NVAR, NVAR:].T

    Avd_bs = A_vd.reshape(4, 25, NVAR).sum(1)
    Apd_bs = A_pd.reshape(4, 25, NVAR).sum(1)
    Cx4 = (f32(-20.0) * (Avd_bs @ J1x)).astype(f32)
    Cy4 = (f32(-20.0) * (Apd_bs @ J1y)).astype(f32)
    Wx = (f32(10.0 * RO) * PtP + f32(RI) * (PddTPdd + PdTPd)).astype(f32)
    Wy = (Wx + f32(2.0 * RL) * PtP).astype(f32)

    # rank-1 b_eq rows: b_eq_x = [0, vx0, 0], b_eq_y = [0, vy0, 0, 0]
    K4 = np.concatenate(
        [
            K1x[1],
            K1y[1],
            (K1x @ J2x + K2x)[1],
            (K1y @ J2y + K2y)[1],
        ]
    ).astype(f32)[None, :]
    T2 = np.stack([np.ones(NUM, f32), t]).astype(f32)

    return {
        "PTp": np.ascontiguousarray(P.T.astype(f32)),
        "PdT": np.ascontiguousarray(Pd.T.astype(f32)),
        "PddT": np.ascontiguousarray(Pdd.T.astype(f32)),
        "Pdk": np.ascontiguousarray(Pd.astype(bf16)),
        "Pddk": np.ascontiguousarray(Pdd.astype(bf16)),
        "Pres": np.ascontiguousarray((-RO * P).astype(bf16)),
        "Plane": np.ascontiguousarray((RL * P).astype(bf16)),
        "J2x2": np.ascontiguousarray((2.0 * J2x).astype(f32)),
        "J2xn": np.ascontiguousarray((-J2x).astype(f32)),
        "WJx": np.ascontiguousarray((Wx @ J2x).astype(f32)),
        "J2y2": np.ascontiguousarray((2.0 * J2y).astype(f32)),
        "J2yn": np.ascontiguousarray((-J2y).astype(f32)),
        "WJy": np.ascontiguousarray((Wy @ J2y).astype(f32)),
        "Cx4": Cx4,
        "Cy4": Cy4,
        "Cx4J": np.ascontiguousarray((Cx4 @ J2x).astype(f32)),
        "Cy4J": np.ascontiguousarray((Cy4 @ J2y).astype(f32)),
        "I11": I,
        "T2": T2,
        "K4": K4,
        "w1": np.ascontiguousarray(np.asarray(inputs["enc_w1"], f32).astype(bf16)),
        "b1": np.ascontiguousarray(np.asarray(inputs["enc_b1"], f32)),
        "w2": np.ascontiguousarray(np.asarray(inputs["enc_w2"], f32).astype(bf16)),
        "b2": np.ascontiguousarray(np.asarray(inputs["enc_b2"], f32)),
        "wml": np.ascontiguousarray(
            np.concatenate(
                [np.asarray(inputs["enc_wmu"], f32), np.asarray(inputs["enc_wlv"], f32)],
                axis=1,
            ).astype(bf16)
        ),
        "bmu": np.ascontiguousarray(np.asarray(inputs["enc_bmu"], f32)),
        "blvh": np.ascontiguousarray((0.5 * np.asarray(inputs["enc_blv"], f32))),
        "dw1": np.ascontiguousarray(np.asarray(inputs["dec_w1"], f32).astype(bf16)),
        "db1": np.ascontiguousarray(np.asarray(inputs["dec_b1"], f32)),
        "dw2": np.ascontiguousarray(np.asarray(inputs["dec_w2"], f32).astype(bf16)),
        "db2": np.ascontiguousarray(np.asarray(inputs["dec_b2"], f32)),
        "dw3": np.ascontiguousarray(np.asarray(inputs["dec_w3"], f32).astype(bf16)),
        "db3": np.ascontiguousarray(np.asarray(inputs["dec_b3"], f32)),
    }


def prep_batch_global(inputs):
    """Batch-dependent inputs as concatenated (ncores*rows, cols) arrays."""
    inp = np.asarray(inputs["inp"], f32)
    B = inp.shape[0]

    inp_mean = np.asarray(inputs["inp_mean"], f32)
    inp_std = np.asarray(inputs["inp_std"], f32)
    inp_n = (inp - inp_mean) / inp_std
    enc_full = np.concatenate(
        [inp_n, np.asarray(inputs["traj_gt"], f32)], axis=1
    ).T.astype(bf16)                                         # (255, B)
    enc_in = np.ascontiguousarray(
        enc_full.reshape(255, NCORES, BPC).transpose(1, 0, 2).reshape(
            NCORES * 255, BPC
        )
    )

    def core_rows(a):
        # (r, B) -> (ncores*r, BPC) grouping batch columns per core
        r = a.shape[0]
        return np.ascontiguousarray(
            a.reshape(r, NCORES, BPC).transpose(1, 0, 2).reshape(NCORES * r, BPC)
        )

    eps_t = core_rows(np.asarray(inputs["eps"], f32).T)
    ise = np.asarray(inputs["initial_state_ego"], f32)
    # per-core single row: [vx0 (BPC) | vy0 (BPC)]
    vxy0 = np.ascontiguousarray(
        ise[:, 2:4].T.reshape(2, NCORES, BPC).transpose(1, 0, 2).reshape(
            NCORES, 2 * BPC
        )
    )
    ylu = np.ascontiguousarray(
        np.stack([np.asarray(inputs["y_lb"], f32), np.asarray(inputs["y_ub"], f32)])
        .reshape(2, NCORES, BPC).transpose(1, 0, 2).reshape(NCORES, 2 * BPC)
    )

    # obstacle states: rows (x, vx, y, vy), free = (stream, obstacle, b64)
    X4 = np.stack(
        [inp[:, 5::5], inp[:, 7::5], inp[:, 6::5], inp[:, 8::5]]
    )                                                        # (4, B, NOBS)
    obsst = np.ascontiguousarray(
        X4.reshape(4, NCORES, 2, B2, NOBS)
        .transpose(1, 0, 2, 4, 3)                            # (c, 4, s, o, b)
        .reshape(NCORES * 4, 2 * NB2)
    )
    return {
        "enc_in": enc_in,
        "eps_t": eps_t,
        "vxy0": vxy0,
        "ylu": ylu,
        "obsst": obsst,
    }


def host_prep(inputs):
    """Per-core input dicts (CoreSim / debugging path)."""
    consts = prep_consts(inputs)
    gb = prep_batch_global(inputs)
    in_maps = []
    for c in range(NCORES):
        m = dict(consts)
        for k, g in gb.items():
            r = g.shape[0] // NCORES
            m[k] = np.ascontiguousarray(g[c * r : (c + 1) * r])
        in_maps.append(m)
    return in_maps


# ---------------------------------------------------------------------------
# persistent runtime: jitted executable + device-resident constants
# ---------------------------------------------------------------------------

class _Runtime:
    def __init__(self):
        install_neuronx_cc_hook()
        nc = build_module()
        self.nc = nc
        assert nc.dbg_addr is None or not nc.dbg_callbacks

        partition_name = (
            nc.partition_id_tensor.name if nc.partition_id_tensor else None
        )
        in_names, out_names, out_avals, zero_shapes = [], [], [], []
        for alloc in nc.m.functions[0].allocations:
            if not isinstance(alloc, mybir.MemoryLocationSet):
                continue
            name = alloc.memorylocations[0].name
            if alloc.kind == "ExternalInput":
                if name != partition_name:
                    in_names.append(name)
            elif alloc.kind == "ExternalOutput":
                shape = tuple(alloc.tensor_shape)
                dtype = mybir.dt.np(alloc.dtype)
                out_avals.append(jax.core.ShapedArray(shape, dtype))
                zero_shapes.append((shape, dtype))
                out_names.append(name)
        self.in_names = in_names
        self.out_names = out_names
        self.zero_shapes = zero_shapes
        n_params = len(in_names)
        n_outs = len(out_names)
        in_names_full = in_names + out_names + (
            [partition_name] if partition_name else []
        )

        def _body(*args):
            operands = list(args)
            if partition_name is not None:
                operands.append(partition_id_tensor())
            outs = _bass_exec_p.bind(
                *operands,
                out_avals=tuple(out_avals),
                in_names=tuple(in_names_full),
                out_names=tuple(out_names),
                lowering_input_output_aliases=(),
                sim_require_finite=True,
                sim_require_nnan=True,
                nc=nc,
            )
            return tuple(outs)

        devices = jax.devices()[:NCORES]
        assert len(devices) == NCORES
        self.mesh = Mesh(np.asarray(devices), ("core",))
        self.sharding = NamedSharding(self.mesh, PartitionSpec("core"))
        donate = tuple(range(n_params, n_params + n_outs))
        self.fn = jax.jit(
            shard_map(
                _body,
                mesh=self.mesh,
                in_specs=(PartitionSpec("core"),) * (n_params + n_outs),
                out_specs=(PartitionSpec("core"),) * n_outs,
                check_rep=False,
            ),
            donate_argnums=donate,
            keep_unused=True,
        )

        self.const_ids = None
        self.const_crc = None
        self.dev_consts = None

    def _const_key_fast(self, inputs):
        return tuple(id(inputs[k]) for k in _CONST_SRC)

    def _const_crc(self, inputs):
        h = 0
        for k in _CONST_SRC:
            a = np.ascontiguousarray(np.asarray(inputs[k]))
            h = zlib.crc32(a, h)
        return h

    def get_consts(self, inputs):
        ids = self._const_key_fast(inputs)
        if self.dev_consts is not None and ids == self.const_ids:
            return self.dev_consts
        crc = self._const_crc(inputs)
        if self.dev_consts is not None and crc == self.const_crc:
            self.const_ids = ids
            return self.dev_consts
        consts = prep_consts(inputs)
        dev = {}
        for name, a in consts.items():
            g = np.ascontiguousarray(
                np.broadcast_to(a, (NCORES,) + a.shape).reshape(
                    NCORES * a.shape[0], *a.shape[1:]
                )
            )
            dev[name] = jax.device_put(g, self.sharding)
        for v in dev.values():
            v.block_until_ready()
        self.const_ids = ids
        self.const_crc = crc
        self.dev_consts = dev
        return dev


_RT = None


def _run_once(rt, inputs):
    dev_consts = rt.get_consts(inputs)
    batch = prep_batch_global(inputs)
    args = [
        dev_consts[n] if n in dev_consts else batch[n] for n in rt.in_names
    ]
    zeros = [np.zeros((NCORES * s[0], *s[1:]), d) for s, d in rt.zero_shapes]
    outs = rt.fn(*args, *zeros)
    out = outs[0]
    try:
        out.copy_to_host_async()
    except Exception:
        pass
    a = np.asarray(out)                                  # (8*22, BPC)
    # rows per core: [h=0 (x) vars 0..10, h=1 (y) vars 0..10]; cols = batch
    res = a.reshape(NCORES, 2 * NVAR, BPC).transpose(0, 2, 1).reshape(
        NCORES * BPC, 2 * NVAR
    )
    return np.ascontiguousarray(res)


def kernel(**inputs):
    global _RT
    first = _RT is None
    if first:
        _RT = _Runtime()
    if first:
        # warm the dispatch fastpath / remote buffer pools on the cold call
        _run_once(_RT, inputs)
    return _run_once(_RT, inputs)
